# revision 1
# baseline (speedup 1.0000x reference)
"""Bass kernel builder for nn_MixtureOfMambaBlock — 8-core SPMD.

Sharding: tokens 8-way (512/core + 128 halo for conv+scan warmup); mixer fully
local per core (weights replicated). Post-mixer h2 all-gathered (bf16), MoE
expert(4) x hid-half(2) sharded, weighted partials reduce-scattered back to
token shards.
"""
import numpy as np
import concourse.bass as bass
import concourse.bacc as bacc
import concourse.mybir as mybir
import concourse.tile as tile

FP = mybir.dt.float32
FR = mybir.dt.float32r
BF = mybir.dt.bfloat16
AF = mybir.ActivationFunctionType
ALU = mybir.AluOpType

B, T, D = 2, 2048, 1024
S, INNER = 64, 2048
E, HH = 4, 2048          # experts, hid-half width
OWN, HALO = 512, 128
NH = OWN + HALO          # 640
KB = D // 128            # 8  d-blocks
MB = INNER // 128        # 16 inner-blocks
OTB = OWN // 128         # 4  own-token blocks
N_CORES = 8

INPUT_SPECS = {
    "x_sh": ([NH, D], FP),
    "ipw": ([D, 2 * INNER], FR), "ipb": ([2 * INNER], FP),
    "cw": ([INNER, 3], FP), "cb": ([INNER], FP),
    "dtw": ([INNER, S], FR), "dtb": ([S], FP),
    "bpw": ([INNER, S], FR), "bpb": ([S], FP),
    "cpw": ([INNER, S], FR), "cpb": ([S], FP),
    "s2iw": ([S, INNER], FR), "s2ib": ([INNER], FP),
    "Dp": ([INNER], FP),
    "ow": ([INNER, D], FR), "ob": ([D], FR),
    "gw": ([D, E], FP), "gb": ([E], FR),
    "ew1": ([D, 2 * HH], BF), "eb1": ([2 * HH], FP),
    "ew2": ([2 * HH, D], BF), "eb2h": ([D], FR),
    "esel": ([128, E], FP),
    "rmask": ([128, 4], FP),
    "ident": ([128, 128], FP),
    "ones1": ([1, 128], FR),
}


def build(debug_outputs=False):
    nc = bacc.Bacc("TRN2", target_bir_lowering=False, debug=False,
                   num_devices=N_CORES)
    dp = {}
    for name, (shape, dt) in INPUT_SPECS.items():
        dp[name] = nc.dram_tensor(name, shape, dt, kind="ExternalInput")
    out_d = nc.dram_tensor("out", [OWN, D], FP, kind="ExternalOutput")
    dbg = {}
    if debug_outputs:
        dbg["xmid"] = nc.dram_tensor("dbg_xmid", [OWN, D], FP, kind="ExternalOutput")
        dbg["h2T"] = nc.dram_tensor("dbg_h2T", [D, OWN], FP, kind="ExternalOutput")
        dbg["wown"] = nc.dram_tensor("dbg_wown", [OWN, E], FP, kind="ExternalOutput")

    rg = [[0, 2, 4, 6], [1, 3, 5, 7]]
    GRP = 4

    with tile.TileContext(nc) as tc:
        with (
            tc.tile_pool(name="outer", bufs=1) as po,
            tc.tile_pool(name="dram", bufs=1, space="DRAM") as pdram,
        ):
            # ---------- DRAM bounce buffers for collectives ----------
            gth_in = [pdram.tile([D, 128], BF, name=f"gth_in{t_}") for t_ in range(OTB)]
            gth_out = [pdram.tile([4 * D, 128], BF, name=f"gth_out{t_}")
                       for t_ in range(OTB)]
            gtw_in = pdram.tile([OWN, E], FP)
            gtw_out = pdram.tile([4 * OWN, E], FP)
            rs_in = [pdram.tile([OWN, D], FP, name=f"rs_in{r}") for r in range(4)]
            rs_out = [pdram.tile([128, D], FP, name=f"rs_out{r}") for r in range(4)]

            # ---------- constants / small weights ----------
            ident = po.tile([128, 128], FP)
            nc.sync.dma_start(ident[:], dp["ident"][:])



            def load_pcol(name, n, blocks):  # [n*128] -> [128, blocks] (col b = block b)
                t = po.tile([128, blocks], FP, name=f"{name}_sb")
                nc.sync.dma_start(
                    t[:], dp[name].ap().rearrange("(m p) -> p m", p=128))
                return t



            def load_vec1(name, n):  # [n] -> [n, 1]
                t = po.tile([n, 1], FP, name=f"{name}_sb")
                nc.sync.dma_start(t[:], dp[name].ap().rearrange("(s o) -> s o", o=1))
                return t


            def load_row(name, n, dt_=FP):  # [n] -> [1, n]
                t = po.tile([1, n], dt_, name=f"{name}_sb")
                nc.sync.dma_start(t[:], dp[name].ap().rearrange("(o s) -> o s", o=1))
                return t
            ob_sb = load_row("ob", D, FR)
            gb_sb = load_row("gb", E, FR)
            eb2h_sb = load_row("eb2h", D, FR)

            def load_kw(name):  # [2048, 64] -> [128, 16, 64], lhsT slice [:, kb, :]
                t = po.tile([128, MB, S], FR, name=f"{name}_sb")
                nc.sync.dma_start(t[:], dp[name].ap().rearrange("(kb p) s -> p kb s", p=128))
                return t


            # persistent activations
            xo = [po.tile([128, D], FP, name=f"xo{t_}", tag=f"xo{t_}") for t_ in range(OTB)]
            xmid = [po.tile([128, D], FP, name=f"xmid{t_}", tag=f"xmid{t_}") for t_ in range(OTB)]

            # =======================================================
            # MIXER
            # =======================================================
            with (
                tc.tile_pool(name="mixer", bufs=1) as pm,
                tc.tile_pool(name="mixt", bufs=1) as pt_pool,
            ):
                hT = [pm.tile([128, NH], FR, name=f"hT{kb}", tag=f"hT{kb}") for kb in range(KB)]
                xm = [pm.tile([128, NH], FR, name=f"xm{m}", tag=f"xm{m}") for m in range(MB)]
                pre = None  # allocated lazily in premix, aliasing xm slots

                # ---- rmsnorm1 + transpose to hT ----
                with nc.named_scope("rms1"), tc.tile_pool(name="ps1", bufs=1, space="PSUM") as psA:
                    for tb in range(NH // 128):
                        if tb == 0:
                            xt = pt_pool.tile([128, D], FP, tag="xt", bufs=2)
                        else:
                            xt = xo[tb - 1]
                        nc.sync.dma_start(xt[:], dp["x_sh"][tb * 128:(tb + 1) * 128, :])
                        scr = pt_pool.tile([128, D], FP, tag="scr", bufs=2)
                        sq = pt_pool.tile([128, 1], FP, tag="sq", bufs=2)
                        nc.scalar.activation(scr[:], xt[:], AF.Square, accum_out=sq[:])
                        nr = pt_pool.tile([128, 1], FP, tag="nr", bufs=2)
                        nc.vector.tensor_scalar(nr[:], sq[:], 1.0 / D, 1e-6, ALU.mult, ALU.add)
                        nc.scalar.sqrt(nr[:], nr[:])
                        nc.vector.reciprocal(nr[:], nr[:])
                        h_t = pt_pool.tile([128, D], FP, tag="scr", bufs=2)
                        nc.vector.tensor_scalar(h_t[:], xt[:], nr[:], None, ALU.mult)
                        for kb in range(KB):
                            ptr = psA.tile([128, 128], FP, tag="ptr", bufs=2)
                            nc.tensor.transpose(ptr[:], h_t[:, kb * 128:(kb + 1) * 128], ident[:])
                            nc.vector.tensor_copy(hT[kb][:, tb * 128:(tb + 1) * 128], ptr[:])

                ipb_sb = load_pcol("ipb", 2 * INNER, 32)
                cb_sb = load_pcol("cb", INNER, 16)
                cw_sb = po.tile([128, 16, 3], FP)  # [p, m, k]
                nc.sync.dma_start(cw_sb[:], dp["cw"].ap().rearrange("(m p) k -> p m k", p=128))

                # ---- in_proj (x_main half) + conv + silu ----
                with nc.named_scope("in_proj"), tc.tile_pool(name="ps2", bufs=1, space="PSUM") as psA:
                    for q in range(4):
                        wq = []
                        for kb in range(KB):
                            wt = pt_pool.tile([128, 512], FR, tag=f"wip{kb}", bufs=1,
                                              name=f"wip{kb}")
                            nc.gpsimd.dma_start(
                                wt[:], dp["ipw"][kb * 128:(kb + 1) * 128,
                                                 q * 512:(q + 1) * 512])
                            wq.append(wt)
                        for mi in range(4):
                            m = q * 4 + mi
                            xzp = pt_pool.tile([128, NH + 2], FP, tag="xzp", bufs=2)
                            nc.vector.memset(xzp[:, 0:2], 0.0)
                            for n0, nw in ((0, 512), (512, 128)):
                                px = psA.tile([128, 512], FP, tag="px", bufs=2)
                                for kb in range(KB):
                                    nc.tensor.matmul(px[:, 0:nw],
                                                     wq[kb][:, mi * 128:(mi + 1) * 128],
                                                     hT[kb][:, n0:n0 + nw],
                                                     start=(kb == 0), stop=(kb == KB - 1))
                                nc.scalar.activation(xzp[:, 2 + n0:2 + n0 + nw], px[:, 0:nw],
                                                     AF.Identity, bias=ipb_sb[:, m:m + 1])
                            cv = pt_pool.tile([128, NH], FP, tag="cv", bufs=2)
                            nc.vector.tensor_scalar(cv[:], xzp[:, 0:NH], cw_sb[:, m, 0:1],
                                                    None, ALU.mult)
                            nc.vector.scalar_tensor_tensor(cv[:], xzp[:, 1:1 + NH],
                                                           cw_sb[:, m, 1:2], cv[:],
                                                           ALU.mult, ALU.add)
                            nc.vector.scalar_tensor_tensor(cv[:], xzp[:, 2:2 + NH],
                                                           cw_sb[:, m, 2:3], cv[:],
                                                           ALU.mult, ALU.add)
                            sgc = pt_pool.tile([128, NH], FP, tag="sgc", bufs=2)
                            nc.scalar.activation(sgc[:], cv[:], AF.Sigmoid, bias=cb_sb[:, m:m + 1])
                            nc.vector.scalar_tensor_tensor(xm[m][:], cv[:], cb_sb[:, m:m + 1],
                                                           sgc[:], ALU.add, ALU.mult)

                dtb_sb = load_vec1("dtb", S)
                bpb_sb = load_vec1("bpb", S)
                cpb_sb = load_vec1("cpb", S)
                dtw_sb = load_kw("dtw")
                bpw_sb = load_kw("bpw")
                cpw_sb = load_kw("cpw")

                # ---- dt/B/C projections + scan ----
                with nc.named_scope("scan"), tc.tile_pool(name="ps3", bufs=1, space="PSUM") as psA:
                    dt_t = pt_pool.tile([S, NH], FP, tag="dt")
                    a_t = pt_pool.tile([S, NH], FP, tag="a")
                    b_t = pt_pool.tile([S, NH], FP, tag="b")
                    c_t = pt_pool.tile([S, NH], FP, tag="c")
                    for n0, nw in ((0, 512), (512, 128)):
                        for wsb, bias_sb, dst, fn in (
                            (dtw_sb, dtb_sb, dt_t, AF.Sigmoid),
                            (cpw_sb, cpb_sb, c_t, AF.Identity),
                        ):
                            pz = psA.tile([S, 512], FP, tag="pz", bufs=2)
                            for kb in range(MB):
                                nc.tensor.matmul(pz[:, 0:nw], wsb[:, kb, :],
                                                 xm[kb][:, n0:n0 + nw],
                                                 start=(kb == 0), stop=(kb == MB - 1))
                            nc.scalar.activation(dst[:, n0:n0 + nw], pz[:, 0:nw], fn,
                                                 bias=bias_sb[:])
                        # b needs dt -> separate pass
                        pz = psA.tile([S, 512], FP, tag="pz", bufs=2)
                        for kb in range(MB):
                            nc.tensor.matmul(pz[:, 0:nw], bpw_sb[:, kb, :],
                                             xm[kb][:, n0:n0 + nw],
                                             start=(kb == 0), stop=(kb == MB - 1))
                        nc.vector.scalar_tensor_tensor(b_t[:, n0:n0 + nw], pz[:, 0:nw],
                                                       bpb_sb[:], dt_t[:, n0:n0 + nw],
                                                       ALU.add, ALU.mult)
                    nc.scalar.activation(a_t[:], dt_t[:], AF.Identity, bias=1.0, scale=-1.0)
                    st_t = pt_pool.tile([S, NH], FP, tag="st")
                    nc.vector.tensor_tensor_scan(st_t[:], a_t[:], b_t[:], 0.0,
                                                 ALU.mult, ALU.add)
                    y_t = pt_pool.tile([S, OWN], FP, tag="dt", name="y_t")
                    nc.vector.tensor_mul(y_t[:], c_t[:, HALO:NH], st_t[:, HALO:NH])

                # ---- layernorm over S (transpose - LN - transpose back) ----
                with nc.named_scope("ln"), tc.tile_pool(name="ps4", bufs=1, space="PSUM") as psA:
                    yln = pt_pool.tile([S, OWN], FR, tag="a", name="yln")
                    for i in range(OTB):
                        ptr = psA.tile([128, 128], FP, tag="ptr", bufs=2)
                        nc.tensor.transpose(ptr[:, 0:S], y_t[:, i * 128:(i + 1) * 128],
                                            ident[0:S, 0:S])
                        yT = pt_pool.tile([128, S], FP, tag="yT", bufs=2)
                        nc.vector.tensor_copy(yT[:], ptr[:, 0:S])
                        mu = pt_pool.tile([128, 1], FP, tag="mu", bufs=2)
                        nc.vector.tensor_reduce(mu[:], yT[:], mybir.AxisListType.X, ALU.add)
                        nc.vector.tensor_scalar_mul(mu[:], mu[:], 1.0 / S)
                        xc = pt_pool.tile([128, S], FP, tag="xc", bufs=2)
                        nc.vector.tensor_scalar_sub(xc[:], yT[:], mu[:])
                        scr2 = pt_pool.tile([128, S], FP, tag="scr2", bufs=2)
                        vv = pt_pool.tile([128, 1], FP, tag="vv", bufs=2)
                        nc.scalar.activation(scr2[:], xc[:], AF.Square, accum_out=vv[:])
                        nc.vector.tensor_scalar(vv[:], vv[:], 1.0 / S, 1e-5, ALU.mult, ALU.add)
                        nc.scalar.sqrt(vv[:], vv[:])
                        nc.vector.reciprocal(vv[:], vv[:])
                        nc.vector.tensor_scalar_mul(xc[:], xc[:], vv[:])
                        ptr2 = psA.tile([128, 128], FP, tag="ptr2", bufs=2)
                        nc.tensor.transpose(ptr2[0:S, :], xc[:], ident[:])
                        nc.vector.tensor_copy(yln[:, i * 128:(i + 1) * 128], ptr2[0:S, :])

                s2ib_sb = load_pcol("s2ib", INNER, 16)
                Dp_sb = load_pcol("Dp", INNER, 16)
                s2iw_sb = po.tile([S, INNER], FR)
                nc.sync.dma_start(s2iw_sb[:], dp["s2iw"][:])
                ones1 = po.tile([1, 128], FR)
                nc.sync.dma_start(ones1[:], dp["ones1"][:])

                # ---- s2i + gate sigmoid + pre_out assembly ----
                with nc.named_scope("premix"), tc.tile_pool(name="ps5", bufs=1, space="PSUM") as psA:
                    pre = []
                    for m in range(MB):
                        q, mi = divmod(m, 4)
                        if mi == 0:
                            wq = []
                            for kb in range(KB):
                                wt = pt_pool.tile([128, 512], FR, tag=f"wip{kb}", bufs=1,
                                                  name=f"wipg{kb}")
                                nc.gpsimd.dma_start(
                                    wt[:], dp["ipw"][kb * 128:(kb + 1) * 128,
                                                     2048 + q * 512:2048 + (q + 1) * 512])
                                wq.append(wt)
                        ps = psA.tile([128, 512], FP, tag="ps", bufs=2)
                        nc.tensor.matmul(ps[:], s2iw_sb[:, m * 128:(m + 1) * 128], yln[:],
                                         start=True, stop=True)
                        pg = psA.tile([128, 512], FP, tag="pg", bufs=2)
                        for kb in range(KB):
                            nc.tensor.matmul(pg[:], wq[kb][:, mi * 128:(mi + 1) * 128],
                                             hT[kb][:, HALO:NH],
                                             start=(kb == 0), stop=(kb == KB - 1))
                        sg = pt_pool.tile([128, OWN], FP, tag="sg", bufs=2)
                        nc.scalar.activation(sg[:], pg[:], AF.Sigmoid,
                                             bias=ipb_sb[:, MB + m:MB + m + 1])
                        tmp = pt_pool.tile([128, OWN], FP, tag="tmp", bufs=2)
                        nc.vector.tensor_scalar(tmp[:], xm[m][:, HALO:NH],
                                                Dp_sb[:, m:m + 1], None, ALU.mult)
                        nc.vector.scalar_tensor_tensor(tmp[:], ps[:], s2ib_sb[:, m:m + 1],
                                                       tmp[:], ALU.add, ALU.add)
                        pre_m = pm.tile([128, OWN], FR, tag=f"xm{m}", name=f"pre{m}")
                        nc.vector.tensor_mul(pre_m[:], tmp[:], sg[:])
                        pre.append(pre_m)

                ob_sb = load_row("ob", D, FR)
                gw_sb = po.tile([128, KB, E], FP)  # [p, kb, e]
                nc.sync.dma_start(gw_sb[:], dp["gw"].ap().rearrange("(kb p) e -> p kb e", p=128))
                gb_sb = load_row("gb", E, FR)

                # ---- per-tb: out projection + residual + rms2 + h2T + gating + gather ----
                with nc.named_scope("outproj"), tc.tile_pool(name="ps6", bufs=1, space="PSUM") as psA:
                    for tb in range(OTB):
                        for nb in range(2):
                            po_t = psA.tile([128, 512], FP, tag="po", bufs=2)
                            for kb in range(MB):
                                owt = pt_pool.tile([128, 512], FR, tag="owt", bufs=2)
                                nc.gpsimd.dma_start(owt[:], dp["ow"][kb * 128:(kb + 1) * 128,
                                                                     nb * 512:(nb + 1) * 512])
                                nc.tensor.matmul(po_t[:], pre[kb][:, tb * 128:(tb + 1) * 128],
                                                 owt[:], start=(kb == 0), stop=False)
                            nc.tensor.matmul(po_t[:], ones1[:],
                                             ob_sb[:, nb * 512:(nb + 1) * 512],
                                             start=False, stop=True)
                            nc.vector.tensor_add(xmid[tb][:, nb * 512:(nb + 1) * 512],
                                                 po_t[:], xo[tb][:, nb * 512:(nb + 1) * 512])
                        # rms2 for this tb
                        scr = pt_pool.tile([128, D], FP, tag="scr", bufs=2)
                        sq = pt_pool.tile([128, 1], FP, tag="sq", bufs=2)
                        nc.scalar.activation(scr[:], xmid[tb][:], AF.Square, accum_out=sq[:])
                        nr = pt_pool.tile([128, 1], FP, tag="nr", bufs=2)
                        nc.vector.tensor_scalar(nr[:], sq[:], 1.0 / D, 1e-6, ALU.mult, ALU.add)
                        nc.scalar.sqrt(nr[:], nr[:])
                        nc.vector.reciprocal(nr[:], nr[:])
                        h2 = pt_pool.tile([128, D], FP, tag="xt", bufs=2, name="h2")
                        nc.vector.tensor_scalar(h2[:], xmid[tb][:], nr[:], None, ALU.mult)
                        pl = psA.tile([128, E], FP, tag="pl", bufs=2)
                        for kb in range(KB):
                            ptr = psA.tile([128, 128], FP, tag="ptr", bufs=2)
                            nc.tensor.transpose(ptr[:], h2[:, kb * 128:(kb + 1) * 128], ident[:])
                            h2T_t = pt_pool.tile([128, 128], FP, tag="h2T", bufs=2)
                            nc.vector.tensor_copy(h2T_t[:], ptr[:])
                            h2T_b = pt_pool.tile([128, 128], BF, tag="h2Tb", bufs=2)
                            nc.vector.tensor_copy(h2T_b[:], h2T_t[:])
                            nc.sync.dma_start(
                                gth_in[tb][kb * 128:(kb + 1) * 128, :], h2T_b[:])
                            if debug_outputs:
                                nc.sync.dma_start(
                                    dbg["h2T"][kb * 128:(kb + 1) * 128,
                                               tb * 128:(tb + 1) * 128], h2T_t[:])
                            nc.tensor.matmul(pl[:], h2T_t[:], gw_sb[:, kb, :],
                                             start=(kb == 0), stop=False)
                        nc.tensor.matmul(pl[:], ones1[:], gb_sb[:], start=False, stop=True)
                        # top-2-of-4 gating
                        m1 = pt_pool.tile([128, 1], FP, tag="m1", bufs=2)
                        nc.vector.tensor_reduce(m1[:], pl[:], mybir.AxisListType.X, ALU.max)
                        eq1 = pt_pool.tile([128, E], FP, tag="eq1", bufs=2)
                        nc.vector.tensor_scalar(eq1[:], pl[:], m1[:], None, ALU.is_equal)
                        msk = pt_pool.tile([128, E], FP, tag="msk", bufs=2)
                        nc.vector.scalar_tensor_tensor(msk[:], eq1[:], -1e30, pl[:],
                                                       ALU.mult, ALU.add)
                        m2 = pt_pool.tile([128, 1], FP, tag="m2", bufs=2)
                        nc.vector.tensor_reduce(m2[:], msk[:], mybir.AxisListType.X, ALU.max)
                        eq2 = pt_pool.tile([128, E], FP, tag="eq2", bufs=2)
                        nc.vector.tensor_scalar(eq2[:], msk[:], m2[:], None, ALU.is_equal)
                        dd = pt_pool.tile([128, 1], FP, tag="dd", bufs=2)
                        nc.vector.tensor_sub(dd[:], m2[:], m1[:])
                        p2 = pt_pool.tile([128, 1], FP, tag="p2", bufs=2)
                        nc.scalar.activation(p2[:], dd[:], AF.Sigmoid)
                        p1b = pt_pool.tile([128, 1], FP, tag="p1b", bufs=2)
                        nc.scalar.activation(p1b[:], p2[:], AF.Identity, bias=1.0, scale=-1.0)
                        wv = pt_pool.tile([128, E], FP, tag="wv", bufs=2)
                        nc.vector.tensor_scalar(wv[:], eq1[:], p1b[:], None, ALU.mult)
                        nc.vector.scalar_tensor_tensor(wv[:], eq2[:], p2[:], wv[:],
                                                       ALU.mult, ALU.add)
                        nc.sync.dma_start(gtw_in[tb * 128:(tb + 1) * 128, :], wv[:])
                        if debug_outputs:
                            nc.sync.dma_start(dbg["wown"][tb * 128:(tb + 1) * 128, :], wv[:])
                            nc.sync.dma_start(dbg["xmid"][tb * 128:(tb + 1) * 128, :],
                                              xmid[tb][:])
                        nc.gpsimd.collective_compute(
                            "AllGather", ALU.bypass, replica_groups=rg,
                            ins=[gth_in[tb].opt()], outs=[gth_out[tb].opt()])
                    with nc.named_scope("gather"):
                        nc.gpsimd.collective_compute(
                            "AllGather", ALU.bypass, replica_groups=rg,
                            ins=[gtw_in.opt()], outs=[gtw_out.opt()])

            # =======================================================
            # MoE (full expert per core, token-half group of 4)
            # =======================================================
            with (
                tc.tile_pool(name="moe", bufs=1) as pq,
                tc.tile_pool(name="psC", bufs=1, space="PSUM") as psC,
            ):
                esel = po.tile([128, E], FP)
                nc.sync.dma_start(esel[:], dp["esel"][:])
                rmask = po.tile([128, 4], FP)
                nc.sync.dma_start(rmask[:], dp["rmask"][:])
                eb1_sb = load_pcol("eb1", 2 * HH, 32)
                eb2h_sb = load_row("eb2h", D, FR)
                HB = 2 * HH // 128  # 32 hid blocks
                with nc.named_scope("moe_w"):
                    ew1_sb = [pq.tile([128, 2 * HH], BF, name=f"ew1_{kb}", tag=f"ew1_{kb}")
                              for kb in range(KB)]
                    for kb in range(KB):
                        nc.gpsimd.dma_start(ew1_sb[kb][:], dp["ew1"][kb * 128:(kb + 1) * 128, :])

                with nc.named_scope("moe"):
                    for r in range(4):
                        h2r = []
                        for kb in range(KB):
                            t = pq.tile([128, OWN], BF, tag=f"h2r{kb}", bufs=2)
                            for t_ in range(OTB):
                                nc.sync.dma_start(
                                    t[:, t_ * 128:(t_ + 1) * 128],
                                    gth_out[t_][r * D + kb * 128: r * D + (kb + 1) * 128, :])
                            h2r.append(t)
                        hid = []
                        for h in range(HB):
                            ph = psC.tile([128, 512], FP, tag="ph", bufs=2)
                            for kb in range(KB):
                                nc.tensor.matmul(ph[:], ew1_sb[kb][:, h * 128:(h + 1) * 128],
                                                 h2r[kb][:], start=(kb == 0), stop=(kb == KB - 1))
                            ht = pq.tile([128, OWN], BF, tag=f"hid{h}", bufs=1)
                            nc.scalar.activation(ht[:], ph[:], AF.Gelu, bias=eb1_sb[:, h:h + 1])
                            hid.append(ht)
                        # per-token weight for this core's expert
                        wvr = pq.tile([128, OTB, E], FP, tag="wvr", bufs=2)
                        nc.sync.dma_start(
                            wvr[:], gtw_out[r * OWN:(r + 1) * OWN, :]
                            .rearrange("(tb p) e -> p tb e", p=128))
                        ws = []
                        for tb in range(OTB):
                            wm_t = pq.tile([128, E], FP, tag="wm", bufs=2)
                            nc.vector.tensor_mul(wm_t[:], wvr[:, tb, :], esel[:])
                            ws_t = pq.tile([128, 1], FP, tag=f"ws{tb}", bufs=2)
                            nc.vector.tensor_reduce(ws_t[:], wm_t[:], mybir.AxisListType.X,
                                                    ALU.add)
                            ws.append(ws_t)
                        for nb in range(2):
                            peo = [psC.tile([128, 512], FP, tag=f"peo{t_}", bufs=1,
                                            name=f"peo{t_}") for t_ in range(OTB)]
                            for h in range(HB):
                                ew2t = pq.tile([128, 512], BF, tag="ew2t", bufs=4)
                                nc.gpsimd.dma_start(
                                    ew2t[:], dp["ew2"][h * 128:(h + 1) * 128,
                                                       nb * 512:(nb + 1) * 512])
                                for tb in range(OTB):
                                    nc.tensor.matmul(
                                        peo[tb][:], hid[h][:, tb * 128:(tb + 1) * 128],
                                        ew2t[:], start=(h == 0), stop=False)
                            for tb in range(OTB):
                                nc.tensor.matmul(peo[tb][:], ones1[:],
                                                 eb2h_sb[:, nb * 512:(nb + 1) * 512],
                                                 start=False, stop=True)
                                wout = pq.tile([128, 512], FP, tag="wout", bufs=3)
                                nc.vector.tensor_scalar(wout[:], peo[tb][:], ws[tb][:],
                                                        None, ALU.mult)
                                nc.vector.scalar_tensor_tensor(
                                    wout[:], xmid[tb][:, nb * 512:(nb + 1) * 512],
                                    rmask[:, r:r + 1], wout[:], ALU.mult, ALU.add)
                                nc.sync.dma_start(
                                    rs_in[r][tb * 128:(tb + 1) * 128,
                                             nb * 512:(nb + 1) * 512], wout[:])
                        nc.gpsimd.collective_compute(
                            "ReduceScatter", ALU.add, replica_groups=rg,
                            ins=[rs_in[r].opt()], outs=[rs_out[r].opt()])

                with nc.named_scope("final"):
                    for r in range(4):
                        nc.sync.dma_start(out_d[r * 128:(r + 1) * 128, :], rs_out[r][:])

    nc.compile()
    return nc


def host_prep(inputs):
    """Build the 8 per-core input maps from full inputs."""
    import ml_dtypes
    f32 = np.float32
    x = np.ascontiguousarray(np.asarray(inputs["x"], f32).reshape(B * T, D))
    n1 = np.asarray(inputs["norm1_w"], f32)
    n2 = np.asarray(inputs["norm2_w"], f32)
    ipw = np.ascontiguousarray(np.asarray(inputs["in_proj_w"], f32) * n1[:, None])
    gw = np.ascontiguousarray(np.asarray(inputs["gate_w"], f32) * n2[:, None])
    ew1f = np.asarray(inputs["e_w1"], f32) * n2[None, :, None]
    ew1b = ew1f.astype(ml_dtypes.bfloat16)
    ew2b = np.asarray(inputs["e_w2"], f32).astype(ml_dtypes.bfloat16)
    ident = np.eye(128, dtype=f32)
    ones1 = np.ones((1, 128), f32)
    shared = {
        "ipw": ipw, "ipb": np.asarray(inputs["in_proj_b"], f32),
        "cw": np.ascontiguousarray(np.asarray(inputs["conv_w"], f32)[:, 0, :]),
        "cb": np.asarray(inputs["conv_b"], f32),
        "dtw": np.asarray(inputs["dt_w"], f32), "dtb": np.asarray(inputs["dt_b"], f32),
        "bpw": np.asarray(inputs["bp_w"], f32), "bpb": np.asarray(inputs["bp_b"], f32),
        "cpw": np.asarray(inputs["cp_w"], f32), "cpb": np.asarray(inputs["cp_b"], f32),
        "s2iw": np.asarray(inputs["s2i_w"], f32), "s2ib": np.asarray(inputs["s2i_b"], f32),
        "Dp": np.asarray(inputs["D_param"], f32),
        "ow": np.asarray(inputs["out_w"], f32), "ob": np.asarray(inputs["out_b"], f32),
        "gw": gw, "gb": np.asarray(inputs["gate_b"], f32),
        "ident": ident, "ones1": ones1,
    }
    eb1 = np.asarray(inputs["e_b1"], f32)
    eb2 = np.asarray(inputs["e_b2"], f32)
    in_maps = []
    for c in range(N_CORES):
        e, th = c // 2, c % 2
        g0 = th * (B * T // 2) + e * OWN
        if e == 0:
            x_sh = np.concatenate([np.zeros((HALO, D), f32), x[g0:g0 + OWN]])
        else:
            x_sh = x[g0 - HALO:g0 + OWN]
        m = dict(shared)
        m["x_sh"] = np.ascontiguousarray(x_sh)
        m["ew1"] = np.ascontiguousarray(ew1b[e])
        m["eb1"] = np.ascontiguousarray(eb1[e])
        m["ew2"] = np.ascontiguousarray(ew2b[e])
        m["eb2h"] = np.ascontiguousarray(eb2[e])
        esel = np.zeros((128, E), f32)
        esel[:, e] = 1.0
        m["esel"] = esel
        rmask = np.zeros((128, 4), f32)
        rmask[:, e] = 1.0
        m["rmask"] = rmask
        in_maps.append(m)
    return in_maps


def unshard_out(results):
    """results: list of 8 dicts with 'out' [OWN, D]; rows r*128+i of core c
    hold global token (c%2)*2048 + r*512 + (c//2)*128 + i."""
    full = np.empty((B * T, D), np.float32)
    for c in range(N_CORES):
        e, th = c // 2, c % 2
        oc = results[c]["out"]
        for r in range(4):
            full[th * 2048 + r * OWN + e * 128: th * 2048 + r * OWN + (e + 1) * 128] = \
                oc[r * 128:(r + 1) * 128]
    return full.reshape(B, T, D)


_NC_CACHE = {}


def _get_nc():
    if "nc" not in _NC_CACHE:
        _NC_CACHE["nc"] = build(debug_outputs=False)
    return _NC_CACHE["nc"]


def kernel(**inputs) -> np.ndarray:
    """Full-input entry point: shards across 8 NeuronCores, runs the Bass
    kernel SPMD, reassembles the full [2, 2048, 1024] output."""
    import sys, types
    try:  # NTFF profile hook shim (missing antenv.axon_hooks in this image)
        import antenv.axon_hooks  # noqa: F401
    except ImportError:
        try:
            import antenv
            from trn_agent_boot.trn_boot import _ntff_profile_via_ctypes
            mod = types.ModuleType("antenv.axon_hooks")
            try:
                _hook = _ntff_profile_via_ctypes("/opt/axon/libaxon_pjrt.so")
            except Exception:
                _hook = None
            mod.get_axon_ntff_profile_hook = lambda: _hook
            mod.set_axon_ntff_profile_hook = lambda h: None
            sys.modules["antenv.axon_hooks"] = mod
            antenv.axon_hooks = mod
        except Exception:
            pass
    from concourse.bass_utils import run_bass_kernel_spmd

    nc = _get_nc()
    in_maps = host_prep(inputs)
    res = run_bass_kernel_spmd(nc, in_maps, core_ids=list(range(N_CORES)))
    out = unshard_out(res.results)
    return out.astype(np.float32)



# revision 25
# speedup vs baseline: 1.5482x; 1.5482x over previous
"""Bass kernel builder for nn_MixtureOfMambaBlock — 8-core SPMD, v2 (bf16).

Sharding: tokens 8-way (512/core + 64 halo for conv+scan warmup); mixer fully
local per core (weights replicated, bf16). Post-mixer h2 all-gathered (bf16),
MoE expert x token-half sharded (bf16 weights SBUF-resident), weighted partials
reduce-scattered (bf16) back to token shards; residual added locally after RS.
"""
import numpy as np
import concourse.bass as bass
import concourse.bacc as bacc
import concourse.mybir as mybir
import concourse.tile as tile

FP = mybir.dt.float32
FR = mybir.dt.float32r
BF = mybir.dt.bfloat16
AF = mybir.ActivationFunctionType
ALU = mybir.AluOpType

B, T, D = 2, 2048, 1024
S, INNER = 64, 2048
E = 4
HID = 4096
OWN, HALO = 512, 64
NH = OWN + HALO          # 576
KB = D // 128            # 8  d-blocks
MB = INNER // 128        # 16 inner-blocks
HB = HID // 128          # 32 hid-blocks
OTB = OWN // 128         # 4  own-token blocks
N_CORES = 8

INPUT_SPECS = {
    "x_sh": ([NH, D], FP),
    "ipw": ([D, 2 * INNER], BF), "ipb": ([2 * INNER], FP),
    "cw": ([INNER, 3], FP), "cb": ([INNER], FP),
    "dtw": ([INNER, S], BF), "dtb": ([S], FP),
    "bpw": ([INNER, S], BF), "bpb": ([S], FP),
    "cpw": ([INNER, S], BF), "cpb": ([S], FP),
    "s2iw": ([S, INNER], BF), "s2ib": ([INNER], FP),
    "Dp": ([INNER], FP),
    "ow": ([INNER, D], BF), "ob": ([D], FR),
    "gw": ([D, E], FP), "gb": ([E], FR),
    "ew1": ([D, HID], BF), "eb1": ([HID], FP),
    "ew2": ([HID, D], BF), "eb2h": ([D], FR),
    "esel": ([128, E], FP),
    "rmask": ([128, 4], FP),
    "ident": ([128, 128], FP),
    "identb": ([128, 128], BF),
    "ones1": ([1, 128], FR),
}


def build(debug_outputs=False):
    nc = bacc.Bacc("TRN2", target_bir_lowering=False, debug=False,
                   num_devices=N_CORES)
    dp = {}
    for name, (shape, dt) in INPUT_SPECS.items():
        dp[name] = nc.dram_tensor(name, shape, dt, kind="ExternalInput")
    out_d = nc.dram_tensor("out", [OWN, D], FP, kind="ExternalOutput")
    dbg = {}
    if debug_outputs:
        dbg["xmid"] = nc.dram_tensor("dbg_xmid", [OWN, D], FP, kind="ExternalOutput")
        dbg["wown"] = nc.dram_tensor("dbg_wown", [OWN, E], FP, kind="ExternalOutput")

    rg = [[0, 2, 4, 6], [1, 3, 5, 7]]

    with tile.TileContext(nc) as tc:
        with (
            tc.tile_pool(name="outer", bufs=1) as po,
            tc.tile_pool(name="dram", bufs=1, space="DRAM") as pdram,
        ):
            # ---------- DRAM bounce buffers for collectives ----------
            gth_in = [pdram.tile([D, 128], BF, name=f"gth_in{t_}") for t_ in range(OTB)]
            gth_out = [pdram.tile([4 * D, 128], BF, name=f"gth_out{t_}")
                       for t_ in range(OTB)]
            gtw_in = pdram.tile([OWN, E], FP)
            gtw_out = pdram.tile([4 * OWN, E], FP)
            rs_in = [pdram.tile([OWN, D], BF, name=f"rs_in{r}") for r in range(4)]
            rs_out = [pdram.tile([128, D], BF, name=f"rs_out{r}") for r in range(4)]

            # ---------- constants / small weights (emit all loads up front) ----
            ident = po.tile([128, 128], FP)
            nc.sync.dma_start(ident[:], dp["ident"][:])
            identb = po.tile([128, 128], BF)
            nc.sync.dma_start(identb[:], dp["identb"][:])

            def load_pcol(name, blocks):  # [blocks*128] -> [128, blocks]
                t = po.tile([128, blocks], FP, name=f"{name}_sb")
                nc.sync.dma_start(
                    t[:], dp[name].ap().rearrange("(m p) -> p m", p=128))
                return t

            def load_vec1(name, n):  # [n] -> [n, 1]
                t = po.tile([n, 1], FP, name=f"{name}_sb")
                nc.sync.dma_start(t[:], dp[name].ap().rearrange("(s o) -> s o", o=1))
                return t

            def load_row(name, n, dt_=FP):  # [n] -> [1, n]
                t = po.tile([1, n], dt_, name=f"{name}_sb")
                nc.sync.dma_start(t[:], dp[name].ap().rearrange("(o s) -> o s", o=1))
                return t

            def load_kw(name, pool):  # [2048, 64] -> [128, 16, 64], lhsT slice [:, kb, :]
                t = pool.tile([128, MB, S], BF, name=f"{name}_sb")
                nc.sync.dma_start(t[:], dp[name].ap().rearrange("(kb p) s -> p kb s", p=128))
                return t

            ob_sb = load_row("ob", D, FR)
            gb_sb = load_row("gb", E, FR)
            eb2h_sb = load_row("eb2h", D, FR)
            ones1 = po.tile([1, 128], FR)
            nc.sync.dma_start(ones1[:], dp["ones1"][:])
            ipb_sb = load_pcol("ipb", 32)
            cb_sb = load_pcol("cb", 16)
            cw_sb = po.tile([128, 16, 3], FP)  # [p, m, k]
            nc.sync.dma_start(cw_sb[:], dp["cw"].ap().rearrange("(m p) k -> p m k", p=128))
            dtb_sb = load_vec1("dtb", S)
            bpb_sb = load_vec1("bpb", S)
            cpb_sb = load_vec1("cpb", S)
            s2ib_sb = load_pcol("s2ib", 16)
            Dp_sb = load_pcol("Dp", 16)
            gw_sb = po.tile([128, KB, E], FP)  # [p, kb, e]
            nc.sync.dma_start(gw_sb[:], dp["gw"].ap().rearrange("(kb p) e -> p kb e", p=128))
            esel = po.tile([128, E], FP)
            nc.sync.dma_start(esel[:], dp["esel"][:])
            rmask = po.tile([128, 4], FP)
            nc.sync.dma_start(rmask[:], dp["rmask"][:])
            eb1_sb = load_pcol("eb1", HB)

            # persistent activations
            xmid = [po.tile([128, D], FP, name=f"xmid{t_}", tag=f"xmid{t_}")
                    for t_ in range(OTB)]

            # =======================================================
            # MIXER
            # =======================================================
            with (
                tc.tile_pool(name="mixer", bufs=1) as pm,
                tc.tile_pool(name="mixt", bufs=1) as pt_pool,
            ):
                # pool allocation order matters: tiles that die early (hT, xm,
                # sg, projection weights) go FIRST so their addresses sit at the
                # pool base — the MoE pool's ew1 tiles (allocated first there)
                # land on them and can start loading before outproj finishes.
                hT = [pm.tile([128, NH], BF, name=f"hT{kb}", tag=f"hT{kb}") for kb in range(KB)]
                xm = [pm.tile([128, NH], BF, name=f"xm{m}", tag=f"xm{m}") for m in range(MB)]
                sg = [pm.tile([128, OWN], BF, name=f"sg{m}", tag=f"sg{m}")
                      for m in range(8)]
                dtw_sb = load_kw("dtw", pm)
                bpw_sb = load_kw("bpw", pm)
                cpw_sb = load_kw("cpw", pm)
                s2iw_sb = pm.tile([S, INNER], BF, name="s2iw_sb")
                nc.sync.dma_start(s2iw_sb[:], dp["s2iw"][:])
                # late-freed tiles (used through outproj) at higher addresses
                ow_sb = pm.tile([128, MB, D], BF, name="ow_sb")
                nc.scalar.dma_start(
                    ow_sb[:], dp["ow"].ap().rearrange("(kb p) d -> p kb d", p=128))
                xo = [pm.tile([128, D], FP, name=f"xo{t_}", tag=f"xo{t_}")
                      for t_ in range(OTB)]

                # ---- rmsnorm1 + transpose to hT ----
                # chunks: [64 halo] + 4x [128 own]
                chunks = [(0, HALO, None)] + [
                    (HALO + t_ * 128, 128, t_) for t_ in range(OTB)]
                with nc.named_scope("rms1"), tc.tile_pool(name="ps1", bufs=1, space="PSUM") as psA:
                    for (row0, rows, t_) in chunks:
                        if t_ is None:
                            xt = pt_pool.tile([HALO, D], FP, tag="xt0")
                        else:
                            xt = xo[t_]
                        # gpsimd queue: ahead of the ipw weight chunks, and not
                        # behind the ~20 small constant loads on the sync queue
                        nc.gpsimd.dma_start(xt[:], dp["x_sh"][row0:row0 + rows, :])
                        scr = pt_pool.tile([128, D], FP, tag="scr", bufs=1)
                        sq = pt_pool.tile([128, 1], FP, tag="sq", bufs=2)
                        nc.scalar.activation(scr[0:rows, :], xt[:], AF.Square,
                                             accum_out=sq[0:rows, :])
                        nr = pt_pool.tile([128, 1], FP, tag="nr", bufs=2)
                        nc.vector.tensor_scalar(nr[0:rows, :], sq[0:rows, :], 1.0 / D,
                                                1e-6, ALU.mult, ALU.add)
                        nc.scalar.sqrt(nr[0:rows, :], nr[0:rows, :])
                        nc.vector.reciprocal(nr[0:rows, :], nr[0:rows, :])
                        h_t = pt_pool.tile([128, D], BF, tag="htb", bufs=2)
                        nc.vector.tensor_scalar(h_t[0:rows, :], xt[:], nr[0:rows, :],
                                                None, ALU.mult)
                        for kb in range(KB):
                            ptr = psA.tile([128, 128], BF, tag="ptr", bufs=2)
                            nc.tensor.transpose(ptr[:, 0:rows],
                                                h_t[0:rows, kb * 128:(kb + 1) * 128],
                                                identb[0:rows, 0:rows])
                            nc.vector.tensor_copy(hT[kb][:, row0:row0 + rows],
                                                  ptr[:, 0:rows])

                # ---- in_proj (x_main half) + conv + silu ----
                with nc.named_scope("in_proj"), tc.tile_pool(name="ps2", bufs=1, space="PSUM") as psA:
                    for q in range(4):
                        wq = []
                        for kb in range(KB):
                            wt = pt_pool.tile([128, 512], BF, tag=f"wip{kb}", bufs=2,
                                              name=f"wip{kb}")
                            nc.gpsimd.dma_start(
                                wt[:], dp["ipw"][kb * 128:(kb + 1) * 128,
                                                 q * 512:(q + 1) * 512])
                            wq.append(wt)
                        for mi in range(4):
                            m = q * 4 + mi
                            xzp = pt_pool.tile([128, NH + 2], FP, tag="xzp", bufs=2)
                            nc.vector.memset(xzp[:, 0:2], 0.0)
                            for n0, nw in ((0, 512), (512, NH - 512)):
                                px = psA.tile([128, 512], FP, tag="px", bufs=2)
                                for kb in range(KB):
                                    nc.tensor.matmul(px[:, 0:nw],
                                                     wq[kb][:, mi * 128:(mi + 1) * 128],
                                                     hT[kb][:, n0:n0 + nw],
                                                     start=(kb == 0), stop=(kb == KB - 1))
                                nc.scalar.activation(xzp[:, 2 + n0:2 + n0 + nw], px[:, 0:nw],
                                                     AF.Identity, bias=ipb_sb[:, m:m + 1])
                            cv = pt_pool.tile([128, NH], FP, tag="cv", bufs=2)
                            nc.vector.tensor_scalar(cv[:], xzp[:, 0:NH], cw_sb[:, m, 0:1],
                                                    None, ALU.mult)
                            nc.vector.scalar_tensor_tensor(cv[:], xzp[:, 1:1 + NH],
                                                           cw_sb[:, m, 1:2], cv[:],
                                                           ALU.mult, ALU.add)
                            nc.vector.scalar_tensor_tensor(cv[:], xzp[:, 2:2 + NH],
                                                           cw_sb[:, m, 2:3], cv[:],
                                                           ALU.mult, ALU.add)
                            sgc = pt_pool.tile([128, NH], BF, tag="sgc", bufs=2)
                            nc.scalar.activation(sgc[:], cv[:], AF.Sigmoid, bias=cb_sb[:, m:m + 1])
                            nc.vector.scalar_tensor_tensor(xm[m][:], cv[:], cb_sb[:, m:m + 1],
                                                           sgc[:], ALU.add, ALU.mult)

                # ---- dt/B/C projections (emitted before gate MMs; feed scan) ----
                with nc.named_scope("scan"), tc.tile_pool(name="ps3", bufs=1, space="PSUM") as psA:
                    dt_t = pt_pool.tile([S, NH], FP, tag="dt")
                    a_t = pt_pool.tile([S, NH], FP, tag="a")
                    b_t = pt_pool.tile([S, NH], FP, tag="b")
                    c_t = pt_pool.tile([S, NH], FP, tag="c")
                    for n0, nw in ((0, 512), (512, NH - 512)):
                        for wsb, bias_sb, dst, fn in (
                            (dtw_sb, dtb_sb, dt_t, AF.Sigmoid),
                            (cpw_sb, cpb_sb, c_t, AF.Identity),
                        ):
                            pz = psA.tile([S, 512], FP, tag="pz", bufs=2)
                            for kb in range(MB):
                                nc.tensor.matmul(pz[:, 0:nw], wsb[:, kb, :],
                                                 xm[kb][:, n0:n0 + nw],
                                                 start=(kb == 0), stop=(kb == MB - 1))
                            nc.scalar.activation(dst[:, n0:n0 + nw], pz[:, 0:nw], fn,
                                                 bias=bias_sb[:])
                        # b needs dt -> separate pass
                        pz = psA.tile([S, 512], FP, tag="pz", bufs=2)
                        for kb in range(MB):
                            nc.tensor.matmul(pz[:, 0:nw], bpw_sb[:, kb, :],
                                             xm[kb][:, n0:n0 + nw],
                                             start=(kb == 0), stop=(kb == MB - 1))
                        nc.vector.scalar_tensor_tensor(b_t[:, n0:n0 + nw], pz[:, 0:nw],
                                                       bpb_sb[:], dt_t[:, n0:n0 + nw],
                                                       ALU.add, ALU.mult)
                    # scan runs on the vector engine while the tensor engine
                    # works through the gate-projection matmuls below
                    nc.vector.tensor_scalar(a_t[:], dt_t[:], -1.0, 1.0,
                                            ALU.mult, ALU.add)
                    st_t = pt_pool.tile([S, NH], FP, tag="st")
                    nc.vector.tensor_tensor_scan(st_t[:], a_t[:], b_t[:], 0.0,
                                                 ALU.mult, ALU.add)
                    y_t = pt_pool.tile([S, OWN], FP, tag="yt", name="y_t")
                    nc.vector.tensor_mul(y_t[:], c_t[:, HALO:NH], st_t[:, HALO:NH])

                # ---- gate half of in_proj, first 8 m: emitted NOW so the
                # tensor queue has work while the (vector-engine) scan runs ----
                def load_wqg(q):
                    wqg = []
                    for kb in range(KB):
                        wt = pt_pool.tile([128, 512], BF, tag=f"wip{kb}", bufs=2,
                                          name=f"wipg{kb}_{q}")
                        nc.gpsimd.dma_start(
                            wt[:], dp["ipw"][kb * 128:(kb + 1) * 128,
                                             2048 + q * 512:2048 + (q + 1) * 512])
                        wqg.append(wt)
                    return wqg

                def gate_mm(psB, wqg, m):
                    mi = m % 4
                    pg = psB.tile([128, 512], FP, tag="pg", bufs=2)
                    for kb in range(KB):
                        nc.tensor.matmul(pg[:], wqg[kb][:, mi * 128:(mi + 1) * 128],
                                         hT[kb][:, HALO:NH],
                                         start=(kb == 0), stop=(kb == KB - 1))
                    return pg

                with nc.named_scope("gateproj"), tc.tile_pool(name="ps4", bufs=1, space="PSUM") as psB:
                    for q in range(2):
                        wqg = load_wqg(q)
                        for mi in range(4):
                            m = q * 4 + mi
                            pg = gate_mm(psB, wqg, m)
                            nc.scalar.activation(sg[m][:], pg[:], AF.Sigmoid,
                                                 bias=ipb_sb[:, MB + m:MB + m + 1])

                # ---- layernorm over S ----
                with nc.named_scope("scanln"), tc.tile_pool(name="ps5", bufs=1, space="PSUM") as psA:
                    yln = pt_pool.tile([S, OWN], BF, tag="a", name="yln")
                    for i in range(OTB):
                        ptr = psA.tile([128, 128], FP, tag="ptr", bufs=2)
                        nc.tensor.transpose(ptr[:, 0:S], y_t[:, i * 128:(i + 1) * 128],
                                            ident[0:S, 0:S])
                        yT = pt_pool.tile([128, S], FP, tag="yT", bufs=2)
                        nc.vector.tensor_copy(yT[:], ptr[:, 0:S])
                        mu = pt_pool.tile([128, 1], FP, tag="mu", bufs=2)
                        nc.vector.tensor_reduce(mu[:], yT[:], mybir.AxisListType.X, ALU.add)
                        nc.vector.tensor_scalar_mul(mu[:], mu[:], 1.0 / S)
                        xc = pt_pool.tile([128, S], FP, tag="xc", bufs=2)
                        nc.vector.tensor_scalar_sub(xc[:], yT[:], mu[:])
                        scr2 = pt_pool.tile([128, S], FP, tag="scr2", bufs=2)
                        vv = pt_pool.tile([128, 1], FP, tag="vv", bufs=2)
                        nc.scalar.activation(scr2[:], xc[:], AF.Square, accum_out=vv[:])
                        nc.vector.tensor_scalar(vv[:], vv[:], 1.0 / S, 1e-5, ALU.mult, ALU.add)
                        nc.scalar.sqrt(vv[:], vv[:])
                        nc.vector.reciprocal(vv[:], vv[:])
                        xcb = pt_pool.tile([128, S], BF, tag="xcb", bufs=2)
                        nc.vector.tensor_scalar_mul(xcb[:], xc[:], vv[:])
                        ptr2 = psA.tile([128, 128], BF, tag="ptr2", bufs=2)
                        nc.tensor.transpose(ptr2[0:S, :], xcb[:], identb[:])
                        nc.vector.tensor_copy(yln[:, i * 128:(i + 1) * 128], ptr2[0:S, :])

                # ---- s2i + pre_out assembly (gate m>=8 computed inline) ----
                with nc.named_scope("premix"), tc.tile_pool(name="ps6", bufs=1, space="PSUM") as psA:
                    pre = []
                    for m in range(MB):
                        if m >= 8:
                            if m % 4 == 0:
                                wqg = load_wqg(m // 4)
                            pg = gate_mm(psA, wqg, m)
                            sg_m = pt_pool.tile([128, OWN], BF, tag="sgi", bufs=2)
                            nc.scalar.activation(sg_m[:], pg[:], AF.Sigmoid,
                                                 bias=ipb_sb[:, MB + m:MB + m + 1])
                        else:
                            sg_m = sg[m]
                        ps = psA.tile([128, 512], FP, tag="ps", bufs=2)
                        nc.tensor.matmul(ps[:], s2iw_sb[:, m * 128:(m + 1) * 128], yln[:],
                                         start=True, stop=True)
                        # Dp*xm + s2ib on the scalar engine; 2 vector ops total
                        tmp = pt_pool.tile([128, OWN], FP, tag="tmp", bufs=2)
                        nc.scalar.activation(tmp[:], xm[m][:, HALO:NH], AF.Identity,
                                             bias=s2ib_sb[:, m:m + 1],
                                             scale=Dp_sb[:, m:m + 1])
                        nc.vector.tensor_add(tmp[:], tmp[:], ps[:])
                        pre_m = pm.tile([128, OWN], BF, name=f"pre{m}", tag=f"pre{m}")
                        nc.vector.tensor_mul(pre_m[:], tmp[:], sg_m[:])
                        pre.append(pre_m)

                # ---- per-tb: out projection + residual + rms2 + h2T + gating + gather ----
                with nc.named_scope("outproj"), tc.tile_pool(name="ps7", bufs=1, space="PSUM") as psA:
                    for tb in range(OTB):
                        po2 = psA.tile([128, 2, 512], FP, tag="po2", bufs=2)
                        for kb in range(MB):
                            for nb in range(2):
                                nc.tensor.matmul(po2[:, nb, :],
                                                 pre[kb][:, tb * 128:(tb + 1) * 128],
                                                 ow_sb[:, kb, nb * 512:(nb + 1) * 512],
                                                 start=(kb == 0), stop=False)
                        for nb in range(2):
                            nc.tensor.matmul(po2[:, nb, :], ones1[:],
                                             ob_sb[:, nb * 512:(nb + 1) * 512],
                                             start=False, stop=True)
                            nc.vector.tensor_add(xmid[tb][:, nb * 512:(nb + 1) * 512],
                                                 po2[:, nb, :],
                                                 xo[tb][:, nb * 512:(nb + 1) * 512])
                        # rms2 for this tb
                        scr = pt_pool.tile([128, D], FP, tag="scr", bufs=1)
                        sq = pt_pool.tile([128, 1], FP, tag="sq", bufs=2)
                        nc.scalar.activation(scr[:], xmid[tb][:], AF.Square, accum_out=sq[:])
                        nr = pt_pool.tile([128, 1], FP, tag="nr", bufs=2)
                        nc.vector.tensor_scalar(nr[:], sq[:], 1.0 / D, 1e-6, ALU.mult, ALU.add)
                        nc.scalar.sqrt(nr[:], nr[:])
                        nc.vector.reciprocal(nr[:], nr[:])
                        h2 = pt_pool.tile([128, D], FP, tag="h2", bufs=2, name="h2")
                        nc.vector.tensor_scalar(h2[:], xmid[tb][:], nr[:], None, ALU.mult)
                        # gating logits must be fp32: bf16 logits flip top-2
                        # selections vs the reference on near-ties (~0.15 abs
                        # error per flipped token)
                        pl = psA.tile([128, E], FP, tag="pl", bufs=2)
                        for kb in range(KB):
                            ptr = psA.tile([128, 128], FP, tag="ptr", bufs=2)
                            nc.tensor.transpose(ptr[:], h2[:, kb * 128:(kb + 1) * 128],
                                                ident[:])
                            h2T_t = pt_pool.tile([128, 128], FP, tag="h2T", bufs=2)
                            nc.vector.tensor_copy(h2T_t[:], ptr[:])
                            h2T_b = pt_pool.tile([128, 128], BF, tag="h2Tb", bufs=2)
                            nc.vector.tensor_copy(h2T_b[:], h2T_t[:])
                            nc.sync.dma_start(
                                gth_in[tb][kb * 128:(kb + 1) * 128, :], h2T_b[:])
                            nc.tensor.matmul(pl[:], h2T_t[:], gw_sb[:, kb, :],
                                             start=(kb == 0), stop=False)
                        nc.tensor.matmul(pl[:], ones1[:], gb_sb[:], start=False, stop=True)
                        # top-2-of-4 gating
                        m1 = pt_pool.tile([128, 1], FP, tag="m1", bufs=2)
                        nc.vector.tensor_reduce(m1[:], pl[:], mybir.AxisListType.X, ALU.max)
                        eq1 = pt_pool.tile([128, E], FP, tag="eq1", bufs=2)
                        nc.vector.tensor_scalar(eq1[:], pl[:], m1[:], None, ALU.is_equal)
                        msk = pt_pool.tile([128, E], FP, tag="msk", bufs=2)
                        nc.vector.scalar_tensor_tensor(msk[:], eq1[:], -1e30, pl[:],
                                                       ALU.mult, ALU.add)
                        m2 = pt_pool.tile([128, 1], FP, tag="m2", bufs=2)
                        nc.vector.tensor_reduce(m2[:], msk[:], mybir.AxisListType.X, ALU.max)
                        eq2 = pt_pool.tile([128, E], FP, tag="eq2", bufs=2)
                        nc.vector.tensor_scalar(eq2[:], msk[:], m2[:], None, ALU.is_equal)
                        dd = pt_pool.tile([128, 1], FP, tag="dd", bufs=2)
                        nc.vector.tensor_sub(dd[:], m2[:], m1[:])
                        p2 = pt_pool.tile([128, 1], FP, tag="p2", bufs=2)
                        nc.scalar.activation(p2[:], dd[:], AF.Sigmoid)
                        p1b = pt_pool.tile([128, 1], FP, tag="p1b", bufs=2)
                        nc.scalar.activation(p1b[:], p2[:], AF.Identity, bias=1.0, scale=-1.0)
                        wv = pt_pool.tile([128, E], FP, tag="wv", bufs=2)
                        nc.vector.tensor_scalar(wv[:], eq1[:], p1b[:], None, ALU.mult)
                        nc.vector.scalar_tensor_tensor(wv[:], eq2[:], p2[:], wv[:],
                                                       ALU.mult, ALU.add)
                        nc.sync.dma_start(gtw_in[tb * 128:(tb + 1) * 128, :], wv[:])
                        if debug_outputs:
                            nc.sync.dma_start(dbg["wown"][tb * 128:(tb + 1) * 128, :], wv[:])
                            nc.sync.dma_start(dbg["xmid"][tb * 128:(tb + 1) * 128, :],
                                              xmid[tb][:])
                        nc.gpsimd.collective_compute(
                            "AllGather", ALU.bypass, replica_groups=rg,
                            ins=[gth_in[tb].opt()], outs=[gth_out[tb].opt()])
                    with nc.named_scope("gatherw"):
                        nc.gpsimd.collective_compute(
                            "AllGather", ALU.bypass, replica_groups=rg,
                            ins=[gtw_in.opt()], outs=[gtw_out.opt()])

            # =======================================================
            # MoE (full expert per core, token-half group of 4)
            # =======================================================
            with (
                tc.tile_pool(name="moe", bufs=1) as pq,
                tc.tile_pool(name="psC", bufs=1, space="PSUM") as psC,
            ):
                # expert weights resident in SBUF for all 4 rounds
                ew1_sb = [pq.tile([128, HID], BF, name=f"ew1_{kb}", tag=f"ew1_{kb}")
                          for kb in range(KB)]
                for kb in range(KB):
                    nc.scalar.dma_start(ew1_sb[kb][:], dp["ew1"][kb * 128:(kb + 1) * 128, :])
                ew2_sb = [pq.tile([128, D], BF, name=f"ew2_{j}", tag=f"ew2_{j}")
                          for j in range(HB)]
                for j in range(HB):
                    nc.scalar.dma_start(ew2_sb[j][:], dp["ew2"][j * 128:(j + 1) * 128, :])

                with nc.named_scope("moe"):
                    for r in range(4):
                        h2r = []
                        for kb in range(KB):
                            t = pq.tile([128, OWN], BF, tag=f"h2r{kb}", bufs=1)
                            for t_ in range(OTB):
                                nc.gpsimd.dma_start(
                                    t[:, t_ * 128:(t_ + 1) * 128],
                                    gth_out[t_][r * D + kb * 128: r * D + (kb + 1) * 128, :])
                            h2r.append(t)
                        hid = []
                        for h in range(HB):
                            ph = psC.tile([128, 512], FP, tag="ph", bufs=2)
                            for kb in range(KB):
                                nc.tensor.matmul(ph[:], ew1_sb[kb][:, h * 128:(h + 1) * 128],
                                                 h2r[kb][:], start=(kb == 0), stop=(kb == KB - 1))
                            ht = pq.tile([128, OWN], BF, tag=f"hid{h}", bufs=1)
                            nc.scalar.activation(ht[:], ph[:], AF.Gelu, bias=eb1_sb[:, h:h + 1])
                            hid.append(ht)
                        # per-token weight for this core's expert
                        wvr = pq.tile([128, OTB, E], FP, tag="wvr", bufs=2)
                        nc.sync.dma_start(
                            wvr[:], gtw_out[r * OWN:(r + 1) * OWN, :]
                            .rearrange("(tb p) e -> p tb e", p=128))
                        ws = []
                        for tb in range(OTB):
                            wm_t = pq.tile([128, E], FP, tag="wm", bufs=2)
                            nc.vector.tensor_mul(wm_t[:], wvr[:, tb, :], esel[:])
                            ws_t = pq.tile([128, 1], FP, tag=f"ws{tb}", bufs=2)
                            nc.vector.tensor_reduce(ws_t[:], wm_t[:], mybir.AxisListType.X,
                                                    ALU.add)
                            ws.append(ws_t)
                        # w2: token-block pairs keep PSUM <= 6 banks
                        for tp in range(2):
                            peo = [psC.tile([128, 2, 512], FP, tag=f"peo{ti}", bufs=1,
                                            name=f"peo{ti}") for ti in range(2)]
                            for h in range(HB):
                                for ti in range(2):
                                    tb = tp * 2 + ti
                                    for nb in range(2):
                                        nc.tensor.matmul(
                                            peo[ti][:, nb, :],
                                            hid[h][:, tb * 128:(tb + 1) * 128],
                                            ew2_sb[h][:, nb * 512:(nb + 1) * 512],
                                            start=(h == 0), stop=False)
                            for ti in range(2):
                                tb = tp * 2 + ti
                                wout = pq.tile([128, D], BF, tag="wout", bufs=2)
                                for nb in range(2):
                                    nc.tensor.matmul(peo[ti][:, nb, :], ones1[:],
                                                     eb2h_sb[:, nb * 512:(nb + 1) * 512],
                                                     start=False, stop=True)
                                    n0 = nb * 512
                                    nc.vector.tensor_scalar(wout[:, n0:n0 + 512],
                                                            peo[ti][:, nb, :],
                                                            ws[tb][:], None, ALU.mult)
                                    # owner (r == e) carries the residual through
                                    # the reduce-scatter
                                    nc.vector.scalar_tensor_tensor(
                                        wout[:, n0:n0 + 512],
                                        xmid[tb][:, n0:n0 + 512], rmask[:, r:r + 1],
                                        wout[:, n0:n0 + 512], ALU.mult, ALU.add)
                                nc.sync.dma_start(
                                    rs_in[r][tb * 128:(tb + 1) * 128, :], wout[:])
                        nc.gpsimd.collective_compute(
                            "ReduceScatter", ALU.add, replica_groups=rg,
                            ins=[rs_in[r].opt()], outs=[rs_out[r].opt()])

                with nc.named_scope("final"):
                    for r in range(4):
                        rsb = pq.tile([128, D], BF, tag="rsb", bufs=2)
                        nc.sync.dma_start(rsb[:], rs_out[r][:])
                        osb = pq.tile([128, D], FP, tag="osb", bufs=1)
                        nc.vector.tensor_copy(osb[:], rsb[:])
                        nc.sync.dma_start(out_d[r * 128:(r + 1) * 128, :], osb[:])

    nc.compile()
    return nc


def host_prep(inputs):
    """Build the 8 per-core input maps from full inputs."""
    import ml_dtypes
    f32 = np.float32
    bf = ml_dtypes.bfloat16
    x = np.ascontiguousarray(np.asarray(inputs["x"], f32).reshape(B * T, D))
    n1 = np.asarray(inputs["norm1_w"], f32)
    n2 = np.asarray(inputs["norm2_w"], f32)
    ipw = np.ascontiguousarray(np.asarray(inputs["in_proj_w"], f32) * n1[:, None]).astype(bf)
    gw = np.ascontiguousarray(np.asarray(inputs["gate_w"], f32) * n2[:, None])
    ew1f = np.asarray(inputs["e_w1"], f32) * n2[None, :, None]
    ew1b = ew1f.astype(bf)
    ew2b = np.asarray(inputs["e_w2"], f32).astype(bf)
    ident = np.eye(128, dtype=f32)
    ones1 = np.ones((1, 128), f32)
    shared = {
        "ipw": ipw, "ipb": np.asarray(inputs["in_proj_b"], f32),
        "cw": np.ascontiguousarray(np.asarray(inputs["conv_w"], f32)[:, 0, :]),
        "cb": np.asarray(inputs["conv_b"], f32),
        "dtw": np.asarray(inputs["dt_w"], f32).astype(bf),
        "dtb": np.asarray(inputs["dt_b"], f32),
        "bpw": np.asarray(inputs["bp_w"], f32).astype(bf),
        "bpb": np.asarray(inputs["bp_b"], f32),
        "cpw": np.asarray(inputs["cp_w"], f32).astype(bf),
        "cpb": np.asarray(inputs["cp_b"], f32),
        "s2iw": np.asarray(inputs["s2i_w"], f32).astype(bf),
        "s2ib": np.asarray(inputs["s2i_b"], f32),
        "Dp": np.asarray(inputs["D_param"], f32),
        "ow": np.asarray(inputs["out_w"], f32).astype(bf),
        "ob": np.asarray(inputs["out_b"], f32),
        "gw": gw, "gb": np.asarray(inputs["gate_b"], f32),
        "ident": ident, "identb": ident.astype(bf), "ones1": ones1,
    }
    eb1 = np.asarray(inputs["e_b1"], f32)
    eb2 = np.asarray(inputs["e_b2"], f32)
    in_maps = []
    for c in range(N_CORES):
        e, th = c // 2, c % 2
        g0 = th * (B * T // 2) + e * OWN
        if e == 0:
            x_sh = np.concatenate([np.zeros((HALO, D), f32), x[g0:g0 + OWN]])
        else:
            x_sh = x[g0 - HALO:g0 + OWN]
        m = dict(shared)
        m["x_sh"] = np.ascontiguousarray(x_sh)
        m["ew1"] = np.ascontiguousarray(ew1b[e])
        m["eb1"] = np.ascontiguousarray(eb1[e])
        m["ew2"] = np.ascontiguousarray(ew2b[e])
        m["eb2h"] = np.ascontiguousarray(eb2[e])
        esel = np.zeros((128, E), f32)
        esel[:, e] = 1.0
        m["esel"] = esel
        rmask = np.zeros((128, 4), f32)
        rmask[:, e] = 1.0
        m["rmask"] = rmask
        in_maps.append(m)
    return in_maps


def unshard_out(results):
    """results: list of 8 dicts with 'out' [OWN, D]; rows r*128+i of core c
    hold global token (c%2)*2048 + r*512 + (c//2)*128 + i."""
    full = np.empty((B * T, D), np.float32)
    for c in range(N_CORES):
        e, th = c // 2, c % 2
        oc = results[c]["out"]
        for r in range(4):
            full[th * 2048 + r * OWN + e * 128: th * 2048 + r * OWN + (e + 1) * 128] = \
                oc[r * 128:(r + 1) * 128]
    return full.reshape(B, T, D)


_NC_CACHE = {}


def _get_nc():
    if "nc" not in _NC_CACHE:
        _NC_CACHE["nc"] = build(debug_outputs=False)
    return _NC_CACHE["nc"]


def kernel(**inputs) -> np.ndarray:
    """Full-input entry point: shards across 8 NeuronCores, runs the Bass
    kernel SPMD, reassembles the full [2, 2048, 1024] output."""
    import sys, types
    try:  # NTFF profile hook shim (missing antenv.axon_hooks in this image)
        import antenv.axon_hooks  # noqa: F401
    except ImportError:
        try:
            import antenv
            from trn_agent_boot.trn_boot import _ntff_profile_via_ctypes
            mod = types.ModuleType("antenv.axon_hooks")
            try:
                _hook = _ntff_profile_via_ctypes("/opt/axon/libaxon_pjrt.so")
            except Exception:
                _hook = None
            mod.get_axon_ntff_profile_hook = lambda: _hook
            mod.set_axon_ntff_profile_hook = lambda h: None
            sys.modules["antenv.axon_hooks"] = mod
            antenv.axon_hooks = mod
        except Exception:
            pass
    from concourse.bass_utils import run_bass_kernel_spmd

    nc = _get_nc()
    in_maps = host_prep(inputs)
    res = run_bass_kernel_spmd(nc, in_maps, core_ids=list(range(N_CORES)))
    out = unshard_out(res.results)
    return out.astype(np.float32)


# revision 33
# speedup vs baseline: 1.8248x; 1.1787x over previous
"""Bass kernel builder for nn_MixtureOfMambaBlock — 8-core SPMD, v2 (bf16).

Sharding: tokens 8-way (512/core + 64 halo for conv+scan warmup); mixer fully
local per core (weights replicated, bf16). Post-mixer h2 all-gathered (bf16),
MoE expert x token-half sharded (bf16 weights SBUF-resident), weighted partials
reduce-scattered (bf16) back to token shards; residual added locally after RS.
"""
import numpy as np
import concourse.bass as bass
import concourse.bacc as bacc
import concourse.mybir as mybir
import concourse.tile as tile

FP = mybir.dt.float32
FR = mybir.dt.float32r
BF = mybir.dt.bfloat16
F8 = mybir.dt.float8e4
W1SCALE = 64.0
AF = mybir.ActivationFunctionType
ALU = mybir.AluOpType

B, T, D = 2, 2048, 1024
S, INNER = 64, 2048
E = 4
HID = 4096
OWN, HALO = 512, 64
NH = OWN + HALO          # 576
KB = D // 128            # 8  d-blocks
MB = INNER // 128        # 16 inner-blocks
HB = HID // 128          # 32 hid-blocks
OTB = OWN // 128         # 4  own-token blocks
N_CORES = 8

INPUT_SPECS = {
    "x_sh": ([NH, D], FP),
    "ipw": ([D, 2 * INNER], BF), "ipb": ([2 * INNER], FP),
    "cw": ([INNER, 3], FP), "cb": ([INNER], FP),
    "dtw": ([INNER, S], BF), "dtb": ([S], FP),
    "bpw": ([INNER, S], BF), "bpb": ([S], FP),
    "cpw": ([INNER, S], BF), "cpb": ([S], FP),
    "s2iw": ([S, INNER], BF), "s2ib": ([INNER], FP),
    "Dp": ([INNER], FP),
    "ow": ([INNER, D], BF), "ob": ([D], FR),
    "gw": ([D, E], FP), "gb": ([E], FR),
    "ew1": ([128, KB // 2 * HID // 128 * 256], F8), "eb1": ([HID], FP),
    "ew2": ([HID, D], BF), "eb2h": ([D], FR),
    "esel": ([128, E], FP),
    "rmask": ([128, 4], FP),
    "ident": ([128, 128], FP),
    "identb": ([128, 128], BF),
    "ones1": ([1, 128], FR),
}


def build(debug_outputs=False):
    nc = bacc.Bacc("TRN2", target_bir_lowering=False, debug=False,
                   num_devices=N_CORES)
    dp = {}
    for name, (shape, dt) in INPUT_SPECS.items():
        dp[name] = nc.dram_tensor(name, shape, dt, kind="ExternalInput")
    out_d = nc.dram_tensor("out", [OWN, D], FP, kind="ExternalOutput")
    dbg = {}
    if debug_outputs:
        dbg["xmid"] = nc.dram_tensor("dbg_xmid", [OWN, D], FP, kind="ExternalOutput")
        dbg["wown"] = nc.dram_tensor("dbg_wown", [OWN, E], FP, kind="ExternalOutput")

    rg = [[0, 2, 4, 6], [1, 3, 5, 7]]

    with tile.TileContext(nc) as tc:
        with (
            tc.tile_pool(name="outer", bufs=1) as po,
            tc.tile_pool(name="dram", bufs=1, space="DRAM") as pdram,
        ):
            # ---------- DRAM bounce buffers for collectives ----------
            gth_in = [pdram.tile([D, 128], F8, name=f"gth_in{t_}") for t_ in range(OTB)]
            gth_out = [pdram.tile([4 * D, 128], F8, name=f"gth_out{t_}")
                       for t_ in range(OTB)]
            gtw_in = pdram.tile([OWN, E], FP)
            gtw_out = pdram.tile([4 * OWN, E], FP)
            rs_in = [pdram.tile([OWN, D], BF, name=f"rs_in{r}") for r in range(4)]
            rs_out = [pdram.tile([128, D], BF, name=f"rs_out{r}") for r in range(4)]

            # ---------- constants / small weights (emit all loads up front) ----
            ident = po.tile([128, 128], FP)
            nc.sync.dma_start(ident[:], dp["ident"][:])
            identb = po.tile([128, 128], BF)
            nc.sync.dma_start(identb[:], dp["identb"][:])

            def load_pcol(name, blocks):  # [blocks*128] -> [128, blocks]
                t = po.tile([128, blocks], FP, name=f"{name}_sb")
                nc.sync.dma_start(
                    t[:], dp[name].ap().rearrange("(m p) -> p m", p=128))
                return t

            def load_vec1(name, n):  # [n] -> [n, 1]
                t = po.tile([n, 1], FP, name=f"{name}_sb")
                nc.sync.dma_start(t[:], dp[name].ap().rearrange("(s o) -> s o", o=1))
                return t

            def load_row(name, n, dt_=FP):  # [n] -> [1, n]
                t = po.tile([1, n], dt_, name=f"{name}_sb")
                nc.sync.dma_start(t[:], dp[name].ap().rearrange("(o s) -> o s", o=1))
                return t

            def load_kw(name, pool):  # [2048, 64] -> [128, 16, 64], lhsT slice [:, kb, :]
                t = pool.tile([128, MB, S], BF, name=f"{name}_sb")
                nc.sync.dma_start(t[:], dp[name].ap().rearrange("(kb p) s -> p kb s", p=128))
                return t

            ob_sb = load_row("ob", D, FR)
            gb_sb = load_row("gb", E, FR)
            eb2h_sb = load_row("eb2h", D, FR)
            ones1 = po.tile([1, 128], FR)
            nc.sync.dma_start(ones1[:], dp["ones1"][:])
            ipb_sb = load_pcol("ipb", 32)
            cb_sb = load_pcol("cb", 16)
            cw_sb = po.tile([128, 16, 3], FP)  # [p, m, k]
            nc.sync.dma_start(cw_sb[:], dp["cw"].ap().rearrange("(m p) k -> p m k", p=128))
            dtb_sb = load_vec1("dtb", S)
            bpb_sb = load_vec1("bpb", S)
            cpb_sb = load_vec1("cpb", S)
            s2ib_sb = load_pcol("s2ib", 16)
            Dp_sb = load_pcol("Dp", 16)
            gw_sb = po.tile([128, KB, E], FP)  # [p, kb, e]
            nc.sync.dma_start(gw_sb[:], dp["gw"].ap().rearrange("(kb p) e -> p kb e", p=128))
            esel = po.tile([128, E], FP)
            nc.sync.dma_start(esel[:], dp["esel"][:])
            rmask = po.tile([128, 4], FP)
            nc.sync.dma_start(rmask[:], dp["rmask"][:])
            eb1_sb = load_pcol("eb1", HB)

            # persistent activations
            xmid = [po.tile([128, D], FP, name=f"xmid{t_}", tag=f"xmid{t_}")
                    for t_ in range(OTB)]

            # =======================================================
            # MIXER
            # =======================================================
            with (
                tc.tile_pool(name="mixer", bufs=1) as pm,
                tc.tile_pool(name="mixt", bufs=1) as pt_pool,
            ):
                # pool allocation order matters: tiles that die early (hT, xm,
                # sg, projection weights) go FIRST so their addresses sit at the
                # pool base — the MoE pool's ew1 tiles (allocated first there)
                # land on them and can start loading before outproj finishes.
                hT = [pm.tile([128, NH], BF, name=f"hT{kb}", tag=f"hT{kb}") for kb in range(KB)]
                xm = [pm.tile([128, NH], BF, name=f"xm{m}", tag=f"xm{m}") for m in range(MB)]
                sg = [pm.tile([128, OWN], BF, name=f"sg{m}", tag=f"sg{m}")
                      for m in range(8)]
                dtw_sb = load_kw("dtw", pm)
                bpw_sb = load_kw("bpw", pm)
                cpw_sb = load_kw("cpw", pm)
                s2iw_sb = pm.tile([S, INNER], BF, name="s2iw_sb")
                nc.sync.dma_start(s2iw_sb[:], dp["s2iw"][:])
                # late-freed tiles (used through outproj) at higher addresses
                ow_sb = pm.tile([128, MB, D], BF, name="ow_sb")
                nc.scalar.dma_start(
                    ow_sb[:], dp["ow"].ap().rearrange("(kb p) d -> p kb d", p=128))
                xo = [pm.tile([128, D], FP, name=f"xo{t_}", tag=f"xo{t_}")
                      for t_ in range(OTB)]

                # ---- rmsnorm1 + transpose to hT ----
                # chunks: [64 halo] + 4x [128 own]
                chunks = [(0, HALO, None)] + [
                    (HALO + t_ * 128, 128, t_) for t_ in range(OTB)]
                with nc.named_scope("rms1"), tc.tile_pool(name="ps1", bufs=1, space="PSUM") as psA:
                    for (row0, rows, t_) in chunks:
                        if t_ is None:
                            xt = pt_pool.tile([HALO, D], FP, tag="xt0")
                        else:
                            xt = xo[t_]
                        # gpsimd queue: ahead of the ipw weight chunks, and not
                        # behind the ~20 small constant loads on the sync queue
                        nc.gpsimd.dma_start(xt[:], dp["x_sh"][row0:row0 + rows, :])
                        scr = pt_pool.tile([128, D], FP, tag="scr", bufs=1)
                        sq = pt_pool.tile([128, 1], FP, tag="sq", bufs=2)
                        nc.scalar.activation(scr[0:rows, :], xt[:], AF.Square,
                                             accum_out=sq[0:rows, :])
                        nr = pt_pool.tile([128, 1], FP, tag="nr", bufs=2)
                        nc.vector.tensor_scalar(nr[0:rows, :], sq[0:rows, :], 1.0 / D,
                                                1e-6, ALU.mult, ALU.add)
                        nc.scalar.sqrt(nr[0:rows, :], nr[0:rows, :])
                        nc.vector.reciprocal(nr[0:rows, :], nr[0:rows, :])
                        h_t = pt_pool.tile([128, D], BF, tag="htb", bufs=2)
                        nc.vector.tensor_scalar(h_t[0:rows, :], xt[:], nr[0:rows, :],
                                                None, ALU.mult)
                        for kb in range(KB):
                            ptr = psA.tile([128, 128], BF, tag="ptr", bufs=2)
                            nc.tensor.transpose(ptr[:, 0:rows],
                                                h_t[0:rows, kb * 128:(kb + 1) * 128],
                                                identb[0:rows, 0:rows])
                            nc.vector.tensor_copy(hT[kb][:, row0:row0 + rows],
                                                  ptr[:, 0:rows])

                # ---- in_proj (x_main half) + conv + silu ----
                with nc.named_scope("in_proj"), tc.tile_pool(name="ps2", bufs=1, space="PSUM") as psA:
                    for q in range(4):
                        wq = []
                        for kb in range(KB):
                            wt = pt_pool.tile([128, 512], BF, tag=f"wip{kb}", bufs=2,
                                              name=f"wip{kb}")
                            nc.gpsimd.dma_start(
                                wt[:], dp["ipw"][kb * 128:(kb + 1) * 128,
                                                 q * 512:(q + 1) * 512])
                            wq.append(wt)
                        for mi in range(4):
                            m = q * 4 + mi
                            xzp = pt_pool.tile([128, NH + 2], FP, tag="xzp", bufs=2)
                            nc.vector.memset(xzp[:, 0:2], 0.0)
                            for n0, nw in ((0, 512), (512, NH - 512)):
                                px = psA.tile([128, 512], FP, tag="px", bufs=2)
                                for kb in range(KB):
                                    nc.tensor.matmul(px[:, 0:nw],
                                                     wq[kb][:, mi * 128:(mi + 1) * 128],
                                                     hT[kb][:, n0:n0 + nw],
                                                     start=(kb == 0), stop=(kb == KB - 1))
                                nc.scalar.activation(xzp[:, 2 + n0:2 + n0 + nw], px[:, 0:nw],
                                                     AF.Identity, bias=ipb_sb[:, m:m + 1])
                            cv = pt_pool.tile([128, NH], FP, tag="cv", bufs=2)
                            nc.vector.tensor_scalar(cv[:], xzp[:, 0:NH], cw_sb[:, m, 0:1],
                                                    None, ALU.mult)
                            nc.vector.scalar_tensor_tensor(cv[:], xzp[:, 1:1 + NH],
                                                           cw_sb[:, m, 1:2], cv[:],
                                                           ALU.mult, ALU.add)
                            nc.vector.scalar_tensor_tensor(cv[:], xzp[:, 2:2 + NH],
                                                           cw_sb[:, m, 2:3], cv[:],
                                                           ALU.mult, ALU.add)
                            sgc = pt_pool.tile([128, NH], BF, tag="sgc", bufs=2)
                            nc.scalar.activation(sgc[:], cv[:], AF.Sigmoid, bias=cb_sb[:, m:m + 1])
                            nc.vector.scalar_tensor_tensor(xm[m][:], cv[:], cb_sb[:, m:m + 1],
                                                           sgc[:], ALU.add, ALU.mult)

                # ---- dt/B/C projections (emitted before gate MMs; feed scan) ----
                with nc.named_scope("scan"), tc.tile_pool(name="ps3", bufs=1, space="PSUM") as psA:
                    dt_t = pt_pool.tile([S, NH], FP, tag="dt")
                    a_t = pt_pool.tile([S, NH], FP, tag="a")
                    b_t = pt_pool.tile([S, NH], FP, tag="b")
                    c_t = pt_pool.tile([S, NH], FP, tag="c")
                    for n0, nw in ((0, 512), (512, NH - 512)):
                        for wsb, bias_sb, dst, fn in (
                            (dtw_sb, dtb_sb, dt_t, AF.Sigmoid),
                            (cpw_sb, cpb_sb, c_t, AF.Identity),
                        ):
                            pz = psA.tile([S, 512], FP, tag="pz", bufs=2)
                            for kb in range(MB):
                                nc.tensor.matmul(pz[:, 0:nw], wsb[:, kb, :],
                                                 xm[kb][:, n0:n0 + nw],
                                                 start=(kb == 0), stop=(kb == MB - 1))
                            nc.scalar.activation(dst[:, n0:n0 + nw], pz[:, 0:nw], fn,
                                                 bias=bias_sb[:])
                        # b needs dt -> separate pass
                        pz = psA.tile([S, 512], FP, tag="pz", bufs=2)
                        for kb in range(MB):
                            nc.tensor.matmul(pz[:, 0:nw], bpw_sb[:, kb, :],
                                             xm[kb][:, n0:n0 + nw],
                                             start=(kb == 0), stop=(kb == MB - 1))
                        nc.vector.scalar_tensor_tensor(b_t[:, n0:n0 + nw], pz[:, 0:nw],
                                                       bpb_sb[:], dt_t[:, n0:n0 + nw],
                                                       ALU.add, ALU.mult)
                    # scan runs on the vector engine while the tensor engine
                    # works through the gate-projection matmuls below
                    nc.vector.tensor_scalar(a_t[:], dt_t[:], -1.0, 1.0,
                                            ALU.mult, ALU.add)
                    st_t = pt_pool.tile([S, NH], FP, tag="st")
                    nc.vector.tensor_tensor_scan(st_t[:], a_t[:], b_t[:], 0.0,
                                                 ALU.mult, ALU.add)
                    y_t = pt_pool.tile([S, OWN], FP, tag="yt", name="y_t")
                    nc.vector.tensor_mul(y_t[:], c_t[:, HALO:NH], st_t[:, HALO:NH])

                # ---- gate half of in_proj, first 8 m: emitted NOW so the
                # tensor queue has work while the (vector-engine) scan runs ----
                def load_wqg(q):
                    wqg = []
                    for kb in range(KB):
                        wt = pt_pool.tile([128, 512], BF, tag=f"wip{kb}", bufs=2,
                                          name=f"wipg{kb}_{q}")
                        nc.gpsimd.dma_start(
                            wt[:], dp["ipw"][kb * 128:(kb + 1) * 128,
                                             2048 + q * 512:2048 + (q + 1) * 512])
                        wqg.append(wt)
                    return wqg

                def gate_mm(psB, wqg, m):
                    mi = m % 4
                    pg = psB.tile([128, 512], FP, tag="pg", bufs=2)
                    for kb in range(KB):
                        nc.tensor.matmul(pg[:], wqg[kb][:, mi * 128:(mi + 1) * 128],
                                         hT[kb][:, HALO:NH],
                                         start=(kb == 0), stop=(kb == KB - 1))
                    return pg

                with nc.named_scope("gateproj"), tc.tile_pool(name="ps4", bufs=1, space="PSUM") as psB:
                    for q in range(2):
                        wqg = load_wqg(q)
                        for mi in range(4):
                            m = q * 4 + mi
                            pg = gate_mm(psB, wqg, m)
                            nc.scalar.activation(sg[m][:], pg[:], AF.Sigmoid,
                                                 bias=ipb_sb[:, MB + m:MB + m + 1])

                # ---- layernorm over S ----
                with nc.named_scope("scanln"), tc.tile_pool(name="ps5", bufs=1, space="PSUM") as psA:
                    yln = pt_pool.tile([S, OWN], BF, tag="a", name="yln")
                    for i in range(OTB):
                        ptr = psA.tile([128, 128], FP, tag="ptr", bufs=2)
                        nc.tensor.transpose(ptr[:, 0:S], y_t[:, i * 128:(i + 1) * 128],
                                            ident[0:S, 0:S])
                        yT = pt_pool.tile([128, S], FP, tag="yT", bufs=2)
                        nc.vector.tensor_copy(yT[:], ptr[:, 0:S])
                        mu = pt_pool.tile([128, 1], FP, tag="mu", bufs=2)
                        nc.vector.tensor_reduce(mu[:], yT[:], mybir.AxisListType.X, ALU.add)
                        nc.vector.tensor_scalar_mul(mu[:], mu[:], 1.0 / S)
                        xc = pt_pool.tile([128, S], FP, tag="xc", bufs=2)
                        nc.vector.tensor_scalar_sub(xc[:], yT[:], mu[:])
                        scr2 = pt_pool.tile([128, S], FP, tag="scr2", bufs=2)
                        vv = pt_pool.tile([128, 1], FP, tag="vv", bufs=2)
                        nc.scalar.activation(scr2[:], xc[:], AF.Square, accum_out=vv[:])
                        nc.vector.tensor_scalar(vv[:], vv[:], 1.0 / S, 1e-5, ALU.mult, ALU.add)
                        nc.scalar.sqrt(vv[:], vv[:])
                        nc.vector.reciprocal(vv[:], vv[:])
                        xcb = pt_pool.tile([128, S], BF, tag="xcb", bufs=2)
                        nc.vector.tensor_scalar_mul(xcb[:], xc[:], vv[:])
                        ptr2 = psA.tile([128, 128], BF, tag="ptr2", bufs=2)
                        nc.tensor.transpose(ptr2[0:S, :], xcb[:], identb[:])
                        nc.vector.tensor_copy(yln[:, i * 128:(i + 1) * 128], ptr2[0:S, :])

                # ---- s2i + pre_out assembly (gate m>=8 computed inline) ----
                with nc.named_scope("premix"), tc.tile_pool(name="ps6", bufs=1, space="PSUM") as psA:
                    pre = []
                    for m in range(MB):
                        if m >= 8:
                            if m % 4 == 0:
                                wqg = load_wqg(m // 4)
                            pg = gate_mm(psA, wqg, m)
                            sg_m = pt_pool.tile([128, OWN], BF, tag="sgi", bufs=2)
                            nc.scalar.activation(sg_m[:], pg[:], AF.Sigmoid,
                                                 bias=ipb_sb[:, MB + m:MB + m + 1])
                        else:
                            sg_m = sg[m]
                        ps = psA.tile([128, 512], FP, tag="ps", bufs=2)
                        nc.tensor.matmul(ps[:], s2iw_sb[:, m * 128:(m + 1) * 128], yln[:],
                                         start=True, stop=True)
                        # Dp*xm + s2ib on the scalar engine; 2 vector ops total
                        tmp = pt_pool.tile([128, OWN], FP, tag="tmp", bufs=2)
                        nc.scalar.activation(tmp[:], xm[m][:, HALO:NH], AF.Identity,
                                             bias=s2ib_sb[:, m:m + 1],
                                             scale=Dp_sb[:, m:m + 1])
                        nc.vector.tensor_add(tmp[:], tmp[:], ps[:])
                        pre_m = pm.tile([128, OWN], BF, name=f"pre{m}", tag=f"pre{m}")
                        nc.vector.tensor_mul(pre_m[:], tmp[:], sg_m[:])
                        pre.append(pre_m)

                # ---- per-tb: out projection + residual + rms2 + h2T + gating + gather ----
                # emission interleave: po2 matmuls of tb+1 are queued before the
                # gating chain of tb, so the tensor engine has work while the
                # rms2/transpose chain for tb runs on scalar/vector
                with nc.named_scope("outproj"), tc.tile_pool(name="ps7", bufs=1, space="PSUM") as psA:
                    def emit_po2(tb):
                        po2 = psA.tile([128, 2, 512], FP, tag="po2", bufs=2)
                        for kb in range(MB):
                            for nb in range(2):
                                nc.tensor.matmul(po2[:, nb, :],
                                                 pre[kb][:, tb * 128:(tb + 1) * 128],
                                                 ow_sb[:, kb, nb * 512:(nb + 1) * 512],
                                                 start=(kb == 0), stop=False)
                        for nb in range(2):
                            nc.tensor.matmul(po2[:, nb, :], ones1[:],
                                             ob_sb[:, nb * 512:(nb + 1) * 512],
                                             start=False, stop=True)
                            nc.vector.tensor_add(xmid[tb][:, nb * 512:(nb + 1) * 512],
                                                 po2[:, nb, :],
                                                 xo[tb][:, nb * 512:(nb + 1) * 512])

                    def emit_gate(tb):
                        # rms2 for this tb
                        scr = pt_pool.tile([128, D], FP, tag="scr", bufs=1)
                        sq = pt_pool.tile([128, 1], FP, tag="sq", bufs=2)
                        nc.scalar.activation(scr[:], xmid[tb][:], AF.Square, accum_out=sq[:])
                        nr = pt_pool.tile([128, 1], FP, tag="nr", bufs=2)
                        nc.vector.tensor_scalar(nr[:], sq[:], 1.0 / D, 1e-6, ALU.mult, ALU.add)
                        nc.scalar.sqrt(nr[:], nr[:])
                        nc.vector.reciprocal(nr[:], nr[:])
                        h2 = pt_pool.tile([128, D], FP, tag="h2", bufs=2, name="h2")
                        nc.vector.tensor_scalar(h2[:], xmid[tb][:], nr[:], None, ALU.mult)
                        # gating logits must be fp32: bf16 logits flip top-2
                        # selections vs the reference on near-ties (~0.15 abs
                        # error per flipped token)
                        pl = psA.tile([128, E], FP, tag="pl", bufs=2)
                        for kb in range(KB):
                            ptr = psA.tile([128, 128], FP, tag="ptr", bufs=2)
                            nc.tensor.transpose(ptr[:], h2[:, kb * 128:(kb + 1) * 128],
                                                ident[:])
                            h2T_t = pt_pool.tile([128, 128], FP, tag="h2T", bufs=2)
                            nc.vector.tensor_copy(h2T_t[:], ptr[:])
                            h2T_8 = pt_pool.tile([128, 128], F8, tag="h2T8", bufs=2)
                            nc.vector.tensor_copy(h2T_8[:], h2T_t[:])
                            nc.sync.dma_start(
                                gth_in[tb][kb * 128:(kb + 1) * 128, :], h2T_8[:])
                            nc.tensor.matmul(pl[:], h2T_t[:], gw_sb[:, kb, :],
                                             start=(kb == 0), stop=False)
                        nc.tensor.matmul(pl[:], ones1[:], gb_sb[:], start=False, stop=True)
                        # top-2-of-4 gating
                        m1 = pt_pool.tile([128, 1], FP, tag="m1", bufs=2)
                        nc.vector.tensor_reduce(m1[:], pl[:], mybir.AxisListType.X, ALU.max)
                        eq1 = pt_pool.tile([128, E], FP, tag="eq1", bufs=2)
                        nc.vector.tensor_scalar(eq1[:], pl[:], m1[:], None, ALU.is_equal)
                        msk = pt_pool.tile([128, E], FP, tag="msk", bufs=2)
                        nc.vector.scalar_tensor_tensor(msk[:], eq1[:], -1e30, pl[:],
                                                       ALU.mult, ALU.add)
                        m2 = pt_pool.tile([128, 1], FP, tag="m2", bufs=2)
                        nc.vector.tensor_reduce(m2[:], msk[:], mybir.AxisListType.X, ALU.max)
                        eq2 = pt_pool.tile([128, E], FP, tag="eq2", bufs=2)
                        nc.vector.tensor_scalar(eq2[:], msk[:], m2[:], None, ALU.is_equal)
                        dd = pt_pool.tile([128, 1], FP, tag="dd", bufs=2)
                        nc.vector.tensor_sub(dd[:], m2[:], m1[:])
                        p2 = pt_pool.tile([128, 1], FP, tag="p2", bufs=2)
                        nc.scalar.activation(p2[:], dd[:], AF.Sigmoid)
                        p1b = pt_pool.tile([128, 1], FP, tag="p1b", bufs=2)
                        nc.scalar.activation(p1b[:], p2[:], AF.Identity, bias=1.0, scale=-1.0)
                        wv = pt_pool.tile([128, E], FP, tag="wv", bufs=2)
                        nc.vector.tensor_scalar(wv[:], eq1[:], p1b[:], None, ALU.mult)
                        nc.vector.scalar_tensor_tensor(wv[:], eq2[:], p2[:], wv[:],
                                                       ALU.mult, ALU.add)
                        nc.sync.dma_start(gtw_in[tb * 128:(tb + 1) * 128, :], wv[:])
                        if debug_outputs:
                            nc.sync.dma_start(dbg["wown"][tb * 128:(tb + 1) * 128, :], wv[:])
                            nc.sync.dma_start(dbg["xmid"][tb * 128:(tb + 1) * 128, :],
                                              xmid[tb][:])
                        nc.gpsimd.collective_compute(
                            "AllGather", ALU.bypass, replica_groups=rg,
                            ins=[gth_in[tb].opt()], outs=[gth_out[tb].opt()])

                    for tb in range(OTB):
                        emit_po2(tb)
                        if tb >= 1:
                            emit_gate(tb - 1)
                    emit_gate(OTB - 1)
                    with nc.named_scope("gatherw"):
                        nc.gpsimd.collective_compute(
                            "AllGather", ALU.bypass, replica_groups=rg,
                            ins=[gtw_in.opt()], outs=[gtw_out.opt()])

            # =======================================================
            # MoE (full expert per core, token-half group of 4)
            # =======================================================
            with (
                tc.tile_pool(name="moe", bufs=1) as pq,
                tc.tile_pool(name="psC", bufs=1, space="PSUM") as psC,
            ):
                # expert weights resident in SBUF for all 4 rounds.
                # w1 runs in fp8 DoubleRow: ew1 arrives pre-scaled by W1SCALE and
                # host-interleaved to [p, h, two, m] per k-pair so each LDWEIGHTS
                # slice [128, 2, 128] is contiguous (strided pair dims fault the PE).
                ew1_sb = [pq.tile([128, HB, 2, 128], F8, name=f"ew1_{i}", tag=f"ew1_{i}")
                          for i in range(KB // 2)]
                for i in range(KB // 2):
                    nc.scalar.dma_start(
                        ew1_sb[i][:], dp["ew1"][:, i * (HB * 256):(i + 1) * (HB * 256)])
                ew2_sb = [pq.tile([128, D], BF, name=f"ew2_{j}", tag=f"ew2_{j}")
                          for j in range(HB)]
                for j in range(HB):
                    nc.scalar.dma_start(ew2_sb[j][:], dp["ew2"][j * 128:(j + 1) * 128, :])

                with nc.named_scope("moe"):
                    for r in range(4):
                        h2r = []
                        for i in range(KB // 2):
                            t = pq.tile([128, 2, OWN], F8, tag=f"h2r{i}", bufs=1)
                            for two in range(2):
                                kb = 2 * i + two
                                for t_ in range(OTB):
                                    nc.gpsimd.dma_start(
                                        t[:, two, t_ * 128:(t_ + 1) * 128],
                                        gth_out[t_][r * D + kb * 128: r * D + (kb + 1) * 128, :])
                            h2r.append(t)
                        hid = []
                        for h in range(HB):
                            ph = psC.tile([128, 512], FP, tag="ph", bufs=2)
                            for i in range(KB // 2):
                                nc.tensor.matmul(ph[:], ew1_sb[i][:, h, :, :],
                                                 h2r[i][:, :, :],
                                                 start=(i == 0), stop=(i == KB // 2 - 1),
                                                 perf_mode=mybir.MatmulPerfMode.DoubleRow)
                            ht = pq.tile([128, OWN], BF, tag=f"hid{h}", bufs=1)
                            nc.scalar.activation(ht[:], ph[:], AF.Gelu, bias=eb1_sb[:, h:h + 1],
                                                 scale=1.0 / W1SCALE)
                            hid.append(ht)
                        # per-token weight for this core's expert
                        wvr = pq.tile([128, OTB, E], FP, tag="wvr", bufs=2)
                        nc.sync.dma_start(
                            wvr[:], gtw_out[r * OWN:(r + 1) * OWN, :]
                            .rearrange("(tb p) e -> p tb e", p=128))
                        ws = []
                        for tb in range(OTB):
                            wm_t = pq.tile([128, E], FP, tag="wm", bufs=2)
                            nc.vector.tensor_mul(wm_t[:], wvr[:, tb, :], esel[:])
                            ws_t = pq.tile([128, 1], FP, tag=f"ws{tb}", bufs=2)
                            nc.vector.tensor_reduce(ws_t[:], wm_t[:], mybir.AxisListType.X,
                                                    ALU.add)
                            ws.append(ws_t)
                        # w2: token-block pairs keep PSUM <= 6 banks
                        for tp in range(2):
                            peo = [psC.tile([128, 2, 512], FP, tag=f"peo{ti}", bufs=1,
                                            name=f"peo{ti}") for ti in range(2)]
                            for h in range(HB):
                                for ti in range(2):
                                    tb = tp * 2 + ti
                                    for nb in range(2):
                                        nc.tensor.matmul(
                                            peo[ti][:, nb, :],
                                            hid[h][:, tb * 128:(tb + 1) * 128],
                                            ew2_sb[h][:, nb * 512:(nb + 1) * 512],
                                            start=(h == 0), stop=False)
                            for ti in range(2):
                                tb = tp * 2 + ti
                                wout = pq.tile([128, D], BF, tag="wout", bufs=2)
                                for nb in range(2):
                                    nc.tensor.matmul(peo[ti][:, nb, :], ones1[:],
                                                     eb2h_sb[:, nb * 512:(nb + 1) * 512],
                                                     start=False, stop=True)
                                    n0 = nb * 512
                                    nc.vector.tensor_scalar(wout[:, n0:n0 + 512],
                                                            peo[ti][:, nb, :],
                                                            ws[tb][:], None, ALU.mult)
                                    # owner (r == e) carries the residual through
                                    # the reduce-scatter
                                    nc.vector.scalar_tensor_tensor(
                                        wout[:, n0:n0 + 512],
                                        xmid[tb][:, n0:n0 + 512], rmask[:, r:r + 1],
                                        wout[:, n0:n0 + 512], ALU.mult, ALU.add)
                                nc.sync.dma_start(
                                    rs_in[r][tb * 128:(tb + 1) * 128, :], wout[:])
                        nc.gpsimd.collective_compute(
                            "ReduceScatter", ALU.add, replica_groups=rg,
                            ins=[rs_in[r].opt()], outs=[rs_out[r].opt()])

                with nc.named_scope("final"):
                    for r in range(4):
                        rsb = pq.tile([128, D], BF, tag="rsb", bufs=2)
                        nc.sync.dma_start(rsb[:], rs_out[r][:])
                        osb = pq.tile([128, D], FP, tag="osb", bufs=1)
                        nc.vector.tensor_copy(osb[:], rsb[:])
                        nc.sync.dma_start(out_d[r * 128:(r + 1) * 128, :], osb[:])

    nc.compile()
    return nc


def host_prep(inputs):
    """Build the 8 per-core input maps from full inputs."""
    import ml_dtypes
    f32 = np.float32
    bf = ml_dtypes.bfloat16
    x = np.ascontiguousarray(np.asarray(inputs["x"], f32).reshape(B * T, D))
    n1 = np.asarray(inputs["norm1_w"], f32)
    n2 = np.asarray(inputs["norm2_w"], f32)
    ipw = np.ascontiguousarray(np.asarray(inputs["in_proj_w"], f32) * n1[:, None]).astype(bf)
    gw = np.ascontiguousarray(np.asarray(inputs["gate_w"], f32) * n2[:, None])
    ew1f = np.asarray(inputs["e_w1"], f32) * n2[None, :, None]
    ew1q = np.clip(ew1f * 64.0, -240.0, 240.0).astype(ml_dtypes.float8_e4m3)
    # [E, k, hid] -> [E, p, i, h, two, m]: k = i*256 + two*128 + p, hid = h*128 + m
    ew1b = ew1q.reshape(E, 4, 2, 128, HID // 128, 128).transpose(0, 3, 1, 4, 2, 5)
    ew1b = np.ascontiguousarray(ew1b.reshape(E, 128, -1))
    ew2b = np.asarray(inputs["e_w2"], f32).astype(bf)
    ident = np.eye(128, dtype=f32)
    ones1 = np.ones((1, 128), f32)
    shared = {
        "ipw": ipw, "ipb": np.asarray(inputs["in_proj_b"], f32),
        "cw": np.ascontiguousarray(np.asarray(inputs["conv_w"], f32)[:, 0, :]),
        "cb": np.asarray(inputs["conv_b"], f32),
        "dtw": np.asarray(inputs["dt_w"], f32).astype(bf),
        "dtb": np.asarray(inputs["dt_b"], f32),
        "bpw": np.asarray(inputs["bp_w"], f32).astype(bf),
        "bpb": np.asarray(inputs["bp_b"], f32),
        "cpw": np.asarray(inputs["cp_w"], f32).astype(bf),
        "cpb": np.asarray(inputs["cp_b"], f32),
        "s2iw": np.asarray(inputs["s2i_w"], f32).astype(bf),
        "s2ib": np.asarray(inputs["s2i_b"], f32),
        "Dp": np.asarray(inputs["D_param"], f32),
        "ow": np.asarray(inputs["out_w"], f32).astype(bf),
        "ob": np.asarray(inputs["out_b"], f32),
        "gw": gw, "gb": np.asarray(inputs["gate_b"], f32),
        "ident": ident, "identb": ident.astype(bf), "ones1": ones1,
    }
    eb1 = np.asarray(inputs["e_b1"], f32)
    eb2 = np.asarray(inputs["e_b2"], f32)
    in_maps = []
    for c in range(N_CORES):
        e, th = c // 2, c % 2
        g0 = th * (B * T // 2) + e * OWN
        if e == 0:
            x_sh = np.concatenate([np.zeros((HALO, D), f32), x[g0:g0 + OWN]])
        else:
            x_sh = x[g0 - HALO:g0 + OWN]
        m = dict(shared)
        m["x_sh"] = np.ascontiguousarray(x_sh)
        m["ew1"] = np.ascontiguousarray(ew1b[e])
        m["eb1"] = np.ascontiguousarray(eb1[e])
        m["ew2"] = np.ascontiguousarray(ew2b[e])
        m["eb2h"] = np.ascontiguousarray(eb2[e])
        esel = np.zeros((128, E), f32)
        esel[:, e] = 1.0
        m["esel"] = esel
        rmask = np.zeros((128, 4), f32)
        rmask[:, e] = 1.0
        m["rmask"] = rmask
        in_maps.append(m)
    return in_maps


def unshard_out(results):
    """results: list of 8 dicts with 'out' [OWN, D]; rows r*128+i of core c
    hold global token (c%2)*2048 + r*512 + (c//2)*128 + i."""
    full = np.empty((B * T, D), np.float32)
    for c in range(N_CORES):
        e, th = c // 2, c % 2
        oc = results[c]["out"]
        for r in range(4):
            full[th * 2048 + r * OWN + e * 128: th * 2048 + r * OWN + (e + 1) * 128] = \
                oc[r * 128:(r + 1) * 128]
    return full.reshape(B, T, D)


_NC_CACHE = {}


def _get_nc():
    if "nc" not in _NC_CACHE:
        _NC_CACHE["nc"] = build(debug_outputs=False)
    return _NC_CACHE["nc"]


def kernel(**inputs) -> np.ndarray:
    """Full-input entry point: shards across 8 NeuronCores, runs the Bass
    kernel SPMD, reassembles the full [2, 2048, 1024] output."""
    import sys, types
    try:  # NTFF profile hook shim (missing antenv.axon_hooks in this image)
        import antenv.axon_hooks  # noqa: F401
    except ImportError:
        try:
            import antenv
            from trn_agent_boot.trn_boot import _ntff_profile_via_ctypes
            mod = types.ModuleType("antenv.axon_hooks")
            try:
                _hook = _ntff_profile_via_ctypes("/opt/axon/libaxon_pjrt.so")
            except Exception:
                _hook = None
            mod.get_axon_ntff_profile_hook = lambda: _hook
            mod.set_axon_ntff_profile_hook = lambda h: None
            sys.modules["antenv.axon_hooks"] = mod
            antenv.axon_hooks = mod
        except Exception:
            pass
    from concourse.bass_utils import run_bass_kernel_spmd

    nc = _get_nc()
    in_maps = host_prep(inputs)
    res = run_bass_kernel_spmd(nc, in_maps, core_ids=list(range(N_CORES)))
    out = unshard_out(res.results)
    return out.astype(np.float32)


# revision 39
# speedup vs baseline: 1.8504x; 1.0140x over previous
"""Bass kernel builder for nn_MixtureOfMambaBlock — 8-core SPMD.

Sharding: tokens 8-way (512/core + 64 halo for conv+scan warmup); mixer fully
local per core (weights replicated, bf16 matmuls; fp32 gating logits to keep
top-2 selection exact). Post-mixer h2 all-gathered in fp8; MoE is expert x
token-half sharded: w1 runs fp8 DoubleRow (weights pre-scaled x64,
host-interleaved k-pairs), w2 in bf16 with both expert weights SBUF-resident.
Weighted expert partials + residual reduce-scattered in bf16 back to token
shards.
"""
import numpy as np
import concourse.bass as bass
import concourse.bacc as bacc
import concourse.mybir as mybir
import concourse.tile as tile

FP = mybir.dt.float32
FR = mybir.dt.float32r
BF = mybir.dt.bfloat16
F8 = mybir.dt.float8e4
W1SCALE = 64.0
AF = mybir.ActivationFunctionType
ALU = mybir.AluOpType

B, T, D = 2, 2048, 1024
S, INNER = 64, 2048
E = 4
HID = 4096
OWN, HALO = 512, 64
NH = OWN + HALO          # 576
KB = D // 128            # 8  d-blocks
MB = INNER // 128        # 16 inner-blocks
HB = HID // 128          # 32 hid-blocks
OTB = OWN // 128         # 4  own-token blocks
N_CORES = 8

INPUT_SPECS = {
    "x_sh": ([NH, D], FP),
    "ipw": ([D, 2 * INNER], BF), "ipb": ([2 * INNER], FP),
    "cw": ([INNER, 3], FP), "cb": ([INNER], FP),
    "dtw": ([INNER, S], BF), "dtb": ([S], FP),
    "bpw": ([INNER, S], BF), "bpb": ([S], FP),
    "cpw": ([INNER, S], BF), "cpb": ([S], FP),
    "s2iw": ([S, INNER], BF), "s2ib": ([INNER], FP),
    "Dp": ([INNER], FP),
    "ow": ([INNER, D], BF), "ob": ([D], FR),
    "gw": ([D, E], FP), "gb": ([E], FR),
    "ew1": ([128, KB // 2 * HID // 128 * 256], F8), "eb1": ([HID], FP),
    "ew2": ([HID, D], BF), "eb2h": ([D], FR),
    "esel": ([128, E], FP),
    "rmask": ([128, 4], FP),
    "ident": ([128, 128], FP),
    "identb": ([128, 128], BF),
    "ones1": ([1, 128], FR),
}


def build(debug_outputs=False):
    nc = bacc.Bacc("TRN2", target_bir_lowering=False, debug=False,
                   num_devices=N_CORES)
    dp = {}
    for name, (shape, dt) in INPUT_SPECS.items():
        dp[name] = nc.dram_tensor(name, shape, dt, kind="ExternalInput")
    out_d = nc.dram_tensor("out", [OWN, D], FP, kind="ExternalOutput")
    dbg = {}
    if debug_outputs:
        dbg["xmid"] = nc.dram_tensor("dbg_xmid", [OWN, D], FP, kind="ExternalOutput")
        dbg["wown"] = nc.dram_tensor("dbg_wown", [OWN, E], FP, kind="ExternalOutput")

    rg = [[0, 2, 4, 6], [1, 3, 5, 7]]

    with tile.TileContext(nc) as tc:
        with (
            tc.tile_pool(name="outer", bufs=1) as po,
            tc.tile_pool(name="dram", bufs=1, space="DRAM") as pdram,
        ):
            # ---------- DRAM bounce buffers for collectives ----------
            # gth layout per tb: [128 d-in-block, kb*128 tok] — matches the
            # transpose PSUM tile directly (1 staging DMA) and lets the MoE
            # load each peer row-block with a single wide DMA.
            gth_in = [pdram.tile([128, KB * 128], F8, name=f"gth_in{t_}")
                      for t_ in range(OTB)]
            gth_out = [pdram.tile([4 * 128, KB * 128], F8, name=f"gth_out{t_}")
                       for t_ in range(OTB)]
            gtw_in = pdram.tile([OWN, E], FP)
            gtw_out = pdram.tile([4 * OWN, E], FP)
            rs_in = [pdram.tile([OWN, D], BF, name=f"rs_in{r}") for r in range(4)]
            rs_out = [pdram.tile([128, D], BF, name=f"rs_out{r}") for r in range(4)]

            # ---------- constants / small weights (emit all loads up front) ----
            ident = po.tile([128, 128], FP)
            nc.sync.dma_start(ident[:], dp["ident"][:])
            identb = po.tile([128, 128], BF)
            nc.sync.dma_start(identb[:], dp["identb"][:])

            def load_pcol(name, blocks):  # [blocks*128] -> [128, blocks]
                t = po.tile([128, blocks], FP, name=f"{name}_sb")
                nc.sync.dma_start(
                    t[:], dp[name].ap().rearrange("(m p) -> p m", p=128))
                return t

            def load_vec1(name, n):  # [n] -> [n, 1]
                t = po.tile([n, 1], FP, name=f"{name}_sb")
                nc.sync.dma_start(t[:], dp[name].ap().rearrange("(s o) -> s o", o=1))
                return t

            def load_row(name, n, dt_=FP):  # [n] -> [1, n]
                t = po.tile([1, n], dt_, name=f"{name}_sb")
                nc.sync.dma_start(t[:], dp[name].ap().rearrange("(o s) -> o s", o=1))
                return t

            def load_kw(name, pool):  # [2048, 64] -> [128, 16, 64], lhsT slice [:, kb, :]
                t = pool.tile([128, MB, S], BF, name=f"{name}_sb")
                nc.sync.dma_start(t[:], dp[name].ap().rearrange("(kb p) s -> p kb s", p=128))
                return t

            ob_sb = load_row("ob", D, FR)
            gb_sb = load_row("gb", E, FR)
            eb2h_sb = load_row("eb2h", D, FR)
            ones1 = po.tile([1, 128], FR)
            nc.sync.dma_start(ones1[:], dp["ones1"][:])
            ipb_sb = load_pcol("ipb", 32)
            cb_sb = load_pcol("cb", 16)
            cw_sb = po.tile([128, 16, 3], FP)  # [p, m, k]
            nc.sync.dma_start(cw_sb[:], dp["cw"].ap().rearrange("(m p) k -> p m k", p=128))
            dtb_sb = load_vec1("dtb", S)
            bpb_sb = load_vec1("bpb", S)
            cpb_sb = load_vec1("cpb", S)
            s2ib_sb = load_pcol("s2ib", 16)
            Dp_sb = load_pcol("Dp", 16)
            gw_sb = po.tile([128, KB, E], FP)  # [p, kb, e]
            nc.sync.dma_start(gw_sb[:], dp["gw"].ap().rearrange("(kb p) e -> p kb e", p=128))
            esel = po.tile([128, E], FP)
            nc.sync.dma_start(esel[:], dp["esel"][:])
            rmask = po.tile([128, 4], FP)
            nc.sync.dma_start(rmask[:], dp["rmask"][:])
            eb1_sb = load_pcol("eb1", HB)

            # persistent activations
            xmid = [po.tile([128, D], FP, name=f"xmid{t_}", tag=f"xmid{t_}")
                    for t_ in range(OTB)]

            # =======================================================
            # MIXER
            # =======================================================
            with (
                tc.tile_pool(name="mixer", bufs=1) as pm,
                tc.tile_pool(name="mixt", bufs=1) as pt_pool,
            ):
                # pool allocation order matters: tiles that die early (hT, xm,
                # sg, projection weights) go FIRST so their addresses sit at the
                # pool base — the MoE pool's ew1 tiles (allocated first there)
                # land on them and can start loading before outproj finishes.
                hT = [pm.tile([128, NH], BF, name=f"hT{kb}", tag=f"hT{kb}") for kb in range(KB)]
                xm = [pm.tile([128, NH], BF, name=f"xm{m}", tag=f"xm{m}") for m in range(MB)]
                sg = [pm.tile([128, OWN], BF, name=f"sg{m}", tag=f"sg{m}")
                      for m in range(8)]
                dtw_sb = load_kw("dtw", pm)
                bpw_sb = load_kw("bpw", pm)
                cpw_sb = load_kw("cpw", pm)
                s2iw_sb = pm.tile([S, INNER], BF, name="s2iw_sb")
                nc.sync.dma_start(s2iw_sb[:], dp["s2iw"][:])
                # late-freed tiles (used through outproj) at higher addresses
                ow_sb = pm.tile([128, MB, D], BF, name="ow_sb")
                nc.scalar.dma_start(
                    ow_sb[:], dp["ow"].ap().rearrange("(kb p) d -> p kb d", p=128))
                xo = [pm.tile([128, D], FP, name=f"xo{t_}", tag=f"xo{t_}")
                      for t_ in range(OTB)]

                # ---- rmsnorm1 + transpose to hT ----
                # chunks: [64 halo] + 4x [128 own]
                chunks = [(0, HALO, None)] + [
                    (HALO + t_ * 128, 128, t_) for t_ in range(OTB)]
                with nc.named_scope("rms1"), tc.tile_pool(name="ps1", bufs=1, space="PSUM") as psA:
                    for (row0, rows, t_) in chunks:
                        if t_ is None:
                            xt = pt_pool.tile([HALO, D], FP, tag="xt0")
                        else:
                            xt = xo[t_]
                        # gpsimd queue: ahead of the ipw weight chunks, and not
                        # behind the ~20 small constant loads on the sync queue
                        nc.gpsimd.dma_start(xt[:], dp["x_sh"][row0:row0 + rows, :])
                        scr = pt_pool.tile([128, D], FP, tag="scr", bufs=1)
                        sq = pt_pool.tile([128, 1], FP, tag="sq", bufs=2)
                        nc.scalar.activation(scr[0:rows, :], xt[:], AF.Square,
                                             accum_out=sq[0:rows, :])
                        nr = pt_pool.tile([128, 1], FP, tag="nr", bufs=2)
                        nc.vector.tensor_scalar(nr[0:rows, :], sq[0:rows, :], 1.0 / D,
                                                1e-6, ALU.mult, ALU.add)
                        nc.scalar.sqrt(nr[0:rows, :], nr[0:rows, :])
                        nc.vector.reciprocal(nr[0:rows, :], nr[0:rows, :])
                        h_t = pt_pool.tile([128, D], BF, tag="htb", bufs=2)
                        nc.vector.tensor_scalar(h_t[0:rows, :], xt[:], nr[0:rows, :],
                                                None, ALU.mult)
                        for kb in range(KB):
                            ptr = psA.tile([128, 128], BF, tag="ptr", bufs=2)
                            nc.tensor.transpose(ptr[:, 0:rows],
                                                h_t[0:rows, kb * 128:(kb + 1) * 128],
                                                identb[0:rows, 0:rows])
                            nc.vector.tensor_copy(hT[kb][:, row0:row0 + rows],
                                                  ptr[:, 0:rows])

                # ---- in_proj (x_main half) + conv + silu ----
                with nc.named_scope("in_proj"), tc.tile_pool(name="ps2", bufs=1, space="PSUM") as psA:
                    for q in range(4):
                        wq = []
                        for kb in range(KB):
                            wt = pt_pool.tile([128, 512], BF, tag=f"wip{kb}", bufs=2,
                                              name=f"wip{kb}")
                            nc.gpsimd.dma_start(
                                wt[:], dp["ipw"][kb * 128:(kb + 1) * 128,
                                                 q * 512:(q + 1) * 512])
                            wq.append(wt)
                        for mi in range(4):
                            m = q * 4 + mi
                            xzp = pt_pool.tile([128, NH + 2], FP, tag="xzp", bufs=2)
                            nc.vector.memset(xzp[:, 0:2], 0.0)
                            for n0, nw in ((0, 512), (512, NH - 512)):
                                px = psA.tile([128, 512], FP, tag="px", bufs=2)
                                for kb in range(KB):
                                    nc.tensor.matmul(px[:, 0:nw],
                                                     wq[kb][:, mi * 128:(mi + 1) * 128],
                                                     hT[kb][:, n0:n0 + nw],
                                                     start=(kb == 0), stop=(kb == KB - 1))
                                nc.scalar.activation(xzp[:, 2 + n0:2 + n0 + nw], px[:, 0:nw],
                                                     AF.Identity, bias=ipb_sb[:, m:m + 1])
                            cv = pt_pool.tile([128, NH], FP, tag="cv", bufs=2)
                            nc.vector.tensor_scalar(cv[:], xzp[:, 0:NH], cw_sb[:, m, 0:1],
                                                    None, ALU.mult)
                            nc.vector.scalar_tensor_tensor(cv[:], xzp[:, 1:1 + NH],
                                                           cw_sb[:, m, 1:2], cv[:],
                                                           ALU.mult, ALU.add)
                            nc.vector.scalar_tensor_tensor(cv[:], xzp[:, 2:2 + NH],
                                                           cw_sb[:, m, 2:3], cv[:],
                                                           ALU.mult, ALU.add)
                            sgc = pt_pool.tile([128, NH], BF, tag="sgc", bufs=2)
                            nc.scalar.activation(sgc[:], cv[:], AF.Sigmoid, bias=cb_sb[:, m:m + 1])
                            nc.vector.scalar_tensor_tensor(xm[m][:], cv[:], cb_sb[:, m:m + 1],
                                                           sgc[:], ALU.add, ALU.mult)

                # ---- dt/B/C projections (emitted before gate MMs; feed scan) ----
                with nc.named_scope("scan"), tc.tile_pool(name="ps3", bufs=1, space="PSUM") as psA:
                    dt_t = pt_pool.tile([S, NH], FP, tag="dt")
                    a_t = pt_pool.tile([S, NH], FP, tag="a")
                    b_t = pt_pool.tile([S, NH], FP, tag="b")
                    c_t = pt_pool.tile([S, NH], FP, tag="c")
                    for n0, nw in ((0, 512), (512, NH - 512)):
                        for wsb, bias_sb, dst, fn in (
                            (dtw_sb, dtb_sb, dt_t, AF.Sigmoid),
                            (cpw_sb, cpb_sb, c_t, AF.Identity),
                        ):
                            pz = psA.tile([S, 512], FP, tag="pz", bufs=2)
                            for kb in range(MB):
                                nc.tensor.matmul(pz[:, 0:nw], wsb[:, kb, :],
                                                 xm[kb][:, n0:n0 + nw],
                                                 start=(kb == 0), stop=(kb == MB - 1))
                            nc.scalar.activation(dst[:, n0:n0 + nw], pz[:, 0:nw], fn,
                                                 bias=bias_sb[:])
                        # b needs dt -> separate pass
                        pz = psA.tile([S, 512], FP, tag="pz", bufs=2)
                        for kb in range(MB):
                            nc.tensor.matmul(pz[:, 0:nw], bpw_sb[:, kb, :],
                                             xm[kb][:, n0:n0 + nw],
                                             start=(kb == 0), stop=(kb == MB - 1))
                        nc.vector.scalar_tensor_tensor(b_t[:, n0:n0 + nw], pz[:, 0:nw],
                                                       bpb_sb[:], dt_t[:, n0:n0 + nw],
                                                       ALU.add, ALU.mult)
                    # scan runs on the vector engine while the tensor engine
                    # works through the gate-projection matmuls below
                    nc.vector.tensor_scalar(a_t[:], dt_t[:], -1.0, 1.0,
                                            ALU.mult, ALU.add)
                    st_t = pt_pool.tile([S, NH], FP, tag="st")
                    nc.vector.tensor_tensor_scan(st_t[:], a_t[:], b_t[:], 0.0,
                                                 ALU.mult, ALU.add)
                    y_t = pt_pool.tile([S, OWN], FP, tag="yt", name="y_t")
                    nc.vector.tensor_mul(y_t[:], c_t[:, HALO:NH], st_t[:, HALO:NH])

                # ---- gate half of in_proj, first 8 m: emitted NOW so the
                # tensor queue has work while the (vector-engine) scan runs ----
                def load_wqg(q):
                    wqg = []
                    for kb in range(KB):
                        wt = pt_pool.tile([128, 512], BF, tag=f"wip{kb}", bufs=2,
                                          name=f"wipg{kb}_{q}")
                        nc.gpsimd.dma_start(
                            wt[:], dp["ipw"][kb * 128:(kb + 1) * 128,
                                             2048 + q * 512:2048 + (q + 1) * 512])
                        wqg.append(wt)
                    return wqg

                def gate_mm(psB, wqg, m):
                    mi = m % 4
                    pg = psB.tile([128, 512], FP, tag="pg", bufs=2)
                    for kb in range(KB):
                        nc.tensor.matmul(pg[:], wqg[kb][:, mi * 128:(mi + 1) * 128],
                                         hT[kb][:, HALO:NH],
                                         start=(kb == 0), stop=(kb == KB - 1))
                    return pg

                with nc.named_scope("gateproj"), tc.tile_pool(name="ps4", bufs=1, space="PSUM") as psB:
                    for q in range(2):
                        wqg = load_wqg(q)
                        for mi in range(4):
                            m = q * 4 + mi
                            pg = gate_mm(psB, wqg, m)
                            nc.scalar.activation(sg[m][:], pg[:], AF.Sigmoid,
                                                 bias=ipb_sb[:, MB + m:MB + m + 1])

                # ---- layernorm over S ----
                with nc.named_scope("scanln"), tc.tile_pool(name="ps5", bufs=1, space="PSUM") as psA:
                    yln = pt_pool.tile([S, OWN], BF, tag="a", name="yln")
                    for i in range(OTB):
                        ptr = psA.tile([128, 128], FP, tag="ptr", bufs=2)
                        nc.tensor.transpose(ptr[:, 0:S], y_t[:, i * 128:(i + 1) * 128],
                                            ident[0:S, 0:S])
                        yT = pt_pool.tile([128, S], FP, tag="yT", bufs=2)
                        nc.vector.tensor_copy(yT[:], ptr[:, 0:S])
                        mu = pt_pool.tile([128, 1], FP, tag="mu", bufs=2)
                        nc.vector.tensor_reduce(mu[:], yT[:], mybir.AxisListType.X, ALU.add)
                        nc.vector.tensor_scalar_mul(mu[:], mu[:], 1.0 / S)
                        xc = pt_pool.tile([128, S], FP, tag="xc", bufs=2)
                        nc.vector.tensor_scalar_sub(xc[:], yT[:], mu[:])
                        scr2 = pt_pool.tile([128, S], FP, tag="scr2", bufs=2)
                        vv = pt_pool.tile([128, 1], FP, tag="vv", bufs=2)
                        nc.scalar.activation(scr2[:], xc[:], AF.Square, accum_out=vv[:])
                        nc.vector.tensor_scalar(vv[:], vv[:], 1.0 / S, 1e-5, ALU.mult, ALU.add)
                        nc.scalar.sqrt(vv[:], vv[:])
                        nc.vector.reciprocal(vv[:], vv[:])
                        xcb = pt_pool.tile([128, S], BF, tag="xcb", bufs=2)
                        nc.vector.tensor_scalar_mul(xcb[:], xc[:], vv[:])
                        ptr2 = psA.tile([128, 128], BF, tag="ptr2", bufs=2)
                        nc.tensor.transpose(ptr2[0:S, :], xcb[:], identb[:])
                        nc.vector.tensor_copy(yln[:, i * 128:(i + 1) * 128], ptr2[0:S, :])

                # ---- s2i + pre_out assembly (gate m>=8 computed inline) ----
                with nc.named_scope("premix"), tc.tile_pool(name="ps6", bufs=1, space="PSUM") as psA:
                    pre = []
                    for m in range(MB):
                        if m >= 8:
                            if m % 4 == 0:
                                wqg = load_wqg(m // 4)
                            pg = gate_mm(psA, wqg, m)
                            sg_m = pt_pool.tile([128, OWN], BF, tag="sgi", bufs=2)
                            nc.scalar.activation(sg_m[:], pg[:], AF.Sigmoid,
                                                 bias=ipb_sb[:, MB + m:MB + m + 1])
                        else:
                            sg_m = sg[m]
                        ps = psA.tile([128, 512], FP, tag="ps", bufs=2)
                        nc.tensor.matmul(ps[:], s2iw_sb[:, m * 128:(m + 1) * 128], yln[:],
                                         start=True, stop=True)
                        # Dp*xm + s2ib on the scalar engine; 2 vector ops total
                        tmp = pt_pool.tile([128, OWN], FP, tag="tmp", bufs=2)
                        nc.scalar.activation(tmp[:], xm[m][:, HALO:NH], AF.Identity,
                                             bias=s2ib_sb[:, m:m + 1],
                                             scale=Dp_sb[:, m:m + 1])
                        nc.vector.tensor_add(tmp[:], tmp[:], ps[:])
                        pre_m = pm.tile([128, OWN], BF, name=f"pre{m}", tag=f"pre{m}")
                        nc.vector.tensor_mul(pre_m[:], tmp[:], sg_m[:])
                        pre.append(pre_m)

                # ---- per-tb: out projection + residual + rms2 + h2T + gating + gather ----
                # emission interleave: po2 matmuls of tb+1 are queued before the
                # gating chain of tb, so the tensor engine has work while the
                # rms2/transpose chain for tb runs on scalar/vector
                with nc.named_scope("outproj"), tc.tile_pool(name="ps7", bufs=1, space="PSUM") as psA:
                    def emit_po2(tb):
                        po2 = psA.tile([128, 2, 512], FP, tag="po2", bufs=2)
                        for kb in range(MB):
                            for nb in range(2):
                                nc.tensor.matmul(po2[:, nb, :],
                                                 pre[kb][:, tb * 128:(tb + 1) * 128],
                                                 ow_sb[:, kb, nb * 512:(nb + 1) * 512],
                                                 start=(kb == 0), stop=False)
                        for nb in range(2):
                            nc.tensor.matmul(po2[:, nb, :], ones1[:],
                                             ob_sb[:, nb * 512:(nb + 1) * 512],
                                             start=False, stop=True)
                            nc.vector.tensor_add(xmid[tb][:, nb * 512:(nb + 1) * 512],
                                                 po2[:, nb, :],
                                                 xo[tb][:, nb * 512:(nb + 1) * 512])

                    def emit_gate(tb):
                        # rms2 for this tb
                        scr = pt_pool.tile([128, D], FP, tag="scr", bufs=1)
                        sq = pt_pool.tile([128, 1], FP, tag="sq", bufs=2)
                        nc.scalar.activation(scr[:], xmid[tb][:], AF.Square, accum_out=sq[:])
                        nr = pt_pool.tile([128, 1], FP, tag="nr", bufs=2)
                        nc.vector.tensor_scalar(nr[:], sq[:], 1.0 / D, 1e-6, ALU.mult, ALU.add)
                        nc.scalar.sqrt(nr[:], nr[:])
                        nc.vector.reciprocal(nr[:], nr[:])
                        h2 = pt_pool.tile([128, D], FP, tag="h2", bufs=1, name="h2")
                        nc.vector.tensor_scalar(h2[:], xmid[tb][:], nr[:], None, ALU.mult)
                        # gating logits must be fp32: bf16 logits flip top-2
                        # selections vs the reference on near-ties (~0.15 abs
                        # error per flipped token). All 8 transposes batch into
                        # one PSUM tile, then 2 wide vector copies + 1 staging
                        # DMA — avoids per-kb tensor<->vector ping-pong.
                        pl = psA.tile([128, E], FP, tag="pl", bufs=2)
                        ptr8 = psA.tile([128, KB * 128], FP, tag="ptr8", bufs=1)
                        for kb in range(KB):
                            nc.tensor.transpose(ptr8[:, kb * 128:(kb + 1) * 128],
                                                h2[:, kb * 128:(kb + 1) * 128],
                                                ident[:])
                        h2T_t = pt_pool.tile([128, KB * 128], FP, tag="h2T", bufs=1)
                        nc.vector.tensor_copy(h2T_t[:], ptr8[:])
                        h2T_8 = pt_pool.tile([128, KB * 128], F8, tag="h2T8", bufs=2)
                        nc.vector.tensor_copy(h2T_8[:], ptr8[:])
                        nc.sync.dma_start(gth_in[tb][:], h2T_8[:])
                        for kb in range(KB):
                            nc.tensor.matmul(pl[:], h2T_t[:, kb * 128:(kb + 1) * 128],
                                             gw_sb[:, kb, :],
                                             start=(kb == 0), stop=False)
                        nc.tensor.matmul(pl[:], ones1[:], gb_sb[:], start=False, stop=True)
                        # top-2-of-4 gating
                        m1 = pt_pool.tile([128, 1], FP, tag="m1", bufs=2)
                        nc.vector.tensor_reduce(m1[:], pl[:], mybir.AxisListType.X, ALU.max)
                        eq1 = pt_pool.tile([128, E], FP, tag="eq1", bufs=2)
                        nc.vector.tensor_scalar(eq1[:], pl[:], m1[:], None, ALU.is_equal)
                        msk = pt_pool.tile([128, E], FP, tag="msk", bufs=2)
                        nc.vector.scalar_tensor_tensor(msk[:], eq1[:], -1e30, pl[:],
                                                       ALU.mult, ALU.add)
                        m2 = pt_pool.tile([128, 1], FP, tag="m2", bufs=2)
                        nc.vector.tensor_reduce(m2[:], msk[:], mybir.AxisListType.X, ALU.max)
                        eq2 = pt_pool.tile([128, E], FP, tag="eq2", bufs=2)
                        nc.vector.tensor_scalar(eq2[:], msk[:], m2[:], None, ALU.is_equal)
                        dd = pt_pool.tile([128, 1], FP, tag="dd", bufs=2)
                        nc.vector.tensor_sub(dd[:], m2[:], m1[:])
                        p2 = pt_pool.tile([128, 1], FP, tag="p2", bufs=2)
                        nc.scalar.activation(p2[:], dd[:], AF.Sigmoid)
                        p1b = pt_pool.tile([128, 1], FP, tag="p1b", bufs=2)
                        nc.scalar.activation(p1b[:], p2[:], AF.Identity, bias=1.0, scale=-1.0)
                        wv = pt_pool.tile([128, E], FP, tag="wv", bufs=2)
                        nc.vector.tensor_scalar(wv[:], eq1[:], p1b[:], None, ALU.mult)
                        nc.vector.scalar_tensor_tensor(wv[:], eq2[:], p2[:], wv[:],
                                                       ALU.mult, ALU.add)
                        nc.sync.dma_start(gtw_in[tb * 128:(tb + 1) * 128, :], wv[:])
                        if debug_outputs:
                            nc.sync.dma_start(dbg["wown"][tb * 128:(tb + 1) * 128, :], wv[:])
                            nc.sync.dma_start(dbg["xmid"][tb * 128:(tb + 1) * 128, :],
                                              xmid[tb][:])
                        nc.gpsimd.collective_compute(
                            "AllGather", ALU.bypass, replica_groups=rg,
                            ins=[gth_in[tb].opt()], outs=[gth_out[tb].opt()])

                    for tb in range(OTB):
                        emit_po2(tb)
                        if tb >= 1:
                            emit_gate(tb - 1)
                    emit_gate(OTB - 1)
                    with nc.named_scope("gatherw"):
                        nc.gpsimd.collective_compute(
                            "AllGather", ALU.bypass, replica_groups=rg,
                            ins=[gtw_in.opt()], outs=[gtw_out.opt()])

            # =======================================================
            # MoE (full expert per core, token-half group of 4)
            # =======================================================
            with (
                tc.tile_pool(name="moe", bufs=1) as pq,
                tc.tile_pool(name="psC", bufs=1, space="PSUM") as psC,
            ):
                # expert weights resident in SBUF for all 4 rounds.
                # w1 runs in fp8 DoubleRow: ew1 arrives pre-scaled by W1SCALE and
                # host-interleaved to [p, h, two, m] per k-pair so each LDWEIGHTS
                # slice [128, 2, 128] is contiguous (strided pair dims fault the PE).
                ew1_sb = [pq.tile([128, HB, 2, 128], F8, name=f"ew1_{i}", tag=f"ew1_{i}")
                          for i in range(KB // 2)]
                for i in range(KB // 2):
                    nc.scalar.dma_start(
                        ew1_sb[i][:], dp["ew1"][:, i * (HB * 256):(i + 1) * (HB * 256)])
                ew2_sb = [pq.tile([128, D], BF, name=f"ew2_{j}", tag=f"ew2_{j}")
                          for j in range(HB)]
                for j in range(HB):
                    nc.scalar.dma_start(ew2_sb[j][:], dp["ew2"][j * 128:(j + 1) * 128, :])

                with nc.named_scope("moe"):
                    for r in range(4):
                        # one wide DMA per peer token-block: src rows are the
                        # peer's [128, kb*128] section, 1KB contiguous lines
                        h2r = pq.tile([128, KB, OWN], F8, tag="h2r", bufs=2)
                        for t_ in range(OTB):
                            nc.gpsimd.dma_start(
                                h2r[:, :, t_ * 128:(t_ + 1) * 128],
                                gth_out[t_][r * 128:(r + 1) * 128, :]
                                .rearrange("p (kb j) -> p kb j", j=128))
                        hid = []
                        for h in range(HB):
                            ph = psC.tile([128, 512], FP, tag="ph", bufs=2)
                            for i in range(KB // 2):
                                nc.tensor.matmul(ph[:], ew1_sb[i][:, h, :, :],
                                                 h2r[:, 2 * i:2 * i + 2, :],
                                                 start=(i == 0), stop=(i == KB // 2 - 1),
                                                 perf_mode=mybir.MatmulPerfMode.DoubleRow)
                            ht = pq.tile([128, OWN], BF, tag=f"hid{h}", bufs=1)
                            nc.scalar.activation(ht[:], ph[:], AF.Gelu, bias=eb1_sb[:, h:h + 1],
                                                 scale=1.0 / W1SCALE)
                            hid.append(ht)
                        # per-token weight for this core's expert
                        wvr = pq.tile([128, OTB, E], FP, tag="wvr", bufs=2)
                        nc.sync.dma_start(
                            wvr[:], gtw_out[r * OWN:(r + 1) * OWN, :]
                            .rearrange("(tb p) e -> p tb e", p=128))
                        ws = []
                        for tb in range(OTB):
                            wm_t = pq.tile([128, E], FP, tag="wm", bufs=2)
                            nc.vector.tensor_mul(wm_t[:], wvr[:, tb, :], esel[:])
                            ws_t = pq.tile([128, 1], FP, tag=f"ws{tb}", bufs=2)
                            nc.vector.tensor_reduce(ws_t[:], wm_t[:], mybir.AxisListType.X,
                                                    ALU.add)
                            ws.append(ws_t)
                        # w2: token-block pairs keep PSUM <= 6 banks
                        for tp in range(2):
                            peo = [psC.tile([128, 2, 512], FP, tag=f"peo{ti}", bufs=1,
                                            name=f"peo{ti}") for ti in range(2)]
                            for h in range(HB):
                                for ti in range(2):
                                    tb = tp * 2 + ti
                                    for nb in range(2):
                                        nc.tensor.matmul(
                                            peo[ti][:, nb, :],
                                            hid[h][:, tb * 128:(tb + 1) * 128],
                                            ew2_sb[h][:, nb * 512:(nb + 1) * 512],
                                            start=(h == 0), stop=False)
                            for ti in range(2):
                                tb = tp * 2 + ti
                                wout = pq.tile([128, D], BF, tag="wout", bufs=2)
                                for nb in range(2):
                                    nc.tensor.matmul(peo[ti][:, nb, :], ones1[:],
                                                     eb2h_sb[:, nb * 512:(nb + 1) * 512],
                                                     start=False, stop=True)
                                    n0 = nb * 512
                                    nc.vector.tensor_scalar(wout[:, n0:n0 + 512],
                                                            peo[ti][:, nb, :],
                                                            ws[tb][:], None, ALU.mult)
                                    # owner (r == e) carries the residual through
                                    # the reduce-scatter
                                    nc.vector.scalar_tensor_tensor(
                                        wout[:, n0:n0 + 512],
                                        xmid[tb][:, n0:n0 + 512], rmask[:, r:r + 1],
                                        wout[:, n0:n0 + 512], ALU.mult, ALU.add)
                                nc.sync.dma_start(
                                    rs_in[r][tb * 128:(tb + 1) * 128, :], wout[:])
                        nc.gpsimd.collective_compute(
                            "ReduceScatter", ALU.add, replica_groups=rg,
                            ins=[rs_in[r].opt()], outs=[rs_out[r].opt()])

                with nc.named_scope("final"):
                    for r in range(4):
                        rsb = pq.tile([128, D], BF, tag="rsb", bufs=2)
                        nc.sync.dma_start(rsb[:], rs_out[r][:])
                        osb = pq.tile([128, D], FP, tag="osb", bufs=1)
                        nc.vector.tensor_copy(osb[:], rsb[:])
                        nc.sync.dma_start(out_d[r * 128:(r + 1) * 128, :], osb[:])

    nc.compile()
    return nc


def host_prep(inputs):
    """Build the 8 per-core input maps from full inputs."""
    import ml_dtypes
    f32 = np.float32
    bf = ml_dtypes.bfloat16
    x = np.ascontiguousarray(np.asarray(inputs["x"], f32).reshape(B * T, D))
    n1 = np.asarray(inputs["norm1_w"], f32)
    n2 = np.asarray(inputs["norm2_w"], f32)
    ipw = np.ascontiguousarray(np.asarray(inputs["in_proj_w"], f32) * n1[:, None]).astype(bf)
    gw = np.ascontiguousarray(np.asarray(inputs["gate_w"], f32) * n2[:, None])
    ew1f = np.asarray(inputs["e_w1"], f32) * n2[None, :, None]
    ew1q = np.clip(ew1f * 64.0, -240.0, 240.0).astype(ml_dtypes.float8_e4m3)
    # [E, k, hid] -> [E, p, i, h, two, m]: k = i*256 + two*128 + p, hid = h*128 + m
    ew1b = ew1q.reshape(E, 4, 2, 128, HID // 128, 128).transpose(0, 3, 1, 4, 2, 5)
    ew1b = np.ascontiguousarray(ew1b.reshape(E, 128, -1))
    ew2b = np.asarray(inputs["e_w2"], f32).astype(bf)
    ident = np.eye(128, dtype=f32)
    ones1 = np.ones((1, 128), f32)
    shared = {
        "ipw": ipw, "ipb": np.asarray(inputs["in_proj_b"], f32),
        "cw": np.ascontiguousarray(np.asarray(inputs["conv_w"], f32)[:, 0, :]),
        "cb": np.asarray(inputs["conv_b"], f32),
        "dtw": np.asarray(inputs["dt_w"], f32).astype(bf),
        "dtb": np.asarray(inputs["dt_b"], f32),
        "bpw": np.asarray(inputs["bp_w"], f32).astype(bf),
        "bpb": np.asarray(inputs["bp_b"], f32),
        "cpw": np.asarray(inputs["cp_w"], f32).astype(bf),
        "cpb": np.asarray(inputs["cp_b"], f32),
        "s2iw": np.asarray(inputs["s2i_w"], f32).astype(bf),
        "s2ib": np.asarray(inputs["s2i_b"], f32),
        "Dp": np.asarray(inputs["D_param"], f32),
        "ow": np.asarray(inputs["out_w"], f32).astype(bf),
        "ob": np.asarray(inputs["out_b"], f32),
        "gw": gw, "gb": np.asarray(inputs["gate_b"], f32),
        "ident": ident, "identb": ident.astype(bf), "ones1": ones1,
    }
    eb1 = np.asarray(inputs["e_b1"], f32)
    eb2 = np.asarray(inputs["e_b2"], f32)
    in_maps = []
    for c in range(N_CORES):
        e, th = c // 2, c % 2
        g0 = th * (B * T // 2) + e * OWN
        if e == 0:
            x_sh = np.concatenate([np.zeros((HALO, D), f32), x[g0:g0 + OWN]])
        else:
            x_sh = x[g0 - HALO:g0 + OWN]
        m = dict(shared)
        m["x_sh"] = np.ascontiguousarray(x_sh)
        m["ew1"] = np.ascontiguousarray(ew1b[e])
        m["eb1"] = np.ascontiguousarray(eb1[e])
        m["ew2"] = np.ascontiguousarray(ew2b[e])
        m["eb2h"] = np.ascontiguousarray(eb2[e])
        esel = np.zeros((128, E), f32)
        esel[:, e] = 1.0
        m["esel"] = esel
        rmask = np.zeros((128, 4), f32)
        rmask[:, e] = 1.0
        m["rmask"] = rmask
        in_maps.append(m)
    return in_maps


def unshard_out(results):
    """results: list of 8 dicts with 'out' [OWN, D]; rows r*128+i of core c
    hold global token (c%2)*2048 + r*512 + (c//2)*128 + i."""
    full = np.empty((B * T, D), np.float32)
    for c in range(N_CORES):
        e, th = c // 2, c % 2
        oc = results[c]["out"]
        for r in range(4):
            full[th * 2048 + r * OWN + e * 128: th * 2048 + r * OWN + (e + 1) * 128] = \
                oc[r * 128:(r + 1) * 128]
    return full.reshape(B, T, D)


_NC_CACHE = {}


def _get_nc():
    if "nc" not in _NC_CACHE:
        _NC_CACHE["nc"] = build(debug_outputs=False)
    return _NC_CACHE["nc"]


def kernel(**inputs) -> np.ndarray:
    """Full-input entry point: shards across 8 NeuronCores, runs the Bass
    kernel SPMD, reassembles the full [2, 2048, 1024] output."""
    import sys, types
    try:  # NTFF profile hook shim (missing antenv.axon_hooks in this image)
        import antenv.axon_hooks  # noqa: F401
    except ImportError:
        try:
            import antenv
            from trn_agent_boot.trn_boot import _ntff_profile_via_ctypes
            mod = types.ModuleType("antenv.axon_hooks")
            try:
                _hook = _ntff_profile_via_ctypes("/opt/axon/libaxon_pjrt.so")
            except Exception:
                _hook = None
            mod.get_axon_ntff_profile_hook = lambda: _hook
            mod.set_axon_ntff_profile_hook = lambda h: None
            sys.modules["antenv.axon_hooks"] = mod
            antenv.axon_hooks = mod
        except Exception:
            pass
    from concourse.bass_utils import run_bass_kernel_spmd

    nc = _get_nc()
    in_maps = host_prep(inputs)
    res = run_bass_kernel_spmd(nc, in_maps, core_ids=list(range(N_CORES)))
    out = unshard_out(res.results)
    return out.astype(np.float32)


# revision 40
# speedup vs baseline: 1.8521x; 1.0010x over previous
"""Bass kernel builder for nn_MixtureOfMambaBlock — 8-core SPMD.

Sharding: tokens 8-way (512/core + 64 halo for conv+scan warmup); mixer fully
local per core (weights replicated, bf16 matmuls; fp32 gating logits to keep
top-2 selection exact). Post-mixer h2 all-gathered in fp8; MoE is expert x
token-half sharded: w1 runs fp8 DoubleRow (weights pre-scaled x64,
host-interleaved k-pairs), w2 in bf16 with both expert weights SBUF-resident.
Weighted expert partials + residual reduce-scattered in bf16 back to token
shards.
"""
import numpy as np
import concourse.bass as bass
import concourse.bacc as bacc
import concourse.mybir as mybir
import concourse.tile as tile

FP = mybir.dt.float32
FR = mybir.dt.float32r
BF = mybir.dt.bfloat16
F8 = mybir.dt.float8e4
W1SCALE = 64.0
AF = mybir.ActivationFunctionType
ALU = mybir.AluOpType

B, T, D = 2, 2048, 1024
S, INNER = 64, 2048
E = 4
HID = 4096
OWN, HALO = 512, 64
NH = OWN + HALO          # 576
KB = D // 128            # 8  d-blocks
MB = INNER // 128        # 16 inner-blocks
HB = HID // 128          # 32 hid-blocks
OTB = OWN // 128         # 4  own-token blocks
N_CORES = 8

INPUT_SPECS = {
    "x_sh": ([NH, D], FP),
    "ipw": ([D, 2 * INNER], BF), "ipb": ([2 * INNER], FP),
    "cw": ([INNER, 3], FP), "cb": ([INNER], FP),
    "dtw": ([INNER, S], BF), "dtb": ([S], FP),
    "bpw": ([INNER, S], BF), "bpb": ([S], FP),
    "cpw": ([INNER, S], BF), "cpb": ([S], FP),
    "s2iw": ([S, INNER], BF), "s2ib": ([INNER], FP),
    "Dp": ([INNER], FP),
    "ow": ([INNER, D], BF), "ob": ([D], FR),
    "gw": ([D, E], FP), "gb": ([E], FR),
    "ew1": ([128, KB // 2 * HID // 128 * 256], F8), "eb1": ([HID], FP),
    "ew2": ([HID, D], BF), "eb2h": ([D], FR),
    "esel": ([128, E], FP),
    "rmask": ([128, 4], FP),
    "ident": ([128, 128], FP),
    "identb": ([128, 128], BF),
    "ones1": ([1, 128], FR),
}


def build(debug_outputs=False):
    nc = bacc.Bacc("TRN2", target_bir_lowering=False, debug=False,
                   num_devices=N_CORES)
    dp = {}
    for name, (shape, dt) in INPUT_SPECS.items():
        dp[name] = nc.dram_tensor(name, shape, dt, kind="ExternalInput")
    out_d = nc.dram_tensor("out", [OWN, D], FP, kind="ExternalOutput")
    dbg = {}
    if debug_outputs:
        dbg["xmid"] = nc.dram_tensor("dbg_xmid", [OWN, D], FP, kind="ExternalOutput")
        dbg["wown"] = nc.dram_tensor("dbg_wown", [OWN, E], FP, kind="ExternalOutput")

    rg = [[0, 2, 4, 6], [1, 3, 5, 7]]

    with tile.TileContext(nc) as tc:
        with (
            tc.tile_pool(name="outer", bufs=1) as po,
            tc.tile_pool(name="dram", bufs=1, space="DRAM") as pdram,
        ):
            # ---------- DRAM bounce buffers for collectives ----------
            # gth layout per tb: [128 d-in-block, kb*128 tok] — matches the
            # transpose PSUM tile directly (1 staging DMA) and lets the MoE
            # load each peer row-block with a single wide DMA.
            gth_in = [pdram.tile([128, KB * 128], F8, name=f"gth_in{t_}")
                      for t_ in range(OTB)]
            gth_out = [pdram.tile([4 * 128, KB * 128], F8, name=f"gth_out{t_}")
                       for t_ in range(OTB)]
            gtw_in = pdram.tile([OWN, E], FP)
            gtw_out = pdram.tile([4 * OWN, E], FP)
            rs_in = [pdram.tile([OWN, D], BF, name=f"rs_in{r}") for r in range(4)]
            rs_out = [pdram.tile([128, D], BF, name=f"rs_out{r}") for r in range(4)]

            # ---------- constants / small weights (emit all loads up front) ----
            ident = po.tile([128, 128], FP)
            nc.sync.dma_start(ident[:], dp["ident"][:])
            identb = po.tile([128, 128], BF)
            nc.sync.dma_start(identb[:], dp["identb"][:])

            def load_pcol(name, blocks):  # [blocks*128] -> [128, blocks]
                t = po.tile([128, blocks], FP, name=f"{name}_sb")
                nc.sync.dma_start(
                    t[:], dp[name].ap().rearrange("(m p) -> p m", p=128))
                return t

            def load_vec1(name, n):  # [n] -> [n, 1]
                t = po.tile([n, 1], FP, name=f"{name}_sb")
                nc.sync.dma_start(t[:], dp[name].ap().rearrange("(s o) -> s o", o=1))
                return t

            def load_row(name, n, dt_=FP):  # [n] -> [1, n]
                t = po.tile([1, n], dt_, name=f"{name}_sb")
                nc.sync.dma_start(t[:], dp[name].ap().rearrange("(o s) -> o s", o=1))
                return t

            def load_kw(name, pool):  # [2048, 64] -> [128, 16, 64], lhsT slice [:, kb, :]
                t = pool.tile([128, MB, S], BF, name=f"{name}_sb")
                nc.sync.dma_start(t[:], dp[name].ap().rearrange("(kb p) s -> p kb s", p=128))
                return t

            ob_sb = load_row("ob", D, FR)
            gb_sb = load_row("gb", E, FR)
            eb2h_sb = load_row("eb2h", D, FR)
            ones1 = po.tile([1, 128], FR)
            nc.sync.dma_start(ones1[:], dp["ones1"][:])
            ipb_sb = load_pcol("ipb", 32)
            cb_sb = load_pcol("cb", 16)
            cw_sb = po.tile([128, 16, 3], FP)  # [p, m, k]
            nc.sync.dma_start(cw_sb[:], dp["cw"].ap().rearrange("(m p) k -> p m k", p=128))
            dtb_sb = load_vec1("dtb", S)
            bpb_sb = load_vec1("bpb", S)
            cpb_sb = load_vec1("cpb", S)
            s2ib_sb = load_pcol("s2ib", 16)
            Dp_sb = load_pcol("Dp", 16)
            gw_sb = po.tile([128, KB, E], FP)  # [p, kb, e]
            nc.sync.dma_start(gw_sb[:], dp["gw"].ap().rearrange("(kb p) e -> p kb e", p=128))
            esel = po.tile([128, E], FP)
            nc.sync.dma_start(esel[:], dp["esel"][:])
            rmask = po.tile([128, 4], FP)
            nc.sync.dma_start(rmask[:], dp["rmask"][:])
            eb1_sb = load_pcol("eb1", HB)

            # persistent activations
            xmid = [po.tile([128, D], FP, name=f"xmid{t_}", tag=f"xmid{t_}")
                    for t_ in range(OTB)]

            # =======================================================
            # MIXER
            # =======================================================
            with (
                tc.tile_pool(name="mixer", bufs=1) as pm,
                tc.tile_pool(name="mixt", bufs=1) as pt_pool,
            ):
                # pool allocation order matters: tiles that die early (hT, xm,
                # sg, projection weights) go FIRST so their addresses sit at the
                # pool base — the MoE pool's ew1 tiles (allocated first there)
                # land on them and can start loading before outproj finishes.
                hT = [pm.tile([128, NH], BF, name=f"hT{kb}", tag=f"hT{kb}") for kb in range(KB)]
                xm = [pm.tile([128, NH], BF, name=f"xm{m}", tag=f"xm{m}") for m in range(MB)]
                sg = [pm.tile([128, OWN], BF, name=f"sg{m}", tag=f"sg{m}")
                      for m in range(8)]
                dtw_sb = load_kw("dtw", pm)
                bpw_sb = load_kw("bpw", pm)
                cpw_sb = load_kw("cpw", pm)
                s2iw_sb = pm.tile([S, INNER], BF, name="s2iw_sb")
                nc.sync.dma_start(s2iw_sb[:], dp["s2iw"][:])
                # late-freed tiles (used through outproj) at higher addresses
                ow_sb = pm.tile([128, MB, D], BF, name="ow_sb")
                nc.scalar.dma_start(
                    ow_sb[:], dp["ow"].ap().rearrange("(kb p) d -> p kb d", p=128))
                xo = [pm.tile([128, D], FP, name=f"xo{t_}", tag=f"xo{t_}")
                      for t_ in range(OTB)]

                # ---- rmsnorm1 + transpose to hT ----
                # chunks: [64 halo] + 4x [128 own]
                chunks = [(0, HALO, None)] + [
                    (HALO + t_ * 128, 128, t_) for t_ in range(OTB)]
                with nc.named_scope("rms1"), tc.tile_pool(name="ps1", bufs=1, space="PSUM") as psA:
                    for (row0, rows, t_) in chunks:
                        if t_ is None:
                            xt = pt_pool.tile([HALO, D], FP, tag="xt0")
                        else:
                            xt = xo[t_]
                        # gpsimd queue: ahead of the ipw weight chunks, and not
                        # behind the ~20 small constant loads on the sync queue
                        nc.gpsimd.dma_start(xt[:], dp["x_sh"][row0:row0 + rows, :])
                        scr = pt_pool.tile([128, D], FP, tag="scr", bufs=1)
                        sq = pt_pool.tile([128, 1], FP, tag="sq", bufs=2)
                        nc.scalar.activation(scr[0:rows, :], xt[:], AF.Square,
                                             accum_out=sq[0:rows, :])
                        nr = pt_pool.tile([128, 1], FP, tag="nr", bufs=2)
                        nc.vector.tensor_scalar(nr[0:rows, :], sq[0:rows, :], 1.0 / D,
                                                1e-6, ALU.mult, ALU.add)
                        nc.scalar.sqrt(nr[0:rows, :], nr[0:rows, :])
                        nc.vector.reciprocal(nr[0:rows, :], nr[0:rows, :])
                        h_t = pt_pool.tile([128, D], BF, tag="htb", bufs=2)
                        nc.vector.tensor_scalar(h_t[0:rows, :], xt[:], nr[0:rows, :],
                                                None, ALU.mult)
                        for kb in range(KB):
                            ptr = psA.tile([128, 128], BF, tag="ptr", bufs=2)
                            nc.tensor.transpose(ptr[:, 0:rows],
                                                h_t[0:rows, kb * 128:(kb + 1) * 128],
                                                identb[0:rows, 0:rows])
                            nc.vector.tensor_copy(hT[kb][:, row0:row0 + rows],
                                                  ptr[:, 0:rows])

                # ---- in_proj (x_main half) + conv + silu ----
                with nc.named_scope("in_proj"), tc.tile_pool(name="ps2", bufs=1, space="PSUM") as psA:
                    for q in range(4):
                        wq = []
                        for kb in range(KB):
                            wt = pt_pool.tile([128, 512], BF, tag=f"wip{kb}", bufs=2,
                                              name=f"wip{kb}")
                            nc.gpsimd.dma_start(
                                wt[:], dp["ipw"][kb * 128:(kb + 1) * 128,
                                                 q * 512:(q + 1) * 512])
                            wq.append(wt)
                        for mi in range(4):
                            m = q * 4 + mi
                            xzp = pt_pool.tile([128, NH + 2], FP, tag="xzp", bufs=2)
                            nc.vector.memset(xzp[:, 0:2], 0.0)
                            for n0, nw in ((0, 512), (512, NH - 512)):
                                px = psA.tile([128, 512], FP, tag="px", bufs=2)
                                for kb in range(KB):
                                    nc.tensor.matmul(px[:, 0:nw],
                                                     wq[kb][:, mi * 128:(mi + 1) * 128],
                                                     hT[kb][:, n0:n0 + nw],
                                                     start=(kb == 0), stop=(kb == KB - 1))
                                nc.scalar.activation(xzp[:, 2 + n0:2 + n0 + nw], px[:, 0:nw],
                                                     AF.Identity, bias=ipb_sb[:, m:m + 1])
                            cv = pt_pool.tile([128, NH], FP, tag="cv", bufs=2)
                            nc.vector.tensor_scalar(cv[:], xzp[:, 0:NH], cw_sb[:, m, 0:1],
                                                    None, ALU.mult)
                            nc.vector.scalar_tensor_tensor(cv[:], xzp[:, 1:1 + NH],
                                                           cw_sb[:, m, 1:2], cv[:],
                                                           ALU.mult, ALU.add)
                            nc.vector.scalar_tensor_tensor(cv[:], xzp[:, 2:2 + NH],
                                                           cw_sb[:, m, 2:3], cv[:],
                                                           ALU.mult, ALU.add)
                            sgc = pt_pool.tile([128, NH], BF, tag="sgc", bufs=2)
                            nc.scalar.activation(sgc[:], cv[:], AF.Sigmoid, bias=cb_sb[:, m:m + 1])
                            nc.vector.scalar_tensor_tensor(xm[m][:], cv[:], cb_sb[:, m:m + 1],
                                                           sgc[:], ALU.add, ALU.mult)

                # ---- dt/B/C projections (emitted before gate MMs; feed scan) ----
                with nc.named_scope("scan"), tc.tile_pool(name="ps3", bufs=1, space="PSUM") as psA:
                    dt_t = pt_pool.tile([S, NH], FP, tag="dt")
                    a_t = pt_pool.tile([S, NH], FP, tag="a")
                    b_t = pt_pool.tile([S, NH], FP, tag="b")
                    c_t = pt_pool.tile([S, NH], FP, tag="c")
                    for n0, nw in ((0, 512), (512, NH - 512)):
                        for wsb, bias_sb, dst, fn in (
                            (dtw_sb, dtb_sb, dt_t, AF.Sigmoid),
                            (cpw_sb, cpb_sb, c_t, AF.Identity),
                        ):
                            pz = psA.tile([S, 512], FP, tag="pz", bufs=2)
                            for kb in range(MB):
                                nc.tensor.matmul(pz[:, 0:nw], wsb[:, kb, :],
                                                 xm[kb][:, n0:n0 + nw],
                                                 start=(kb == 0), stop=(kb == MB - 1))
                            nc.scalar.activation(dst[:, n0:n0 + nw], pz[:, 0:nw], fn,
                                                 bias=bias_sb[:])
                        # b needs dt -> separate pass
                        pz = psA.tile([S, 512], FP, tag="pz", bufs=2)
                        for kb in range(MB):
                            nc.tensor.matmul(pz[:, 0:nw], bpw_sb[:, kb, :],
                                             xm[kb][:, n0:n0 + nw],
                                             start=(kb == 0), stop=(kb == MB - 1))
                        nc.vector.scalar_tensor_tensor(b_t[:, n0:n0 + nw], pz[:, 0:nw],
                                                       bpb_sb[:], dt_t[:, n0:n0 + nw],
                                                       ALU.add, ALU.mult)
                    # scan runs on the vector engine while the tensor engine
                    # works through the gate-projection matmuls below
                    nc.vector.tensor_scalar(a_t[:], dt_t[:], -1.0, 1.0,
                                            ALU.mult, ALU.add)
                    st_t = pt_pool.tile([S, NH], FP, tag="st")
                    nc.vector.tensor_tensor_scan(st_t[:], a_t[:], b_t[:], 0.0,
                                                 ALU.mult, ALU.add)
                    y_t = pt_pool.tile([S, OWN], FP, tag="yt", name="y_t")
                    nc.vector.tensor_mul(y_t[:], c_t[:, HALO:NH], st_t[:, HALO:NH])

                # ---- gate half of in_proj, first 8 m: emitted NOW so the
                # tensor queue has work while the (vector-engine) scan runs ----
                def load_wqg(q):
                    wqg = []
                    for kb in range(KB):
                        wt = pt_pool.tile([128, 512], BF, tag=f"wip{kb}", bufs=2,
                                          name=f"wipg{kb}_{q}")
                        nc.gpsimd.dma_start(
                            wt[:], dp["ipw"][kb * 128:(kb + 1) * 128,
                                             2048 + q * 512:2048 + (q + 1) * 512])
                        wqg.append(wt)
                    return wqg

                def gate_mm(psB, wqg, m):
                    mi = m % 4
                    pg = psB.tile([128, 512], FP, tag="pg", bufs=2)
                    for kb in range(KB):
                        nc.tensor.matmul(pg[:], wqg[kb][:, mi * 128:(mi + 1) * 128],
                                         hT[kb][:, HALO:NH],
                                         start=(kb == 0), stop=(kb == KB - 1))
                    return pg

                with nc.named_scope("gateproj"), tc.tile_pool(name="ps4", bufs=1, space="PSUM") as psB:
                    for q in range(2):
                        wqg = load_wqg(q)
                        for mi in range(4):
                            m = q * 4 + mi
                            pg = gate_mm(psB, wqg, m)
                            nc.scalar.activation(sg[m][:], pg[:], AF.Sigmoid,
                                                 bias=ipb_sb[:, MB + m:MB + m + 1])

                # ---- layernorm over S ----
                with nc.named_scope("scanln"), tc.tile_pool(name="ps5", bufs=1, space="PSUM") as psA:
                    yln = pt_pool.tile([S, OWN], BF, tag="a", name="yln")
                    for i in range(OTB):
                        ptr = psA.tile([128, 128], FP, tag="ptr", bufs=2)
                        nc.tensor.transpose(ptr[:, 0:S], y_t[:, i * 128:(i + 1) * 128],
                                            ident[0:S, 0:S])
                        yT = pt_pool.tile([128, S], FP, tag="yT", bufs=2)
                        nc.vector.tensor_copy(yT[:], ptr[:, 0:S])
                        mu = pt_pool.tile([128, 1], FP, tag="mu", bufs=2)
                        nc.vector.tensor_reduce(mu[:], yT[:], mybir.AxisListType.X, ALU.add)
                        nc.vector.tensor_scalar_mul(mu[:], mu[:], 1.0 / S)
                        xc = pt_pool.tile([128, S], FP, tag="xc", bufs=2)
                        nc.vector.tensor_scalar_sub(xc[:], yT[:], mu[:])
                        scr2 = pt_pool.tile([128, S], FP, tag="scr2", bufs=2)
                        vv = pt_pool.tile([128, 1], FP, tag="vv", bufs=2)
                        nc.scalar.activation(scr2[:], xc[:], AF.Square, accum_out=vv[:])
                        nc.vector.tensor_scalar(vv[:], vv[:], 1.0 / S, 1e-5, ALU.mult, ALU.add)
                        nc.scalar.sqrt(vv[:], vv[:])
                        nc.vector.reciprocal(vv[:], vv[:])
                        xcb = pt_pool.tile([128, S], BF, tag="xcb", bufs=2)
                        nc.vector.tensor_scalar_mul(xcb[:], xc[:], vv[:])
                        ptr2 = psA.tile([128, 128], BF, tag="ptr2", bufs=2)
                        nc.tensor.transpose(ptr2[0:S, :], xcb[:], identb[:])
                        nc.vector.tensor_copy(yln[:, i * 128:(i + 1) * 128], ptr2[0:S, :])

                # ---- s2i + pre_out assembly (gate m>=8 computed inline) ----
                with nc.named_scope("premix"), tc.tile_pool(name="ps6", bufs=1, space="PSUM") as psA:
                    pre = []
                    for m in range(MB):
                        if m >= 8:
                            if m % 4 == 0:
                                wqg = load_wqg(m // 4)
                            pg = gate_mm(psA, wqg, m)
                            sg_m = pt_pool.tile([128, OWN], BF, tag="sgi", bufs=2)
                            nc.scalar.activation(sg_m[:], pg[:], AF.Sigmoid,
                                                 bias=ipb_sb[:, MB + m:MB + m + 1])
                        else:
                            sg_m = sg[m]
                        ps = psA.tile([128, 512], FP, tag="ps", bufs=2)
                        nc.tensor.matmul(ps[:], s2iw_sb[:, m * 128:(m + 1) * 128], yln[:],
                                         start=True, stop=True)
                        # Dp*xm + s2ib on the scalar engine; 2 vector ops total
                        tmp = pt_pool.tile([128, OWN], FP, tag="tmp", bufs=2)
                        nc.scalar.activation(tmp[:], xm[m][:, HALO:NH], AF.Identity,
                                             bias=s2ib_sb[:, m:m + 1],
                                             scale=Dp_sb[:, m:m + 1])
                        nc.vector.tensor_add(tmp[:], tmp[:], ps[:])
                        pre_m = pm.tile([128, OWN], BF, name=f"pre{m}", tag=f"pre{m}")
                        nc.vector.tensor_mul(pre_m[:], tmp[:], sg_m[:])
                        pre.append(pre_m)

                # ---- per-tb: out projection + residual + rms2 + h2T + gating + gather ----
                # emission interleave: po2 matmuls of tb+1 are queued before the
                # gating chain of tb, so the tensor engine has work while the
                # rms2/transpose chain for tb runs on scalar/vector
                with nc.named_scope("outproj"), tc.tile_pool(name="ps7", bufs=1, space="PSUM") as psA:
                    def emit_po2(tb):
                        po2 = psA.tile([128, 2, 512], FP, tag="po2", bufs=2)
                        for kb in range(MB):
                            for nb in range(2):
                                nc.tensor.matmul(po2[:, nb, :],
                                                 pre[kb][:, tb * 128:(tb + 1) * 128],
                                                 ow_sb[:, kb, nb * 512:(nb + 1) * 512],
                                                 start=(kb == 0), stop=False)
                        for nb in range(2):
                            nc.tensor.matmul(po2[:, nb, :], ones1[:],
                                             ob_sb[:, nb * 512:(nb + 1) * 512],
                                             start=False, stop=True)
                            nc.vector.tensor_add(xmid[tb][:, nb * 512:(nb + 1) * 512],
                                                 po2[:, nb, :],
                                                 xo[tb][:, nb * 512:(nb + 1) * 512])

                    def emit_gate(tb):
                        # rms2 for this tb
                        scr = pt_pool.tile([128, D], FP, tag="scr", bufs=1)
                        sq = pt_pool.tile([128, 1], FP, tag="sq", bufs=2)
                        nc.scalar.activation(scr[:], xmid[tb][:], AF.Square, accum_out=sq[:])
                        nr = pt_pool.tile([128, 1], FP, tag="nr", bufs=2)
                        nc.vector.tensor_scalar(nr[:], sq[:], 1.0 / D, 1e-6, ALU.mult, ALU.add)
                        nc.scalar.sqrt(nr[:], nr[:])
                        nc.vector.reciprocal(nr[:], nr[:])
                        h2 = pt_pool.tile([128, D], FP, tag="h2", bufs=1, name="h2")
                        nc.vector.tensor_scalar(h2[:], xmid[tb][:], nr[:], None, ALU.mult)
                        # gating logits must be fp32: bf16 logits flip top-2
                        # selections vs the reference on near-ties (~0.15 abs
                        # error per flipped token). All 8 transposes batch into
                        # one PSUM tile, then 2 wide vector copies + 1 staging
                        # DMA — avoids per-kb tensor<->vector ping-pong.
                        pl = psA.tile([128, E], FP, tag="pl", bufs=2)
                        ptr8 = psA.tile([128, KB * 128], FP, tag="ptr8", bufs=1)
                        for kb in range(KB):
                            nc.tensor.transpose(ptr8[:, kb * 128:(kb + 1) * 128],
                                                h2[:, kb * 128:(kb + 1) * 128],
                                                ident[:])
                        h2T_t = pt_pool.tile([128, KB * 128], FP, tag="h2T", bufs=1)
                        nc.vector.tensor_copy(h2T_t[:], ptr8[:])
                        h2T_8 = pt_pool.tile([128, KB * 128], F8, tag="h2T8", bufs=2)
                        nc.vector.tensor_copy(h2T_8[:], ptr8[:])
                        nc.sync.dma_start(gth_in[tb][:], h2T_8[:])
                        for kb in range(KB):
                            nc.tensor.matmul(pl[:], h2T_t[:, kb * 128:(kb + 1) * 128],
                                             gw_sb[:, kb, :],
                                             start=(kb == 0), stop=False)
                        nc.tensor.matmul(pl[:], ones1[:], gb_sb[:], start=False, stop=True)
                        # top-2-of-4 gating
                        m1 = pt_pool.tile([128, 1], FP, tag="m1", bufs=2)
                        nc.vector.tensor_reduce(m1[:], pl[:], mybir.AxisListType.X, ALU.max)
                        eq1 = pt_pool.tile([128, E], FP, tag="eq1", bufs=2)
                        nc.vector.tensor_scalar(eq1[:], pl[:], m1[:], None, ALU.is_equal)
                        msk = pt_pool.tile([128, E], FP, tag="msk", bufs=2)
                        nc.vector.scalar_tensor_tensor(msk[:], eq1[:], -1e30, pl[:],
                                                       ALU.mult, ALU.add)
                        m2 = pt_pool.tile([128, 1], FP, tag="m2", bufs=2)
                        nc.vector.tensor_reduce(m2[:], msk[:], mybir.AxisListType.X, ALU.max)
                        eq2 = pt_pool.tile([128, E], FP, tag="eq2", bufs=2)
                        nc.vector.tensor_scalar(eq2[:], msk[:], m2[:], None, ALU.is_equal)
                        dd = pt_pool.tile([128, 1], FP, tag="dd", bufs=2)
                        nc.vector.tensor_sub(dd[:], m2[:], m1[:])
                        p2 = pt_pool.tile([128, 1], FP, tag="p2", bufs=2)
                        nc.scalar.activation(p2[:], dd[:], AF.Sigmoid)
                        p1b = pt_pool.tile([128, 1], FP, tag="p1b", bufs=2)
                        nc.scalar.activation(p1b[:], p2[:], AF.Identity, bias=1.0, scale=-1.0)
                        wv = pt_pool.tile([128, E], FP, tag="wv", bufs=2)
                        nc.vector.tensor_scalar(wv[:], eq1[:], p1b[:], None, ALU.mult)
                        nc.vector.scalar_tensor_tensor(wv[:], eq2[:], p2[:], wv[:],
                                                       ALU.mult, ALU.add)
                        nc.sync.dma_start(gtw_in[tb * 128:(tb + 1) * 128, :], wv[:])
                        if debug_outputs:
                            nc.sync.dma_start(dbg["wown"][tb * 128:(tb + 1) * 128, :], wv[:])
                            nc.sync.dma_start(dbg["xmid"][tb * 128:(tb + 1) * 128, :],
                                              xmid[tb][:])
                        nc.gpsimd.collective_compute(
                            "AllGather", ALU.bypass, replica_groups=rg,
                            ins=[gth_in[tb].opt()], outs=[gth_out[tb].opt()])

                    for tb in range(OTB):
                        emit_po2(tb)
                        emit_gate(tb)
                    with nc.named_scope("gatherw"):
                        nc.gpsimd.collective_compute(
                            "AllGather", ALU.bypass, replica_groups=rg,
                            ins=[gtw_in.opt()], outs=[gtw_out.opt()])

            # =======================================================
            # MoE (full expert per core, token-half group of 4)
            # =======================================================
            with (
                tc.tile_pool(name="moe", bufs=1) as pq,
                tc.tile_pool(name="psC", bufs=1, space="PSUM") as psC,
            ):
                # expert weights resident in SBUF for all 4 rounds.
                # w1 runs in fp8 DoubleRow: ew1 arrives pre-scaled by W1SCALE and
                # host-interleaved to [p, h, two, m] per k-pair so each LDWEIGHTS
                # slice [128, 2, 128] is contiguous (strided pair dims fault the PE).
                ew1_sb = [pq.tile([128, HB, 2, 128], F8, name=f"ew1_{i}", tag=f"ew1_{i}")
                          for i in range(KB // 2)]
                for i in range(KB // 2):
                    nc.scalar.dma_start(
                        ew1_sb[i][:], dp["ew1"][:, i * (HB * 256):(i + 1) * (HB * 256)])
                ew2_sb = [pq.tile([128, D], BF, name=f"ew2_{j}", tag=f"ew2_{j}")
                          for j in range(HB)]
                for j in range(HB):
                    nc.scalar.dma_start(ew2_sb[j][:], dp["ew2"][j * 128:(j + 1) * 128, :])

                with nc.named_scope("moe"):
                    for r in range(4):
                        # one wide DMA per peer token-block: src rows are the
                        # peer's [128, kb*128] section, 1KB contiguous lines
                        h2r = pq.tile([128, KB, OWN], F8, tag="h2r", bufs=2)
                        for t_ in range(OTB):
                            nc.gpsimd.dma_start(
                                h2r[:, :, t_ * 128:(t_ + 1) * 128],
                                gth_out[t_][r * 128:(r + 1) * 128, :]
                                .rearrange("p (kb j) -> p kb j", j=128))
                        hid = []
                        for h in range(HB):
                            ph = psC.tile([128, 512], FP, tag="ph", bufs=2)
                            for i in range(KB // 2):
                                nc.tensor.matmul(ph[:], ew1_sb[i][:, h, :, :],
                                                 h2r[:, 2 * i:2 * i + 2, :],
                                                 start=(i == 0), stop=(i == KB // 2 - 1),
                                                 perf_mode=mybir.MatmulPerfMode.DoubleRow)
                            ht = pq.tile([128, OWN], BF, tag=f"hid{h}", bufs=1)
                            nc.scalar.activation(ht[:], ph[:], AF.Gelu, bias=eb1_sb[:, h:h + 1],
                                                 scale=1.0 / W1SCALE)
                            hid.append(ht)
                        # per-token weight for this core's expert
                        wvr = pq.tile([128, OTB, E], FP, tag="wvr", bufs=2)
                        nc.sync.dma_start(
                            wvr[:], gtw_out[r * OWN:(r + 1) * OWN, :]
                            .rearrange("(tb p) e -> p tb e", p=128))
                        ws = []
                        for tb in range(OTB):
                            wm_t = pq.tile([128, E], FP, tag="wm", bufs=2)
                            nc.vector.tensor_mul(wm_t[:], wvr[:, tb, :], esel[:])
                            ws_t = pq.tile([128, 1], FP, tag=f"ws{tb}", bufs=2)
                            nc.vector.tensor_reduce(ws_t[:], wm_t[:], mybir.AxisListType.X,
                                                    ALU.add)
                            ws.append(ws_t)
                        # w2: token-block pairs keep PSUM <= 6 banks
                        for tp in range(2):
                            peo = [psC.tile([128, 2, 512], FP, tag=f"peo{ti}", bufs=1,
                                            name=f"peo{ti}") for ti in range(2)]
                            for h in range(HB):
                                for ti in range(2):
                                    tb = tp * 2 + ti
                                    for nb in range(2):
                                        nc.tensor.matmul(
                                            peo[ti][:, nb, :],
                                            hid[h][:, tb * 128:(tb + 1) * 128],
                                            ew2_sb[h][:, nb * 512:(nb + 1) * 512],
                                            start=(h == 0), stop=False)
                            for ti in range(2):
                                tb = tp * 2 + ti
                                wout = pq.tile([128, D], BF, tag="wout", bufs=2)
                                for nb in range(2):
                                    nc.tensor.matmul(peo[ti][:, nb, :], ones1[:],
                                                     eb2h_sb[:, nb * 512:(nb + 1) * 512],
                                                     start=False, stop=True)
                                    n0 = nb * 512
                                    nc.vector.tensor_scalar(wout[:, n0:n0 + 512],
                                                            peo[ti][:, nb, :],
                                                            ws[tb][:], None, ALU.mult)
                                    # owner (r == e) carries the residual through
                                    # the reduce-scatter
                                    nc.vector.scalar_tensor_tensor(
                                        wout[:, n0:n0 + 512],
                                        xmid[tb][:, n0:n0 + 512], rmask[:, r:r + 1],
                                        wout[:, n0:n0 + 512], ALU.mult, ALU.add)
                                nc.sync.dma_start(
                                    rs_in[r][tb * 128:(tb + 1) * 128, :], wout[:])
                        nc.gpsimd.collective_compute(
                            "ReduceScatter", ALU.add, replica_groups=rg,
                            ins=[rs_in[r].opt()], outs=[rs_out[r].opt()])

                with nc.named_scope("final"):
                    for r in range(4):
                        rsb = pq.tile([128, D], BF, tag="rsb", bufs=2)
                        nc.sync.dma_start(rsb[:], rs_out[r][:])
                        osb = pq.tile([128, D], FP, tag="osb", bufs=1)
                        nc.vector.tensor_copy(osb[:], rsb[:])
                        nc.sync.dma_start(out_d[r * 128:(r + 1) * 128, :], osb[:])

    nc.compile()
    return nc


def host_prep(inputs):
    """Build the 8 per-core input maps from full inputs."""
    import ml_dtypes
    f32 = np.float32
    bf = ml_dtypes.bfloat16
    x = np.ascontiguousarray(np.asarray(inputs["x"], f32).reshape(B * T, D))
    n1 = np.asarray(inputs["norm1_w"], f32)
    n2 = np.asarray(inputs["norm2_w"], f32)
    ipw = np.ascontiguousarray(np.asarray(inputs["in_proj_w"], f32) * n1[:, None]).astype(bf)
    gw = np.ascontiguousarray(np.asarray(inputs["gate_w"], f32) * n2[:, None])
    ew1f = np.asarray(inputs["e_w1"], f32) * n2[None, :, None]
    ew1q = np.clip(ew1f * 64.0, -240.0, 240.0).astype(ml_dtypes.float8_e4m3)
    # [E, k, hid] -> [E, p, i, h, two, m]: k = i*256 + two*128 + p, hid = h*128 + m
    ew1b = ew1q.reshape(E, 4, 2, 128, HID // 128, 128).transpose(0, 3, 1, 4, 2, 5)
    ew1b = np.ascontiguousarray(ew1b.reshape(E, 128, -1))
    ew2b = np.asarray(inputs["e_w2"], f32).astype(bf)
    ident = np.eye(128, dtype=f32)
    ones1 = np.ones((1, 128), f32)
    shared = {
        "ipw": ipw, "ipb": np.asarray(inputs["in_proj_b"], f32),
        "cw": np.ascontiguousarray(np.asarray(inputs["conv_w"], f32)[:, 0, :]),
        "cb": np.asarray(inputs["conv_b"], f32),
        "dtw": np.asarray(inputs["dt_w"], f32).astype(bf),
        "dtb": np.asarray(inputs["dt_b"], f32),
        "bpw": np.asarray(inputs["bp_w"], f32).astype(bf),
        "bpb": np.asarray(inputs["bp_b"], f32),
        "cpw": np.asarray(inputs["cp_w"], f32).astype(bf),
        "cpb": np.asarray(inputs["cp_b"], f32),
        "s2iw": np.asarray(inputs["s2i_w"], f32).astype(bf),
        "s2ib": np.asarray(inputs["s2i_b"], f32),
        "Dp": np.asarray(inputs["D_param"], f32),
        "ow": np.asarray(inputs["out_w"], f32).astype(bf),
        "ob": np.asarray(inputs["out_b"], f32),
        "gw": gw, "gb": np.asarray(inputs["gate_b"], f32),
        "ident": ident, "identb": ident.astype(bf), "ones1": ones1,
    }
    eb1 = np.asarray(inputs["e_b1"], f32)
    eb2 = np.asarray(inputs["e_b2"], f32)
    in_maps = []
    for c in range(N_CORES):
        e, th = c // 2, c % 2
        g0 = th * (B * T // 2) + e * OWN
        if e == 0:
            x_sh = np.concatenate([np.zeros((HALO, D), f32), x[g0:g0 + OWN]])
        else:
            x_sh = x[g0 - HALO:g0 + OWN]
        m = dict(shared)
        m["x_sh"] = np.ascontiguousarray(x_sh)
        m["ew1"] = np.ascontiguousarray(ew1b[e])
        m["eb1"] = np.ascontiguousarray(eb1[e])
        m["ew2"] = np.ascontiguousarray(ew2b[e])
        m["eb2h"] = np.ascontiguousarray(eb2[e])
        esel = np.zeros((128, E), f32)
        esel[:, e] = 1.0
        m["esel"] = esel
        rmask = np.zeros((128, 4), f32)
        rmask[:, e] = 1.0
        m["rmask"] = rmask
        in_maps.append(m)
    return in_maps


def unshard_out(results):
    """results: list of 8 dicts with 'out' [OWN, D]; rows r*128+i of core c
    hold global token (c%2)*2048 + r*512 + (c//2)*128 + i."""
    full = np.empty((B * T, D), np.float32)
    for c in range(N_CORES):
        e, th = c // 2, c % 2
        oc = results[c]["out"]
        for r in range(4):
            full[th * 2048 + r * OWN + e * 128: th * 2048 + r * OWN + (e + 1) * 128] = \
                oc[r * 128:(r + 1) * 128]
    return full.reshape(B, T, D)


_NC_CACHE = {}


def _get_nc():
    if "nc" not in _NC_CACHE:
        _NC_CACHE["nc"] = build(debug_outputs=False)
    return _NC_CACHE["nc"]


def kernel(**inputs) -> np.ndarray:
    """Full-input entry point: shards across 8 NeuronCores, runs the Bass
    kernel SPMD, reassembles the full [2, 2048, 1024] output."""
    import sys, types
    try:  # NTFF profile hook shim (missing antenv.axon_hooks in this image)
        import antenv.axon_hooks  # noqa: F401
    except ImportError:
        try:
            import antenv
            from trn_agent_boot.trn_boot import _ntff_profile_via_ctypes
            mod = types.ModuleType("antenv.axon_hooks")
            try:
                _hook = _ntff_profile_via_ctypes("/opt/axon/libaxon_pjrt.so")
            except Exception:
                _hook = None
            mod.get_axon_ntff_profile_hook = lambda: _hook
            mod.set_axon_ntff_profile_hook = lambda h: None
            sys.modules["antenv.axon_hooks"] = mod
            antenv.axon_hooks = mod
        except Exception:
            pass
    from concourse.bass_utils import run_bass_kernel_spmd

    nc = _get_nc()
    in_maps = host_prep(inputs)
    res = run_bass_kernel_spmd(nc, in_maps, core_ids=list(range(N_CORES)))
    out = unshard_out(res.results)
    return out.astype(np.float32)


# revision 45
# speedup vs baseline: 1.8551x; 1.0016x over previous
"""Bass kernel builder for nn_MixtureOfMambaBlock — 8-core SPMD.

Sharding: tokens 8-way (512/core + 64 halo for conv+scan warmup); mixer fully
local per core (weights replicated, bf16 matmuls; fp32 gating logits to keep
top-2 selection exact). Post-mixer h2 all-gathered in fp8; MoE is expert x
token-half sharded: w1 runs fp8 DoubleRow (weights pre-scaled x64,
host-interleaved k-pairs), w2 in bf16 with both expert weights SBUF-resident.
Weighted expert partials + residual reduce-scattered in bf16 back to token
shards.
"""
import numpy as np
import concourse.bass as bass
import concourse.bacc as bacc
import concourse.mybir as mybir
import concourse.tile as tile

FP = mybir.dt.float32
FR = mybir.dt.float32r
BF = mybir.dt.bfloat16
F8 = mybir.dt.float8e4
W1SCALE = 64.0
AF = mybir.ActivationFunctionType
ALU = mybir.AluOpType

B, T, D = 2, 2048, 1024
S, INNER = 64, 2048
E = 4
HID = 4096
OWN, HALO = 512, 64
NH = OWN + HALO          # 576
KB = D // 128            # 8  d-blocks
MB = INNER // 128        # 16 inner-blocks
HB = HID // 128          # 32 hid-blocks
OTB = OWN // 128         # 4  own-token blocks
N_CORES = 8

INPUT_SPECS = {
    "x_sh": ([NH, D], FP),
    "ipw": ([D, 2 * INNER], BF), "ipb": ([2 * INNER], FP),
    "cw": ([INNER, 3], FP), "cb": ([INNER], FP),
    "dtw": ([INNER, S], BF), "dtb": ([S], FP),
    "bpw": ([INNER, S], BF), "bpb": ([S], FP),
    "cpw": ([INNER, S], BF), "cpb": ([S], FP),
    "s2iw": ([S, INNER], BF), "s2ib": ([INNER], FP),
    "Dp": ([INNER], FP),
    "ow": ([INNER, D], BF), "ob": ([D], FR),
    "gw": ([D, E], FP), "gb": ([E], FR),
    "ew1": ([128, KB // 2 * HID // 128 * 256], F8), "eb1": ([HID], FP),
    "ew2": ([HID, D], BF), "eb2h": ([D], FR),
    "esel": ([128, E], FP),
    "rmask": ([128, 4], FP),
    "ident": ([128, 128], FP),
    "identb": ([128, 128], BF),
    "ones1": ([1, 128], FR),
}


def build(debug_outputs=False):
    nc = bacc.Bacc("TRN2", target_bir_lowering=False, debug=False,
                   num_devices=N_CORES)
    dp = {}
    for name, (shape, dt) in INPUT_SPECS.items():
        dp[name] = nc.dram_tensor(name, shape, dt, kind="ExternalInput")
    out_d = nc.dram_tensor("out", [OWN, D], FP, kind="ExternalOutput")
    dbg = {}
    if debug_outputs:
        dbg["xmid"] = nc.dram_tensor("dbg_xmid", [OWN, D], FP, kind="ExternalOutput")
        dbg["wown"] = nc.dram_tensor("dbg_wown", [OWN, E], FP, kind="ExternalOutput")

    rg = [[0, 2, 4, 6], [1, 3, 5, 7]]

    with tile.TileContext(nc) as tc:
        with (
            tc.tile_pool(name="outer", bufs=1) as po,
            tc.tile_pool(name="dram", bufs=1, space="DRAM") as pdram,
        ):
            # ---------- DRAM bounce buffers for collectives ----------
            # gth layout per tb: [128 d-in-block, kb*128 tok] — matches the
            # transpose PSUM tile directly (1 staging DMA) and lets the MoE
            # load each peer row-block with a single wide DMA.
            gth_in = [pdram.tile([128, KB * 128], F8, name=f"gth_in{t_}")
                      for t_ in range(OTB)]
            gth_out = [pdram.tile([4 * 128, KB * 128], F8, name=f"gth_out{t_}")
                       for t_ in range(OTB)]
            gtw_in = pdram.tile([OWN, E], FP)
            gtw_out = pdram.tile([4 * OWN, E], FP)
            rs_in = [pdram.tile([OWN, D], BF, name=f"rs_in{r}") for r in range(3)]
            rs_out = [pdram.tile([128, D], BF, name=f"rs_out{r}") for r in range(3)]
            # last round splits its reduce-scatter into two column halves so the
            # first half's wire hides under the second half's compute
            rs3 = [pdram.tile([OWN, 512], BF, name=f"rs3_{nb}") for nb in range(2)]
            rs3_out = [pdram.tile([128, 512], BF, name=f"rs3o_{nb}") for nb in range(2)]

            # ---------- constants / small weights (emit all loads up front) ----
            ident = po.tile([128, 128], FP)
            nc.sync.dma_start(ident[:], dp["ident"][:])
            identb = po.tile([128, 128], BF)
            nc.sync.dma_start(identb[:], dp["identb"][:])

            def load_pcol(name, blocks):  # [blocks*128] -> [128, blocks]
                t = po.tile([128, blocks], FP, name=f"{name}_sb")
                nc.sync.dma_start(
                    t[:], dp[name].ap().rearrange("(m p) -> p m", p=128))
                return t

            def load_vec1(name, n):  # [n] -> [n, 1]
                t = po.tile([n, 1], FP, name=f"{name}_sb")
                nc.sync.dma_start(t[:], dp[name].ap().rearrange("(s o) -> s o", o=1))
                return t

            def load_row(name, n, dt_=FP):  # [n] -> [1, n]
                t = po.tile([1, n], dt_, name=f"{name}_sb")
                nc.sync.dma_start(t[:], dp[name].ap().rearrange("(o s) -> o s", o=1))
                return t

            def load_kw(name, pool):  # [2048, 64] -> [128, 16, 64], lhsT slice [:, kb, :]
                t = pool.tile([128, MB, S], BF, name=f"{name}_sb")
                nc.sync.dma_start(t[:], dp[name].ap().rearrange("(kb p) s -> p kb s", p=128))
                return t

            ob_sb = load_row("ob", D, FR)
            gb_sb = load_row("gb", E, FR)
            eb2h_sb = load_row("eb2h", D, FR)
            ones1 = po.tile([1, 128], FR)
            nc.sync.dma_start(ones1[:], dp["ones1"][:])
            ipb_sb = load_pcol("ipb", 32)
            cb_sb = load_pcol("cb", 16)
            cw_sb = po.tile([128, 16, 3], FP)  # [p, m, k]
            nc.sync.dma_start(cw_sb[:], dp["cw"].ap().rearrange("(m p) k -> p m k", p=128))
            dtb_sb = load_vec1("dtb", S)
            bpb_sb = load_vec1("bpb", S)
            cpb_sb = load_vec1("cpb", S)
            s2ib_sb = load_pcol("s2ib", 16)
            Dp_sb = load_pcol("Dp", 16)
            gw_sb = po.tile([128, KB, E], FP)  # [p, kb, e]
            nc.sync.dma_start(gw_sb[:], dp["gw"].ap().rearrange("(kb p) e -> p kb e", p=128))
            esel = po.tile([128, E], FP)
            nc.sync.dma_start(esel[:], dp["esel"][:])
            rmask = po.tile([128, 4], FP)
            nc.sync.dma_start(rmask[:], dp["rmask"][:])
            eb1_sb = load_pcol("eb1", HB)

            # persistent activations
            xmid = [po.tile([128, D], FP, name=f"xmid{t_}", tag=f"xmid{t_}")
                    for t_ in range(OTB)]

            # =======================================================
            # MIXER
            # =======================================================
            with (
                tc.tile_pool(name="mixer", bufs=1) as pm,
                tc.tile_pool(name="mixt", bufs=1) as pt_pool,
            ):
                # pool allocation order matters: tiles that die early (hT, xm,
                # sg, projection weights) go FIRST so their addresses sit at the
                # pool base — the MoE pool's ew1 tiles (allocated first there)
                # land on them and can start loading before outproj finishes.
                hT = [pm.tile([128, NH], BF, name=f"hT{kb}", tag=f"hT{kb}") for kb in range(KB)]
                xm = [pm.tile([128, NH], BF, name=f"xm{m}", tag=f"xm{m}") for m in range(MB)]
                sg = [pm.tile([128, OWN], BF, name=f"sg{m}", tag=f"sg{m}")
                      for m in range(8)]
                dtw_sb = load_kw("dtw", pm)
                bpw_sb = load_kw("bpw", pm)
                cpw_sb = load_kw("cpw", pm)
                s2iw_sb = pm.tile([S, INNER], BF, name="s2iw_sb")
                nc.sync.dma_start(s2iw_sb[:], dp["s2iw"][:])
                # late-freed tiles (used through outproj) at higher addresses
                ow_sb = pm.tile([128, MB, D], BF, name="ow_sb")
                nc.scalar.dma_start(
                    ow_sb[:], dp["ow"].ap().rearrange("(kb p) d -> p kb d", p=128))
                xo = [pm.tile([128, D], FP, name=f"xo{t_}", tag=f"xo{t_}")
                      for t_ in range(OTB)]

                # ---- rmsnorm1 + transpose to hT ----
                # chunks: [64 halo] + 4x [128 own]
                chunks = [(0, HALO, None)] + [
                    (HALO + t_ * 128, 128, t_) for t_ in range(OTB)]
                with nc.named_scope("rms1"), tc.tile_pool(name="ps1", bufs=1, space="PSUM") as psA:
                    for (row0, rows, t_) in chunks:
                        if t_ is None:
                            xt = pt_pool.tile([HALO, D], FP, tag="xt0")
                        else:
                            xt = xo[t_]
                        # gpsimd queue: ahead of the ipw weight chunks, and not
                        # behind the ~20 small constant loads on the sync queue
                        nc.gpsimd.dma_start(xt[:], dp["x_sh"][row0:row0 + rows, :])
                        scr = pt_pool.tile([128, D], FP, tag="scr", bufs=1)
                        sq = pt_pool.tile([128, 1], FP, tag="sq", bufs=2)
                        nc.scalar.activation(scr[0:rows, :], xt[:], AF.Square,
                                             accum_out=sq[0:rows, :])
                        nr = pt_pool.tile([128, 1], FP, tag="nr", bufs=2)
                        nc.vector.tensor_scalar(nr[0:rows, :], sq[0:rows, :], 1.0 / D,
                                                1e-6, ALU.mult, ALU.add)
                        nc.scalar.sqrt(nr[0:rows, :], nr[0:rows, :])
                        nc.vector.reciprocal(nr[0:rows, :], nr[0:rows, :])
                        h_t = pt_pool.tile([128, D], BF, tag="htb", bufs=2)
                        nc.vector.tensor_scalar(h_t[0:rows, :], xt[:], nr[0:rows, :],
                                                None, ALU.mult)
                        for kb in range(KB):
                            ptr = psA.tile([128, 128], BF, tag="ptr", bufs=2)
                            nc.tensor.transpose(ptr[:, 0:rows],
                                                h_t[0:rows, kb * 128:(kb + 1) * 128],
                                                identb[0:rows, 0:rows])
                            nc.vector.tensor_copy(hT[kb][:, row0:row0 + rows],
                                                  ptr[:, 0:rows])

                # ---- in_proj (x_main half) + conv + silu ----
                with nc.named_scope("in_proj"), tc.tile_pool(name="ps2", bufs=1, space="PSUM") as psA:
                    for q in range(4):
                        wq = []
                        for kb in range(KB):
                            wt = pt_pool.tile([128, 512], BF, tag=f"wip{kb}", bufs=2,
                                              name=f"wip{kb}")
                            nc.gpsimd.dma_start(
                                wt[:], dp["ipw"][kb * 128:(kb + 1) * 128,
                                                 q * 512:(q + 1) * 512])
                            wq.append(wt)
                        for mi in range(4):
                            m = q * 4 + mi
                            xzp = pt_pool.tile([128, NH + 2], FP, tag="xzp", bufs=2)
                            nc.vector.memset(xzp[:, 0:2], 0.0)
                            for n0, nw in ((0, 512), (512, NH - 512)):
                                px = psA.tile([128, 512], FP, tag="px", bufs=2)
                                for kb in range(KB):
                                    nc.tensor.matmul(px[:, 0:nw],
                                                     wq[kb][:, mi * 128:(mi + 1) * 128],
                                                     hT[kb][:, n0:n0 + nw],
                                                     start=(kb == 0), stop=(kb == KB - 1))
                                nc.scalar.activation(xzp[:, 2 + n0:2 + n0 + nw], px[:, 0:nw],
                                                     AF.Identity, bias=ipb_sb[:, m:m + 1])
                            cv = pt_pool.tile([128, NH], FP, tag="cv", bufs=2)
                            nc.vector.tensor_scalar(cv[:], xzp[:, 0:NH], cw_sb[:, m, 0:1],
                                                    None, ALU.mult)
                            nc.vector.scalar_tensor_tensor(cv[:], xzp[:, 1:1 + NH],
                                                           cw_sb[:, m, 1:2], cv[:],
                                                           ALU.mult, ALU.add)
                            nc.vector.scalar_tensor_tensor(cv[:], xzp[:, 2:2 + NH],
                                                           cw_sb[:, m, 2:3], cv[:],
                                                           ALU.mult, ALU.add)
                            sgc = pt_pool.tile([128, NH], BF, tag="sgc", bufs=2)
                            nc.scalar.activation(sgc[:], cv[:], AF.Sigmoid, bias=cb_sb[:, m:m + 1])
                            nc.vector.scalar_tensor_tensor(xm[m][:], cv[:], cb_sb[:, m:m + 1],
                                                           sgc[:], ALU.add, ALU.mult)

                # ---- dt/B/C projections (emitted before gate MMs; feed scan) ----
                with nc.named_scope("scan"), tc.tile_pool(name="ps3", bufs=1, space="PSUM") as psA:
                    dt_t = pt_pool.tile([S, NH], FP, tag="dt")
                    a_t = pt_pool.tile([S, NH], FP, tag="a")
                    b_t = pt_pool.tile([S, NH], FP, tag="b")
                    c_t = pt_pool.tile([S, NH], FP, tag="c")
                    for n0, nw in ((0, 512), (512, NH - 512)):
                        for wsb, bias_sb, dst, fn in (
                            (dtw_sb, dtb_sb, dt_t, AF.Sigmoid),
                            (cpw_sb, cpb_sb, c_t, AF.Identity),
                        ):
                            pz = psA.tile([S, 512], FP, tag="pz", bufs=2)
                            for kb in range(MB):
                                nc.tensor.matmul(pz[:, 0:nw], wsb[:, kb, :],
                                                 xm[kb][:, n0:n0 + nw],
                                                 start=(kb == 0), stop=(kb == MB - 1))
                            nc.scalar.activation(dst[:, n0:n0 + nw], pz[:, 0:nw], fn,
                                                 bias=bias_sb[:])
                        # b needs dt -> separate pass
                        pz = psA.tile([S, 512], FP, tag="pz", bufs=2)
                        for kb in range(MB):
                            nc.tensor.matmul(pz[:, 0:nw], bpw_sb[:, kb, :],
                                             xm[kb][:, n0:n0 + nw],
                                             start=(kb == 0), stop=(kb == MB - 1))
                        nc.vector.scalar_tensor_tensor(b_t[:, n0:n0 + nw], pz[:, 0:nw],
                                                       bpb_sb[:], dt_t[:, n0:n0 + nw],
                                                       ALU.add, ALU.mult)
                    # scan runs on the vector engine while the tensor engine
                    # works through the gate-projection matmuls below
                    nc.vector.tensor_scalar(a_t[:], dt_t[:], -1.0, 1.0,
                                            ALU.mult, ALU.add)
                    st_t = pt_pool.tile([S, NH], FP, tag="st")
                    nc.vector.tensor_tensor_scan(st_t[:], a_t[:], b_t[:], 0.0,
                                                 ALU.mult, ALU.add)
                    y_t = pt_pool.tile([S, OWN], FP, tag="yt", name="y_t")
                    nc.vector.tensor_mul(y_t[:], c_t[:, HALO:NH], st_t[:, HALO:NH])

                # ---- gate half of in_proj, first 8 m: emitted NOW so the
                # tensor queue has work while the (vector-engine) scan runs ----
                def load_wqg(q):
                    wqg = []
                    for kb in range(KB):
                        wt = pt_pool.tile([128, 512], BF, tag=f"wip{kb}", bufs=2,
                                          name=f"wipg{kb}_{q}")
                        nc.gpsimd.dma_start(
                            wt[:], dp["ipw"][kb * 128:(kb + 1) * 128,
                                             2048 + q * 512:2048 + (q + 1) * 512])
                        wqg.append(wt)
                    return wqg

                def gate_mm(psB, wqg, m):
                    mi = m % 4
                    pg = psB.tile([128, 512], FP, tag="pg", bufs=2)
                    for kb in range(KB):
                        nc.tensor.matmul(pg[:], wqg[kb][:, mi * 128:(mi + 1) * 128],
                                         hT[kb][:, HALO:NH],
                                         start=(kb == 0), stop=(kb == KB - 1))
                    return pg

                sgi_pre = {}
                with nc.named_scope("gateproj"), tc.tile_pool(name="ps4", bufs=1, space="PSUM") as psB:
                    for q in range(2):
                        wqg = load_wqg(q)
                        for mi in range(4):
                            m = q * 4 + mi
                            pg = gate_mm(psB, wqg, m)
                            nc.scalar.activation(sg[m][:], pg[:], AF.Sigmoid,
                                                 bias=ipb_sb[:, MB + m:MB + m + 1])
                    # two more gate projections emitted here: covers the tail of
                    # the scan+LN vector chain before the LN transposes
                    wqg2 = load_wqg(2)
                    for m in (8, 9):
                        pg = gate_mm(psB, wqg2, m)
                        sg_m = pt_pool.tile([128, OWN], BF, tag="sgi", bufs=2)
                        nc.scalar.activation(sg_m[:], pg[:], AF.Sigmoid,
                                             bias=ipb_sb[:, MB + m:MB + m + 1])
                        sgi_pre[m] = sg_m

                # ---- layernorm over S ----
                with nc.named_scope("scanln"), tc.tile_pool(name="ps5", bufs=1, space="PSUM") as psA:
                    yln = pt_pool.tile([S, OWN], BF, tag="a", name="yln")
                    for i in range(OTB):
                        ptr = psA.tile([128, 128], FP, tag="ptr", bufs=2)
                        nc.tensor.transpose(ptr[:, 0:S], y_t[:, i * 128:(i + 1) * 128],
                                            ident[0:S, 0:S])
                        yT = pt_pool.tile([128, S], FP, tag="yT", bufs=2)
                        nc.vector.tensor_copy(yT[:], ptr[:, 0:S])
                        mu = pt_pool.tile([128, 1], FP, tag="mu", bufs=2)
                        nc.vector.tensor_reduce(mu[:], yT[:], mybir.AxisListType.X, ALU.add)
                        nc.vector.tensor_scalar_mul(mu[:], mu[:], 1.0 / S)
                        xc = pt_pool.tile([128, S], FP, tag="xc", bufs=2)
                        nc.vector.tensor_scalar_sub(xc[:], yT[:], mu[:])
                        scr2 = pt_pool.tile([128, S], FP, tag="scr2", bufs=2)
                        vv = pt_pool.tile([128, 1], FP, tag="vv", bufs=2)
                        nc.scalar.activation(scr2[:], xc[:], AF.Square, accum_out=vv[:])
                        nc.vector.tensor_scalar(vv[:], vv[:], 1.0 / S, 1e-5, ALU.mult, ALU.add)
                        nc.scalar.sqrt(vv[:], vv[:])
                        nc.vector.reciprocal(vv[:], vv[:])
                        xcb = pt_pool.tile([128, S], BF, tag="xcb", bufs=2)
                        nc.vector.tensor_scalar_mul(xcb[:], xc[:], vv[:])
                        ptr2 = psA.tile([128, 128], BF, tag="ptr2", bufs=2)
                        nc.tensor.transpose(ptr2[0:S, :], xcb[:], identb[:])
                        nc.vector.tensor_copy(yln[:, i * 128:(i + 1) * 128], ptr2[0:S, :])

                # ---- s2i + pre_out assembly (gate m>=8 computed inline) ----
                with nc.named_scope("premix"), tc.tile_pool(name="ps6", bufs=1, space="PSUM") as psA:
                    pre = []
                    wqg = wqg2
                    for m in range(MB):
                        if m in sgi_pre:
                            sg_m = sgi_pre[m]
                        elif m >= 10:
                            if m == 12:
                                wqg = load_wqg(3)
                            pg = gate_mm(psA, wqg, m)
                            sg_m = pt_pool.tile([128, OWN], BF, tag="sgi", bufs=2)
                            nc.scalar.activation(sg_m[:], pg[:], AF.Sigmoid,
                                                 bias=ipb_sb[:, MB + m:MB + m + 1])
                        else:
                            sg_m = sg[m]
                        ps = psA.tile([128, 512], FP, tag="ps", bufs=2)
                        nc.tensor.matmul(ps[:], s2iw_sb[:, m * 128:(m + 1) * 128], yln[:],
                                         start=True, stop=True)
                        # Dp*xm + s2ib on the scalar engine; 2 vector ops total
                        tmp = pt_pool.tile([128, OWN], FP, tag="tmp", bufs=2)
                        nc.scalar.activation(tmp[:], xm[m][:, HALO:NH], AF.Identity,
                                             bias=s2ib_sb[:, m:m + 1],
                                             scale=Dp_sb[:, m:m + 1])
                        nc.vector.tensor_add(tmp[:], tmp[:], ps[:])
                        pre_m = pm.tile([128, OWN], BF, name=f"pre{m}", tag=f"pre{m}")
                        nc.vector.tensor_mul(pre_m[:], tmp[:], sg_m[:])
                        pre.append(pre_m)

                # ---- per-tb: out projection + residual + rms2 + h2T + gating + gather ----
                # emission interleave: po2 matmuls of tb+1 are queued before the
                # gating chain of tb, so the tensor engine has work while the
                # rms2/transpose chain for tb runs on scalar/vector
                with nc.named_scope("outproj"), tc.tile_pool(name="ps7", bufs=1, space="PSUM") as psA:
                    def emit_po2(tb):
                        po2 = psA.tile([128, 2, 512], FP, tag="po2", bufs=2)
                        for kb in range(MB):
                            for nb in range(2):
                                nc.tensor.matmul(po2[:, nb, :],
                                                 pre[kb][:, tb * 128:(tb + 1) * 128],
                                                 ow_sb[:, kb, nb * 512:(nb + 1) * 512],
                                                 start=(kb == 0), stop=False)
                        for nb in range(2):
                            nc.tensor.matmul(po2[:, nb, :], ones1[:],
                                             ob_sb[:, nb * 512:(nb + 1) * 512],
                                             start=False, stop=True)
                            nc.vector.tensor_add(xmid[tb][:, nb * 512:(nb + 1) * 512],
                                                 po2[:, nb, :],
                                                 xo[tb][:, nb * 512:(nb + 1) * 512])

                    def emit_gate(tb):
                        # rms2 for this tb
                        scr = pt_pool.tile([128, D], FP, tag="scr", bufs=1)
                        sq = pt_pool.tile([128, 1], FP, tag="sq", bufs=2)
                        nc.scalar.activation(scr[:], xmid[tb][:], AF.Square, accum_out=sq[:])
                        nr = pt_pool.tile([128, 1], FP, tag="nr", bufs=2)
                        nc.vector.tensor_scalar(nr[:], sq[:], 1.0 / D, 1e-6, ALU.mult, ALU.add)
                        nc.scalar.sqrt(nr[:], nr[:])
                        nc.vector.reciprocal(nr[:], nr[:])
                        h2 = pt_pool.tile([128, D], FP, tag="h2", bufs=1, name="h2")
                        nc.vector.tensor_scalar(h2[:], xmid[tb][:], nr[:], None, ALU.mult)
                        # gating logits must be fp32: bf16 logits flip top-2
                        # selections vs the reference on near-ties (~0.15 abs
                        # error per flipped token). All 8 transposes batch into
                        # one PSUM tile, then 2 wide vector copies + 1 staging
                        # DMA — avoids per-kb tensor<->vector ping-pong.
                        pl = psA.tile([128, E], FP, tag="pl", bufs=2)
                        ptr8 = psA.tile([128, KB * 128], FP, tag="ptr8", bufs=1)
                        for kb in range(KB):
                            nc.tensor.transpose(ptr8[:, kb * 128:(kb + 1) * 128],
                                                h2[:, kb * 128:(kb + 1) * 128],
                                                ident[:])
                        h2T_t = pt_pool.tile([128, KB * 128], FP, tag="h2T", bufs=1)
                        nc.vector.tensor_copy(h2T_t[:], ptr8[:])
                        h2T_8 = pt_pool.tile([128, KB * 128], F8, tag="h2T8", bufs=2)
                        nc.vector.tensor_copy(h2T_8[:], ptr8[:])
                        nc.sync.dma_start(gth_in[tb][:], h2T_8[:])
                        for kb in range(KB):
                            nc.tensor.matmul(pl[:], h2T_t[:, kb * 128:(kb + 1) * 128],
                                             gw_sb[:, kb, :],
                                             start=(kb == 0), stop=False)
                        nc.tensor.matmul(pl[:], ones1[:], gb_sb[:], start=False, stop=True)
                        # top-2-of-4 gating
                        m1 = pt_pool.tile([128, 1], FP, tag="m1", bufs=2)
                        nc.vector.tensor_reduce(m1[:], pl[:], mybir.AxisListType.X, ALU.max)
                        eq1 = pt_pool.tile([128, E], FP, tag="eq1", bufs=2)
                        nc.vector.tensor_scalar(eq1[:], pl[:], m1[:], None, ALU.is_equal)
                        msk = pt_pool.tile([128, E], FP, tag="msk", bufs=2)
                        nc.vector.scalar_tensor_tensor(msk[:], eq1[:], -1e30, pl[:],
                                                       ALU.mult, ALU.add)
                        m2 = pt_pool.tile([128, 1], FP, tag="m2", bufs=2)
                        nc.vector.tensor_reduce(m2[:], msk[:], mybir.AxisListType.X, ALU.max)
                        eq2 = pt_pool.tile([128, E], FP, tag="eq2", bufs=2)
                        nc.vector.tensor_scalar(eq2[:], msk[:], m2[:], None, ALU.is_equal)
                        dd = pt_pool.tile([128, 1], FP, tag="dd", bufs=2)
                        nc.vector.tensor_sub(dd[:], m2[:], m1[:])
                        p2 = pt_pool.tile([128, 1], FP, tag="p2", bufs=2)
                        nc.scalar.activation(p2[:], dd[:], AF.Sigmoid)
                        p1b = pt_pool.tile([128, 1], FP, tag="p1b", bufs=2)
                        nc.scalar.activation(p1b[:], p2[:], AF.Identity, bias=1.0, scale=-1.0)
                        wv = pt_pool.tile([128, E], FP, tag="wv", bufs=2)
                        nc.vector.tensor_scalar(wv[:], eq1[:], p1b[:], None, ALU.mult)
                        nc.vector.scalar_tensor_tensor(wv[:], eq2[:], p2[:], wv[:],
                                                       ALU.mult, ALU.add)
                        nc.sync.dma_start(gtw_in[tb * 128:(tb + 1) * 128, :], wv[:])
                        if debug_outputs:
                            nc.sync.dma_start(dbg["wown"][tb * 128:(tb + 1) * 128, :], wv[:])
                            nc.sync.dma_start(dbg["xmid"][tb * 128:(tb + 1) * 128, :],
                                              xmid[tb][:])
                        nc.gpsimd.collective_compute(
                            "AllGather", ALU.bypass, replica_groups=rg,
                            ins=[gth_in[tb].opt()], outs=[gth_out[tb].opt()])

                    for tb in range(OTB):
                        emit_po2(tb)
                        emit_gate(tb)
                    with nc.named_scope("gatherw"):
                        nc.gpsimd.collective_compute(
                            "AllGather", ALU.bypass, replica_groups=rg,
                            ins=[gtw_in.opt()], outs=[gtw_out.opt()])

            # =======================================================
            # MoE (full expert per core, token-half group of 4)
            # =======================================================
            with (
                tc.tile_pool(name="moe", bufs=1) as pq,
                tc.tile_pool(name="psC", bufs=1, space="PSUM") as psC,
            ):
                # expert weights resident in SBUF for all 4 rounds.
                # w1 runs in fp8 DoubleRow: ew1 arrives pre-scaled by W1SCALE and
                # host-interleaved to [p, h, two, m] per k-pair so each LDWEIGHTS
                # slice [128, 2, 128] is contiguous (strided pair dims fault the PE).
                ew1_sb = [pq.tile([128, HB, 2, 128], F8, name=f"ew1_{i}", tag=f"ew1_{i}")
                          for i in range(KB // 2)]
                for i in range(KB // 2):
                    nc.scalar.dma_start(
                        ew1_sb[i][:], dp["ew1"][:, i * (HB * 256):(i + 1) * (HB * 256)])
                ew2_sb = [pq.tile([128, D], BF, name=f"ew2_{j}", tag=f"ew2_{j}")
                          for j in range(HB)]
                for j in range(HB):
                    nc.scalar.dma_start(ew2_sb[j][:], dp["ew2"][j * 128:(j + 1) * 128, :])

                with nc.named_scope("moe"):
                    for r in range(4):
                        # one wide DMA per peer token-block: src rows are the
                        # peer's [128, kb*128] section, 1KB contiguous lines
                        h2r = pq.tile([128, KB, OWN], F8, tag="h2r", bufs=2)
                        for t_ in range(OTB):
                            nc.gpsimd.dma_start(
                                h2r[:, :, t_ * 128:(t_ + 1) * 128],
                                gth_out[t_][r * 128:(r + 1) * 128, :]
                                .rearrange("p (kb j) -> p kb j", j=128))
                        hid = []
                        for h in range(HB):
                            ph = psC.tile([128, 512], FP, tag="ph", bufs=2)
                            for i in range(KB // 2):
                                nc.tensor.matmul(ph[:], ew1_sb[i][:, h, :, :],
                                                 h2r[:, 2 * i:2 * i + 2, :],
                                                 start=(i == 0), stop=(i == KB // 2 - 1),
                                                 perf_mode=mybir.MatmulPerfMode.DoubleRow)
                            ht = pq.tile([128, OWN], BF, tag=f"hid{h}", bufs=1)
                            nc.scalar.activation(ht[:], ph[:], AF.Gelu, bias=eb1_sb[:, h:h + 1],
                                                 scale=1.0 / W1SCALE)
                            hid.append(ht)
                        # per-token weight for this core's expert
                        wvr = pq.tile([128, OTB, E], FP, tag="wvr", bufs=2)
                        nc.sync.dma_start(
                            wvr[:], gtw_out[r * OWN:(r + 1) * OWN, :]
                            .rearrange("(tb p) e -> p tb e", p=128))
                        ws = []
                        for tb in range(OTB):
                            wm_t = pq.tile([128, E], FP, tag="wm", bufs=2)
                            nc.vector.tensor_mul(wm_t[:], wvr[:, tb, :], esel[:])
                            ws_t = pq.tile([128, 1], FP, tag=f"ws{tb}", bufs=2)
                            nc.vector.tensor_reduce(ws_t[:], wm_t[:], mybir.AxisListType.X,
                                                    ALU.add)
                            ws.append(ws_t)
                        if r < 3:
                            # w2: token-block pairs keep PSUM <= 6 banks
                            for tp in range(2):
                                peo = [psC.tile([128, 2, 512], FP, tag=f"peo{ti}", bufs=1,
                                                name=f"peo{ti}") for ti in range(2)]
                                for h in range(HB):
                                    for ti in range(2):
                                        tb = tp * 2 + ti
                                        for nb in range(2):
                                            nc.tensor.matmul(
                                                peo[ti][:, nb, :],
                                                hid[h][:, tb * 128:(tb + 1) * 128],
                                                ew2_sb[h][:, nb * 512:(nb + 1) * 512],
                                                start=(h == 0), stop=False)
                                for ti in range(2):
                                    tb = tp * 2 + ti
                                    wout = pq.tile([128, D], BF, tag="wout", bufs=2)
                                    for nb in range(2):
                                        nc.tensor.matmul(peo[ti][:, nb, :], ones1[:],
                                                         eb2h_sb[:, nb * 512:(nb + 1) * 512],
                                                         start=False, stop=True)
                                        n0 = nb * 512
                                        nc.vector.tensor_scalar(wout[:, n0:n0 + 512],
                                                                peo[ti][:, nb, :],
                                                                ws[tb][:], None, ALU.mult)
                                        # owner (r == e) carries the residual
                                        # through the reduce-scatter
                                        nc.vector.scalar_tensor_tensor(
                                            wout[:, n0:n0 + 512],
                                            xmid[tb][:, n0:n0 + 512], rmask[:, r:r + 1],
                                            wout[:, n0:n0 + 512], ALU.mult, ALU.add)
                                    nc.sync.dma_start(
                                        rs_in[r][tb * 128:(tb + 1) * 128, :], wout[:])
                            nc.gpsimd.collective_compute(
                                "ReduceScatter", ALU.add, replica_groups=rg,
                                ins=[rs_in[r].opt()], outs=[rs_out[r].opt()])
                        else:
                            # last round: nb-major so each column half's RS fires
                            # as soon as that half is done (hides the RS wire)
                            for nb in range(2):
                                n0 = nb * 512
                                for tp in range(2):
                                    peh = [psC.tile([128, 512], FP, tag=f"peh{ti}",
                                                    bufs=1, name=f"peh{ti}")
                                           for ti in range(2)]
                                    for h in range(HB):
                                        for ti in range(2):
                                            tb = tp * 2 + ti
                                            nc.tensor.matmul(
                                                peh[ti][:],
                                                hid[h][:, tb * 128:(tb + 1) * 128],
                                                ew2_sb[h][:, n0:n0 + 512],
                                                start=(h == 0), stop=False)
                                    for ti in range(2):
                                        tb = tp * 2 + ti
                                        nc.tensor.matmul(peh[ti][:], ones1[:],
                                                         eb2h_sb[:, n0:n0 + 512],
                                                         start=False, stop=True)
                                        wouth = pq.tile([128, 512], BF, tag="wouth",
                                                        bufs=2)
                                        nc.vector.tensor_scalar(wouth[:], peh[ti][:],
                                                                ws[tb][:], None, ALU.mult)
                                        nc.vector.scalar_tensor_tensor(
                                            wouth[:], xmid[tb][:, n0:n0 + 512],
                                            rmask[:, r:r + 1], wouth[:],
                                            ALU.mult, ALU.add)
                                        nc.sync.dma_start(
                                            rs3[nb][tb * 128:(tb + 1) * 128, :],
                                            wouth[:])
                                nc.gpsimd.collective_compute(
                                    "ReduceScatter", ALU.add, replica_groups=rg,
                                    ins=[rs3[nb].opt()], outs=[rs3_out[nb].opt()])

                with nc.named_scope("final"):
                    for r in range(3):
                        rsb = pq.tile([128, D], BF, tag="rsb", bufs=2)
                        nc.sync.dma_start(rsb[:], rs_out[r][:])
                        osb = pq.tile([128, D], FP, tag="osb", bufs=1)
                        nc.vector.tensor_copy(osb[:], rsb[:])
                        nc.sync.dma_start(out_d[r * 128:(r + 1) * 128, :], osb[:])
                    for nb in range(2):
                        rsbh = pq.tile([128, 512], BF, tag="rsbh", bufs=2)
                        nc.sync.dma_start(rsbh[:], rs3_out[nb][:])
                        osbh = pq.tile([128, 512], FP, tag="osbh", bufs=2)
                        nc.vector.tensor_copy(osbh[:], rsbh[:])
                        nc.sync.dma_start(
                            out_d[3 * 128:4 * 128, nb * 512:(nb + 1) * 512], osbh[:])

    nc.compile()
    return nc


def host_prep(inputs):
    """Build the 8 per-core input maps from full inputs."""
    import ml_dtypes
    f32 = np.float32
    bf = ml_dtypes.bfloat16
    x = np.ascontiguousarray(np.asarray(inputs["x"], f32).reshape(B * T, D))
    n1 = np.asarray(inputs["norm1_w"], f32)
    n2 = np.asarray(inputs["norm2_w"], f32)
    ipw = np.ascontiguousarray(np.asarray(inputs["in_proj_w"], f32) * n1[:, None]).astype(bf)
    gw = np.ascontiguousarray(np.asarray(inputs["gate_w"], f32) * n2[:, None])
    ew1f = np.asarray(inputs["e_w1"], f32) * n2[None, :, None]
    ew1q = np.clip(ew1f * 64.0, -240.0, 240.0).astype(ml_dtypes.float8_e4m3)
    # [E, k, hid] -> [E, p, i, h, two, m]: k = i*256 + two*128 + p, hid = h*128 + m
    ew1b = ew1q.reshape(E, 4, 2, 128, HID // 128, 128).transpose(0, 3, 1, 4, 2, 5)
    ew1b = np.ascontiguousarray(ew1b.reshape(E, 128, -1))
    ew2b = np.asarray(inputs["e_w2"], f32).astype(bf)
    ident = np.eye(128, dtype=f32)
    ones1 = np.ones((1, 128), f32)
    shared = {
        "ipw": ipw, "ipb": np.asarray(inputs["in_proj_b"], f32),
        "cw": np.ascontiguousarray(np.asarray(inputs["conv_w"], f32)[:, 0, :]),
        "cb": np.asarray(inputs["conv_b"], f32),
        "dtw": np.asarray(inputs["dt_w"], f32).astype(bf),
        "dtb": np.asarray(inputs["dt_b"], f32),
        "bpw": np.asarray(inputs["bp_w"], f32).astype(bf),
        "bpb": np.asarray(inputs["bp_b"], f32),
        "cpw": np.asarray(inputs["cp_w"], f32).astype(bf),
        "cpb": np.asarray(inputs["cp_b"], f32),
        "s2iw": np.asarray(inputs["s2i_w"], f32).astype(bf),
        "s2ib": np.asarray(inputs["s2i_b"], f32),
        "Dp": np.asarray(inputs["D_param"], f32),
        "ow": np.asarray(inputs["out_w"], f32).astype(bf),
        "ob": np.asarray(inputs["out_b"], f32),
        "gw": gw, "gb": np.asarray(inputs["gate_b"], f32),
        "ident": ident, "identb": ident.astype(bf), "ones1": ones1,
    }
    eb1 = np.asarray(inputs["e_b1"], f32)
    eb2 = np.asarray(inputs["e_b2"], f32)
    in_maps = []
    for c in range(N_CORES):
        e, th = c // 2, c % 2
        g0 = th * (B * T // 2) + e * OWN
        if e == 0:
            x_sh = np.concatenate([np.zeros((HALO, D), f32), x[g0:g0 + OWN]])
        else:
            x_sh = x[g0 - HALO:g0 + OWN]
        m = dict(shared)
        m["x_sh"] = np.ascontiguousarray(x_sh)
        m["ew1"] = np.ascontiguousarray(ew1b[e])
        m["eb1"] = np.ascontiguousarray(eb1[e])
        m["ew2"] = np.ascontiguousarray(ew2b[e])
        m["eb2h"] = np.ascontiguousarray(eb2[e])
        esel = np.zeros((128, E), f32)
        esel[:, e] = 1.0
        m["esel"] = esel
        rmask = np.zeros((128, 4), f32)
        rmask[:, e] = 1.0
        m["rmask"] = rmask
        in_maps.append(m)
    return in_maps


def unshard_out(results):
    """results: list of 8 dicts with 'out' [OWN, D]; rows r*128+i of core c
    hold global token (c%2)*2048 + r*512 + (c//2)*128 + i."""
    full = np.empty((B * T, D), np.float32)
    for c in range(N_CORES):
        e, th = c // 2, c % 2
        oc = results[c]["out"]
        for r in range(4):
            full[th * 2048 + r * OWN + e * 128: th * 2048 + r * OWN + (e + 1) * 128] = \
                oc[r * 128:(r + 1) * 128]
    return full.reshape(B, T, D)


_NC_CACHE = {}


def _get_nc():
    if "nc" not in _NC_CACHE:
        _NC_CACHE["nc"] = build(debug_outputs=False)
    return _NC_CACHE["nc"]


def kernel(**inputs) -> np.ndarray:
    """Full-input entry point: shards across 8 NeuronCores, runs the Bass
    kernel SPMD, reassembles the full [2, 2048, 1024] output."""
    import sys, types
    try:  # NTFF profile hook shim (missing antenv.axon_hooks in this image)
        import antenv.axon_hooks  # noqa: F401
    except ImportError:
        try:
            import antenv
            from trn_agent_boot.trn_boot import _ntff_profile_via_ctypes
            mod = types.ModuleType("antenv.axon_hooks")
            try:
                _hook = _ntff_profile_via_ctypes("/opt/axon/libaxon_pjrt.so")
            except Exception:
                _hook = None
            mod.get_axon_ntff_profile_hook = lambda: _hook
            mod.set_axon_ntff_profile_hook = lambda h: None
            sys.modules["antenv.axon_hooks"] = mod
            antenv.axon_hooks = mod
        except Exception:
            pass
    from concourse.bass_utils import run_bass_kernel_spmd

    nc = _get_nc()
    in_maps = host_prep(inputs)
    res = run_bass_kernel_spmd(nc, in_maps, core_ids=list(range(N_CORES)))
    out = unshard_out(res.results)
    return out.astype(np.float32)


# revision 49
# speedup vs baseline: 1.8707x; 1.0084x over previous
"""Bass kernel builder for nn_MixtureOfMambaBlock — 8-core SPMD.

Sharding: tokens 8-way (512/core + 64 halo for conv+scan warmup); mixer fully
local per core (weights replicated, bf16 matmuls; fp32 gating logits to keep
top-2 selection exact). Post-mixer h2 all-gathered in fp8; MoE is expert x
token-half sharded: w1 runs fp8 DoubleRow (weights pre-scaled x64,
host-interleaved k-pairs), w2 in bf16 with both expert weights SBUF-resident.
Weighted expert partials + residual reduce-scattered in bf16 back to token
shards.
"""
import numpy as np
import concourse.bass as bass
import concourse.bacc as bacc
import concourse.mybir as mybir
import concourse.tile as tile

FP = mybir.dt.float32
FR = mybir.dt.float32r
BF = mybir.dt.bfloat16
F8 = mybir.dt.float8e4
W1SCALE = 64.0
AF = mybir.ActivationFunctionType
ALU = mybir.AluOpType

B, T, D = 2, 2048, 1024
S, INNER = 64, 2048
E = 4
HID = 4096
OWN, HALO = 512, 64
NH = OWN + HALO          # 576
KB = D // 128            # 8  d-blocks
MB = INNER // 128        # 16 inner-blocks
HB = HID // 128          # 32 hid-blocks
OTB = OWN // 128         # 4  own-token blocks
N_CORES = 8

INPUT_SPECS = {
    "x_sh": ([NH, D], FP),
    "ipw": ([D, 2 * INNER], BF), "ipb": ([2 * INNER], FP),
    "cw": ([INNER, 3], FP), "cb": ([INNER], FP),
    "dtw": ([INNER, S], BF), "dtb": ([S], FP),
    "bpw": ([INNER, S], BF), "bpb": ([S], FP),
    "cpw": ([INNER, S], BF), "cpb": ([S], FP),
    "s2iw": ([S, INNER], BF), "s2ib": ([INNER], FP),
    "Dp": ([INNER], FP),
    "ow": ([INNER, D], BF), "ob": ([D], FR),
    "gw": ([D, E], FP), "gb": ([E], FR),
    "ew1": ([128, KB // 2 * HID // 128 * 256], F8), "eb1": ([HID], FP),
    "ew2": ([HID, D], BF), "eb2h": ([D], FR),
    "esel": ([128, E], FP),
    "rmask": ([128, 4], FP),
    "ident": ([128, 128], FP),
    "identb": ([128, 128], BF),
    "ones1": ([1, 128], FR),
}


def build(debug_outputs=False):
    nc = bacc.Bacc("TRN2", target_bir_lowering=False, debug=False,
                   num_devices=N_CORES)
    dp = {}
    for name, (shape, dt) in INPUT_SPECS.items():
        dp[name] = nc.dram_tensor(name, shape, dt, kind="ExternalInput")
    out_d = nc.dram_tensor("out", [OWN, D], FP, kind="ExternalOutput")
    dbg = {}
    if debug_outputs:
        dbg["xmid"] = nc.dram_tensor("dbg_xmid", [OWN, D], FP, kind="ExternalOutput")
        dbg["wown"] = nc.dram_tensor("dbg_wown", [OWN, E], FP, kind="ExternalOutput")

    rg = [[0, 2, 4, 6], [1, 3, 5, 7]]

    with tile.TileContext(nc) as tc:
        with (
            tc.tile_pool(name="outer", bufs=1) as po,
            tc.tile_pool(name="dram", bufs=1, space="DRAM") as pdram,
        ):
            # ---------- DRAM bounce buffers for collectives ----------
            # gth layout per tb: [128 d-in-block, kb*128 tok] — matches the
            # transpose PSUM tile directly (1 staging DMA) and lets the MoE
            # load each peer row-block with a single wide DMA.
            gth_in = [pdram.tile([128, KB * 128], F8, name=f"gth_in{t_}")
                      for t_ in range(OTB)]
            gth_out = [pdram.tile([4 * 128, KB * 128], F8, name=f"gth_out{t_}")
                       for t_ in range(OTB)]
            gtw_in = pdram.tile([OWN, E], FP)
            gtw_out = pdram.tile([4 * OWN, E], FP)
            rs_in = [pdram.tile([OWN, D], BF, name=f"rs_in{r}") for r in range(3)]
            rs_out = [pdram.tile([128, D], BF, name=f"rs_out{r}") for r in range(3)]
            # last round splits its reduce-scatter into two column halves so the
            # first half's wire hides under the second half's compute
            rs3 = [pdram.tile([OWN, 512], BF, name=f"rs3_{nb}") for nb in range(2)]
            rs3_out = [pdram.tile([128, 512], BF, name=f"rs3o_{nb}") for nb in range(2)]

            # ---------- constants / small weights (emit all loads up front) ----
            ident = po.tile([128, 128], FP)
            nc.sync.dma_start(ident[:], dp["ident"][:])
            identb = po.tile([128, 128], BF)
            nc.sync.dma_start(identb[:], dp["identb"][:])

            def load_pcol(name, blocks):  # [blocks*128] -> [128, blocks]
                t = po.tile([128, blocks], FP, name=f"{name}_sb")
                nc.sync.dma_start(
                    t[:], dp[name].ap().rearrange("(m p) -> p m", p=128))
                return t

            def load_vec1(name, n):  # [n] -> [n, 1]
                t = po.tile([n, 1], FP, name=f"{name}_sb")
                nc.sync.dma_start(t[:], dp[name].ap().rearrange("(s o) -> s o", o=1))
                return t

            def load_row(name, n, dt_=FP):  # [n] -> [1, n]
                t = po.tile([1, n], dt_, name=f"{name}_sb")
                nc.sync.dma_start(t[:], dp[name].ap().rearrange("(o s) -> o s", o=1))
                return t

            def load_kw(name, pool):  # [2048, 64] -> [128, 16, 64], lhsT slice [:, kb, :]
                t = pool.tile([128, MB, S], BF, name=f"{name}_sb")
                nc.sync.dma_start(t[:], dp[name].ap().rearrange("(kb p) s -> p kb s", p=128))
                return t

            ob_sb = load_row("ob", D, FR)
            gb_sb = load_row("gb", E, FR)
            eb2h_sb = load_row("eb2h", D, FR)
            ones1 = po.tile([1, 128], FR)
            nc.sync.dma_start(ones1[:], dp["ones1"][:])
            ipb_sb = load_pcol("ipb", 32)
            cb_sb = load_pcol("cb", 16)
            cw_sb = po.tile([128, 16, 3], FP)  # [p, m, k]
            nc.sync.dma_start(cw_sb[:], dp["cw"].ap().rearrange("(m p) k -> p m k", p=128))
            dtb_sb = load_vec1("dtb", S)
            bpb_sb = load_vec1("bpb", S)
            cpb_sb = load_vec1("cpb", S)
            s2ib_sb = load_pcol("s2ib", 16)
            Dp_sb = load_pcol("Dp", 16)
            gw_sb = po.tile([128, KB, E], FP)  # [p, kb, e]
            nc.sync.dma_start(gw_sb[:], dp["gw"].ap().rearrange("(kb p) e -> p kb e", p=128))
            esel = po.tile([128, E], FP)
            nc.sync.dma_start(esel[:], dp["esel"][:])
            rmask = po.tile([128, 4], FP)
            nc.sync.dma_start(rmask[:], dp["rmask"][:])
            eb1_sb = load_pcol("eb1", HB)

            # persistent activations
            xmid = [po.tile([128, D], FP, name=f"xmid{t_}", tag=f"xmid{t_}")
                    for t_ in range(OTB)]

            # =======================================================
            # MIXER
            # =======================================================
            with (
                tc.tile_pool(name="mixer", bufs=1) as pm,
                tc.tile_pool(name="mixt", bufs=1) as pt_pool,
            ):
                # pool allocation order matters: tiles that die early (hT, xm,
                # sg, projection weights) go FIRST so their addresses sit at the
                # pool base — the MoE pool's ew1 tiles (allocated first there)
                # land on them and can start loading before outproj finishes.
                hT = [pm.tile([128, NH], BF, name=f"hT{kb}", tag=f"hT{kb}") for kb in range(KB)]
                xm = [pm.tile([128, NH], BF, name=f"xm{m}", tag=f"xm{m}") for m in range(MB)]
                sg = [pm.tile([128, OWN], BF, name=f"sg{m}", tag=f"sg{m}")
                      for m in range(8)]
                dtw_sb = load_kw("dtw", pm)
                bpw_sb = load_kw("bpw", pm)
                cpw_sb = load_kw("cpw", pm)
                s2iw_sb = pm.tile([S, INNER], BF, name="s2iw_sb")
                nc.sync.dma_start(s2iw_sb[:], dp["s2iw"][:])
                # late-freed tiles (used through outproj) at higher addresses
                ow_sb = pm.tile([128, MB, D], BF, name="ow_sb")
                nc.scalar.dma_start(
                    ow_sb[:], dp["ow"].ap().rearrange("(kb p) d -> p kb d", p=128))
                xo = [pm.tile([128, D], FP, name=f"xo{t_}", tag=f"xo{t_}")
                      for t_ in range(OTB)]

                # ---- rmsnorm1 + transpose to hT ----
                # chunks: [64 halo] + 4x [128 own]
                chunks = [(0, HALO, None)] + [
                    (HALO + t_ * 128, 128, t_) for t_ in range(OTB)]
                with nc.named_scope("rms1"), tc.tile_pool(name="ps1", bufs=1, space="PSUM") as psA:
                    for (row0, rows, t_) in chunks:
                        if t_ is None:
                            xt = pt_pool.tile([HALO, D], FP, tag="xt0")
                        else:
                            xt = xo[t_]
                        # gpsimd queue: ahead of the ipw weight chunks, and not
                        # behind the ~20 small constant loads on the sync queue
                        nc.gpsimd.dma_start(xt[:], dp["x_sh"][row0:row0 + rows, :])
                        scr = pt_pool.tile([128, D], FP, tag="scr", bufs=1)
                        sq = pt_pool.tile([128, 1], FP, tag="sq", bufs=2)
                        nc.scalar.activation(scr[0:rows, :], xt[:], AF.Square,
                                             accum_out=sq[0:rows, :])
                        nr = pt_pool.tile([128, 1], FP, tag="nr", bufs=2)
                        nc.vector.tensor_scalar(nr[0:rows, :], sq[0:rows, :], 1.0 / D,
                                                1e-6, ALU.mult, ALU.add)
                        nc.scalar.sqrt(nr[0:rows, :], nr[0:rows, :])
                        nc.vector.reciprocal(nr[0:rows, :], nr[0:rows, :])
                        h_t = pt_pool.tile([128, D], BF, tag="htb", bufs=2)
                        nc.vector.tensor_scalar(h_t[0:rows, :], xt[:], nr[0:rows, :],
                                                None, ALU.mult)
                        for kb in range(KB):
                            ptr = psA.tile([128, 128], BF, tag="ptr", bufs=2)
                            nc.tensor.transpose(ptr[:, 0:rows],
                                                h_t[0:rows, kb * 128:(kb + 1) * 128],
                                                identb[0:rows, 0:rows])
                            nc.vector.tensor_copy(hT[kb][:, row0:row0 + rows],
                                                  ptr[:, 0:rows])

                # ---- in_proj (x_main half) + conv + silu ----
                with nc.named_scope("in_proj"), tc.tile_pool(name="ps2", bufs=1, space="PSUM") as psA:
                    for q in range(4):
                        wq = []
                        for kb in range(KB):
                            wt = pt_pool.tile([128, 512], BF, tag=f"wip{kb}", bufs=2,
                                              name=f"wip{kb}")
                            nc.gpsimd.dma_start(
                                wt[:], dp["ipw"][kb * 128:(kb + 1) * 128,
                                                 q * 512:(q + 1) * 512])
                            wq.append(wt)
                        for mi in range(4):
                            m = q * 4 + mi
                            xzp = pt_pool.tile([128, NH + 2], FP, tag="xzp", bufs=2)
                            nc.vector.memset(xzp[:, 0:2], 0.0)
                            for n0, nw in ((0, 512), (512, NH - 512)):
                                px = psA.tile([128, 512], FP, tag="px", bufs=2)
                                for kb in range(KB):
                                    nc.tensor.matmul(px[:, 0:nw],
                                                     wq[kb][:, mi * 128:(mi + 1) * 128],
                                                     hT[kb][:, n0:n0 + nw],
                                                     start=(kb == 0), stop=(kb == KB - 1))
                                nc.scalar.activation(xzp[:, 2 + n0:2 + n0 + nw], px[:, 0:nw],
                                                     AF.Identity, bias=ipb_sb[:, m:m + 1])
                            cv = pt_pool.tile([128, NH], FP, tag="cv", bufs=2)
                            nc.vector.tensor_scalar(cv[:], xzp[:, 0:NH], cw_sb[:, m, 0:1],
                                                    None, ALU.mult)
                            nc.vector.scalar_tensor_tensor(cv[:], xzp[:, 1:1 + NH],
                                                           cw_sb[:, m, 1:2], cv[:],
                                                           ALU.mult, ALU.add)
                            nc.vector.scalar_tensor_tensor(cv[:], xzp[:, 2:2 + NH],
                                                           cw_sb[:, m, 2:3], cv[:],
                                                           ALU.mult, ALU.add)
                            sgc = pt_pool.tile([128, NH], BF, tag="sgc", bufs=2)
                            nc.scalar.activation(sgc[:], cv[:], AF.Sigmoid, bias=cb_sb[:, m:m + 1])
                            nc.vector.scalar_tensor_tensor(xm[m][:], cv[:], cb_sb[:, m:m + 1],
                                                           sgc[:], ALU.add, ALU.mult)

                # ---- dt/B/C projections (emitted before gate MMs; feed scan) ----
                with nc.named_scope("scan"), tc.tile_pool(name="ps3", bufs=1, space="PSUM") as psA:
                    dt_t = pt_pool.tile([S, NH], FP, tag="dt")
                    a_t = pt_pool.tile([S, NH], FP, tag="a")
                    b_t = pt_pool.tile([S, NH], FP, tag="b")
                    c_t = pt_pool.tile([S, NH], FP, tag="c")
                    for n0, nw in ((0, 512), (512, NH - 512)):
                        for wsb, bias_sb, dst, fn in (
                            (dtw_sb, dtb_sb, dt_t, AF.Sigmoid),
                            (cpw_sb, cpb_sb, c_t, AF.Identity),
                        ):
                            pz = psA.tile([S, 512], FP, tag="pz", bufs=2)
                            for kb in range(MB):
                                nc.tensor.matmul(pz[:, 0:nw], wsb[:, kb, :],
                                                 xm[kb][:, n0:n0 + nw],
                                                 start=(kb == 0), stop=(kb == MB - 1))
                            nc.scalar.activation(dst[:, n0:n0 + nw], pz[:, 0:nw], fn,
                                                 bias=bias_sb[:])
                        # b needs dt -> separate pass
                        pz = psA.tile([S, 512], FP, tag="pz", bufs=2)
                        for kb in range(MB):
                            nc.tensor.matmul(pz[:, 0:nw], bpw_sb[:, kb, :],
                                             xm[kb][:, n0:n0 + nw],
                                             start=(kb == 0), stop=(kb == MB - 1))
                        nc.vector.scalar_tensor_tensor(b_t[:, n0:n0 + nw], pz[:, 0:nw],
                                                       bpb_sb[:], dt_t[:, n0:n0 + nw],
                                                       ALU.add, ALU.mult)
                    # scan runs on the vector engine while the tensor engine
                    # works through the gate-projection matmuls below
                    nc.vector.tensor_scalar(a_t[:], dt_t[:], -1.0, 1.0,
                                            ALU.mult, ALU.add)
                    st_t = pt_pool.tile([S, NH], FP, tag="st")
                    nc.vector.tensor_tensor_scan(st_t[:], a_t[:], b_t[:], 0.0,
                                                 ALU.mult, ALU.add)
                    y_t = pt_pool.tile([S, OWN], FP, tag="yt", name="y_t")
                    nc.vector.tensor_mul(y_t[:], c_t[:, HALO:NH], st_t[:, HALO:NH])

                # ---- gate half of in_proj, first 8 m: emitted NOW so the
                # tensor queue has work while the (vector-engine) scan runs ----
                def load_wqg(q):
                    wqg = []
                    for kb in range(KB):
                        wt = pt_pool.tile([128, 512], BF, tag=f"wip{kb}", bufs=2,
                                          name=f"wipg{kb}_{q}")
                        nc.gpsimd.dma_start(
                            wt[:], dp["ipw"][kb * 128:(kb + 1) * 128,
                                             2048 + q * 512:2048 + (q + 1) * 512])
                        wqg.append(wt)
                    return wqg

                def gate_mm(psB, wqg, m):
                    mi = m % 4
                    pg = psB.tile([128, 512], FP, tag="pg", bufs=2)
                    for kb in range(KB):
                        nc.tensor.matmul(pg[:], wqg[kb][:, mi * 128:(mi + 1) * 128],
                                         hT[kb][:, HALO:NH],
                                         start=(kb == 0), stop=(kb == KB - 1))
                    return pg

                sgi_pre = {}
                with nc.named_scope("gateproj"), tc.tile_pool(name="ps4", bufs=1, space="PSUM") as psB:
                    for q in range(2):
                        wqg = load_wqg(q)
                        for mi in range(4):
                            m = q * 4 + mi
                            pg = gate_mm(psB, wqg, m)
                            nc.scalar.activation(sg[m][:], pg[:], AF.Sigmoid,
                                                 bias=ipb_sb[:, MB + m:MB + m + 1])
                    # two more gate projections emitted here: covers the tail of
                    # the scan+LN vector chain before the LN transposes
                    wqg2 = load_wqg(2)
                    for m in (8, 9):
                        pg = gate_mm(psB, wqg2, m)
                        sg_m = pt_pool.tile([128, OWN], BF, tag="sgi", bufs=3)
                        nc.scalar.activation(sg_m[:], pg[:], AF.Sigmoid,
                                             bias=ipb_sb[:, MB + m:MB + m + 1])
                        sgi_pre[m] = sg_m

                # ---- layernorm over S ----
                with nc.named_scope("scanln"), tc.tile_pool(name="ps5", bufs=1, space="PSUM") as psA:
                    yln = pt_pool.tile([S, OWN], BF, tag="a", name="yln")
                    for i in range(OTB):
                        ptr = psA.tile([128, 128], FP, tag="ptr", bufs=2)
                        nc.tensor.transpose(ptr[:, 0:S], y_t[:, i * 128:(i + 1) * 128],
                                            ident[0:S, 0:S])
                        yT = pt_pool.tile([128, S], FP, tag="yT", bufs=2)
                        nc.vector.tensor_copy(yT[:], ptr[:, 0:S])
                        mu = pt_pool.tile([128, 1], FP, tag="mu", bufs=2)
                        nc.vector.tensor_reduce(mu[:], yT[:], mybir.AxisListType.X, ALU.add)
                        nc.vector.tensor_scalar_mul(mu[:], mu[:], 1.0 / S)
                        xc = pt_pool.tile([128, S], FP, tag="xc", bufs=2)
                        nc.vector.tensor_scalar_sub(xc[:], yT[:], mu[:])
                        scr2 = pt_pool.tile([128, S], FP, tag="scr2", bufs=2)
                        vv = pt_pool.tile([128, 1], FP, tag="vv", bufs=2)
                        nc.scalar.activation(scr2[:], xc[:], AF.Square, accum_out=vv[:])
                        nc.vector.tensor_scalar(vv[:], vv[:], 1.0 / S, 1e-5, ALU.mult, ALU.add)
                        nc.scalar.sqrt(vv[:], vv[:])
                        nc.vector.reciprocal(vv[:], vv[:])
                        xcb = pt_pool.tile([128, S], BF, tag="xcb", bufs=2)
                        nc.vector.tensor_scalar_mul(xcb[:], xc[:], vv[:])
                        ptr2 = psA.tile([128, 128], BF, tag="ptr2", bufs=2)
                        nc.tensor.transpose(ptr2[0:S, :], xcb[:], identb[:])
                        nc.vector.tensor_copy(yln[:, i * 128:(i + 1) * 128], ptr2[0:S, :])

                # ---- s2i + pre_out assembly (gate m>=8 computed inline) ----
                with nc.named_scope("premix"), tc.tile_pool(name="ps6", bufs=1, space="PSUM") as psA:
                    pre = []
                    wqg_box = [wqg2]
                    sgq = dict(sgi_pre)

                    def emit_gate_inline(mm):
                        if mm == 12:
                            wqg_box[0] = load_wqg(3)
                        pg = gate_mm(psA, wqg_box[0], mm)
                        t = pt_pool.tile([128, OWN], BF, tag="sgi", bufs=3)
                        nc.scalar.activation(t[:], pg[:], AF.Sigmoid,
                                             bias=ipb_sb[:, MB + mm:MB + mm + 1])
                        sgq[mm] = t

                    # inline gates run two iterations ahead of the yln-dependent
                    # s2i matmuls so the tensor queue never stalls on the LN tail
                    emit_gate_inline(10)
                    for m in range(MB):
                        if 9 <= m <= 13:
                            emit_gate_inline(m + 2)
                        sg_m = sgq[m] if m >= 8 else sg[m]
                        ps = psA.tile([128, 512], FP, tag="ps", bufs=2)
                        nc.tensor.matmul(ps[:], s2iw_sb[:, m * 128:(m + 1) * 128], yln[:],
                                         start=True, stop=True)
                        # Dp*xm + s2ib on the scalar engine; 2 vector ops total
                        tmp = pt_pool.tile([128, OWN], FP, tag="tmp", bufs=2)
                        nc.scalar.activation(tmp[:], xm[m][:, HALO:NH], AF.Identity,
                                             bias=s2ib_sb[:, m:m + 1],
                                             scale=Dp_sb[:, m:m + 1])
                        nc.vector.tensor_add(tmp[:], tmp[:], ps[:])
                        pre_m = pm.tile([128, OWN], BF, name=f"pre{m}", tag=f"pre{m}")
                        nc.vector.tensor_mul(pre_m[:], tmp[:], sg_m[:])
                        pre.append(pre_m)

                # ---- per-tb: out projection + residual + rms2 + h2T + gating + gather ----
                # emission interleave: po2 matmuls of tb+1 are queued before the
                # gating chain of tb, so the tensor engine has work while the
                # rms2/transpose chain for tb runs on scalar/vector
                with nc.named_scope("outproj"), tc.tile_pool(name="ps7", bufs=1, space="PSUM") as psA:
                    def emit_po2(tb):
                        po2 = psA.tile([128, 2, 512], FP, tag="po2", bufs=2)
                        for kb in range(MB):
                            for nb in range(2):
                                nc.tensor.matmul(po2[:, nb, :],
                                                 pre[kb][:, tb * 128:(tb + 1) * 128],
                                                 ow_sb[:, kb, nb * 512:(nb + 1) * 512],
                                                 start=(kb == 0), stop=False)
                        for nb in range(2):
                            nc.tensor.matmul(po2[:, nb, :], ones1[:],
                                             ob_sb[:, nb * 512:(nb + 1) * 512],
                                             start=False, stop=True)
                            nc.vector.tensor_add(xmid[tb][:, nb * 512:(nb + 1) * 512],
                                                 po2[:, nb, :],
                                                 xo[tb][:, nb * 512:(nb + 1) * 512])

                    def emit_gate(tb):
                        # rms2 for this tb
                        scr = pt_pool.tile([128, D], FP, tag="scr", bufs=1)
                        sq = pt_pool.tile([128, 1], FP, tag="sq", bufs=2)
                        nc.scalar.activation(scr[:], xmid[tb][:], AF.Square, accum_out=sq[:])
                        nr = pt_pool.tile([128, 1], FP, tag="nr", bufs=2)
                        nc.vector.tensor_scalar(nr[:], sq[:], 1.0 / D, 1e-6, ALU.mult, ALU.add)
                        nc.scalar.sqrt(nr[:], nr[:])
                        nc.vector.reciprocal(nr[:], nr[:])
                        h2 = pt_pool.tile([128, D], FP, tag="h2", bufs=1, name="h2")
                        nc.vector.tensor_scalar(h2[:], xmid[tb][:], nr[:], None, ALU.mult)
                        # gating logits must be fp32: bf16 logits flip top-2
                        # selections vs the reference on near-ties (~0.15 abs
                        # error per flipped token). All 8 transposes batch into
                        # one PSUM tile, then 2 wide vector copies + 1 staging
                        # DMA — avoids per-kb tensor<->vector ping-pong.
                        pl = psA.tile([128, E], FP, tag="pl", bufs=2)
                        ptr8 = psA.tile([128, KB * 128], FP, tag="ptr8", bufs=1)
                        for kb in range(KB):
                            nc.tensor.transpose(ptr8[:, kb * 128:(kb + 1) * 128],
                                                h2[:, kb * 128:(kb + 1) * 128],
                                                ident[:])
                        h2T_t = pt_pool.tile([128, KB * 128], FP, tag="h2T", bufs=1)
                        nc.vector.tensor_copy(h2T_t[:], ptr8[:])
                        h2T_8 = pt_pool.tile([128, KB * 128], F8, tag="h2T8", bufs=2)
                        nc.vector.tensor_copy(h2T_8[:], ptr8[:])
                        nc.sync.dma_start(gth_in[tb][:], h2T_8[:])
                        for kb in range(KB):
                            nc.tensor.matmul(pl[:], h2T_t[:, kb * 128:(kb + 1) * 128],
                                             gw_sb[:, kb, :],
                                             start=(kb == 0), stop=False)
                        nc.tensor.matmul(pl[:], ones1[:], gb_sb[:], start=False, stop=True)
                        # top-2-of-4 gating
                        m1 = pt_pool.tile([128, 1], FP, tag="m1", bufs=2)
                        nc.vector.tensor_reduce(m1[:], pl[:], mybir.AxisListType.X, ALU.max)
                        eq1 = pt_pool.tile([128, E], FP, tag="eq1", bufs=2)
                        nc.vector.tensor_scalar(eq1[:], pl[:], m1[:], None, ALU.is_equal)
                        msk = pt_pool.tile([128, E], FP, tag="msk", bufs=2)
                        nc.vector.scalar_tensor_tensor(msk[:], eq1[:], -1e30, pl[:],
                                                       ALU.mult, ALU.add)
                        m2 = pt_pool.tile([128, 1], FP, tag="m2", bufs=2)
                        nc.vector.tensor_reduce(m2[:], msk[:], mybir.AxisListType.X, ALU.max)
                        eq2 = pt_pool.tile([128, E], FP, tag="eq2", bufs=2)
                        nc.vector.tensor_scalar(eq2[:], msk[:], m2[:], None, ALU.is_equal)
                        dd = pt_pool.tile([128, 1], FP, tag="dd", bufs=2)
                        nc.vector.tensor_sub(dd[:], m2[:], m1[:])
                        p2 = pt_pool.tile([128, 1], FP, tag="p2", bufs=2)
                        nc.scalar.activation(p2[:], dd[:], AF.Sigmoid)
                        p1b = pt_pool.tile([128, 1], FP, tag="p1b", bufs=2)
                        nc.scalar.activation(p1b[:], p2[:], AF.Identity, bias=1.0, scale=-1.0)
                        wv = pt_pool.tile([128, E], FP, tag="wv", bufs=2)
                        nc.vector.tensor_scalar(wv[:], eq1[:], p1b[:], None, ALU.mult)
                        nc.vector.scalar_tensor_tensor(wv[:], eq2[:], p2[:], wv[:],
                                                       ALU.mult, ALU.add)
                        nc.sync.dma_start(gtw_in[tb * 128:(tb + 1) * 128, :], wv[:])
                        if debug_outputs:
                            nc.sync.dma_start(dbg["wown"][tb * 128:(tb + 1) * 128, :], wv[:])
                            nc.sync.dma_start(dbg["xmid"][tb * 128:(tb + 1) * 128, :],
                                              xmid[tb][:])
                        nc.gpsimd.collective_compute(
                            "AllGather", ALU.bypass, replica_groups=rg,
                            ins=[gth_in[tb].opt()], outs=[gth_out[tb].opt()])

                    for tb in range(OTB):
                        emit_po2(tb)
                        emit_gate(tb)
                    with nc.named_scope("gatherw"):
                        nc.gpsimd.collective_compute(
                            "AllGather", ALU.bypass, replica_groups=rg,
                            ins=[gtw_in.opt()], outs=[gtw_out.opt()])

            # =======================================================
            # MoE (full expert per core, token-half group of 4)
            # =======================================================
            with (
                tc.tile_pool(name="moe", bufs=1) as pq,
                tc.tile_pool(name="psC", bufs=1, space="PSUM") as psC,
            ):
                # expert weights resident in SBUF for all 4 rounds.
                # w1 runs in fp8 DoubleRow: ew1 arrives pre-scaled by W1SCALE and
                # host-interleaved to [p, h, two, m] per k-pair so each LDWEIGHTS
                # slice [128, 2, 128] is contiguous (strided pair dims fault the PE).
                ew1_sb = [pq.tile([128, HB, 2, 128], F8, name=f"ew1_{i}", tag=f"ew1_{i}")
                          for i in range(KB // 2)]
                for i in range(KB // 2):
                    nc.scalar.dma_start(
                        ew1_sb[i][:], dp["ew1"][:, i * (HB * 256):(i + 1) * (HB * 256)])
                ew2_sb = [pq.tile([128, D], BF, name=f"ew2_{j}", tag=f"ew2_{j}")
                          for j in range(HB)]
                for j in range(HB):
                    nc.scalar.dma_start(ew2_sb[j][:], dp["ew2"][j * 128:(j + 1) * 128, :])

                with nc.named_scope("moe"):
                    for r in range(4):
                        # one wide DMA per peer token-block: src rows are the
                        # peer's [128, kb*128] section, 1KB contiguous lines
                        h2r = pq.tile([128, KB, OWN], F8, tag="h2r", bufs=2)
                        for t_ in range(OTB):
                            nc.gpsimd.dma_start(
                                h2r[:, :, t_ * 128:(t_ + 1) * 128],
                                gth_out[t_][r * 128:(r + 1) * 128, :]
                                .rearrange("p (kb j) -> p kb j", j=128))
                        hid = []
                        for h in range(HB):
                            ph = psC.tile([128, 512], FP, tag="ph", bufs=2)
                            for i in range(KB // 2):
                                nc.tensor.matmul(ph[:], ew1_sb[i][:, h, :, :],
                                                 h2r[:, 2 * i:2 * i + 2, :],
                                                 start=(i == 0), stop=(i == KB // 2 - 1),
                                                 perf_mode=mybir.MatmulPerfMode.DoubleRow)
                            ht = pq.tile([128, OWN], BF, tag=f"hid{h}", bufs=1)
                            nc.scalar.activation(ht[:], ph[:], AF.Gelu, bias=eb1_sb[:, h:h + 1],
                                                 scale=1.0 / W1SCALE)
                            hid.append(ht)
                        # per-token weight for this core's expert
                        wvr = pq.tile([128, OTB, E], FP, tag="wvr", bufs=2)
                        nc.sync.dma_start(
                            wvr[:], gtw_out[r * OWN:(r + 1) * OWN, :]
                            .rearrange("(tb p) e -> p tb e", p=128))
                        ws = []
                        for tb in range(OTB):
                            wm_t = pq.tile([128, E], FP, tag="wm", bufs=2)
                            nc.vector.tensor_mul(wm_t[:], wvr[:, tb, :], esel[:])
                            ws_t = pq.tile([128, 1], FP, tag=f"ws{tb}", bufs=2)
                            nc.vector.tensor_reduce(ws_t[:], wm_t[:], mybir.AxisListType.X,
                                                    ALU.add)
                            ws.append(ws_t)
                        if r < 3:
                            # w2: token-block pairs keep PSUM <= 6 banks
                            for tp in range(2):
                                peo = [psC.tile([128, 2, 512], FP, tag=f"peo{ti}", bufs=1,
                                                name=f"peo{ti}") for ti in range(2)]
                                for h in range(HB):
                                    for ti in range(2):
                                        tb = tp * 2 + ti
                                        for nb in range(2):
                                            nc.tensor.matmul(
                                                peo[ti][:, nb, :],
                                                hid[h][:, tb * 128:(tb + 1) * 128],
                                                ew2_sb[h][:, nb * 512:(nb + 1) * 512],
                                                start=(h == 0), stop=False)
                                for ti in range(2):
                                    tb = tp * 2 + ti
                                    wout = pq.tile([128, D], BF, tag="wout", bufs=2)
                                    for nb in range(2):
                                        nc.tensor.matmul(peo[ti][:, nb, :], ones1[:],
                                                         eb2h_sb[:, nb * 512:(nb + 1) * 512],
                                                         start=False, stop=True)
                                        n0 = nb * 512
                                        nc.vector.tensor_scalar(wout[:, n0:n0 + 512],
                                                                peo[ti][:, nb, :],
                                                                ws[tb][:], None, ALU.mult)
                                        # owner (r == e) carries the residual
                                        # through the reduce-scatter
                                        nc.vector.scalar_tensor_tensor(
                                            wout[:, n0:n0 + 512],
                                            xmid[tb][:, n0:n0 + 512], rmask[:, r:r + 1],
                                            wout[:, n0:n0 + 512], ALU.mult, ALU.add)
                                    nc.sync.dma_start(
                                        rs_in[r][tb * 128:(tb + 1) * 128, :], wout[:])
                            nc.gpsimd.collective_compute(
                                "ReduceScatter", ALU.add, replica_groups=rg,
                                ins=[rs_in[r].opt()], outs=[rs_out[r].opt()])
                        else:
                            # last round: nb-major so each column half's RS fires
                            # as soon as that half is done (hides the RS wire)
                            for nb in range(2):
                                n0 = nb * 512
                                for tp in range(2):
                                    peh = [psC.tile([128, 512], FP, tag=f"peh{ti}",
                                                    bufs=1, name=f"peh{ti}")
                                           for ti in range(2)]
                                    for h in range(HB):
                                        for ti in range(2):
                                            tb = tp * 2 + ti
                                            nc.tensor.matmul(
                                                peh[ti][:],
                                                hid[h][:, tb * 128:(tb + 1) * 128],
                                                ew2_sb[h][:, n0:n0 + 512],
                                                start=(h == 0), stop=False)
                                    for ti in range(2):
                                        tb = tp * 2 + ti
                                        nc.tensor.matmul(peh[ti][:], ones1[:],
                                                         eb2h_sb[:, n0:n0 + 512],
                                                         start=False, stop=True)
                                        wouth = pq.tile([128, 512], BF, tag="wouth",
                                                        bufs=2)
                                        nc.vector.tensor_scalar(wouth[:], peh[ti][:],
                                                                ws[tb][:], None, ALU.mult)
                                        nc.vector.scalar_tensor_tensor(
                                            wouth[:], xmid[tb][:, n0:n0 + 512],
                                            rmask[:, r:r + 1], wouth[:],
                                            ALU.mult, ALU.add)
                                        nc.sync.dma_start(
                                            rs3[nb][tb * 128:(tb + 1) * 128, :],
                                            wouth[:])
                                nc.gpsimd.collective_compute(
                                    "ReduceScatter", ALU.add, replica_groups=rg,
                                    ins=[rs3[nb].opt()], outs=[rs3_out[nb].opt()])

                with nc.named_scope("final"):
                    for r in range(3):
                        rsb = pq.tile([128, D], BF, tag="rsb", bufs=2)
                        nc.sync.dma_start(rsb[:], rs_out[r][:])
                        osb = pq.tile([128, D], FP, tag="osb", bufs=1)
                        nc.vector.tensor_copy(osb[:], rsb[:])
                        nc.sync.dma_start(out_d[r * 128:(r + 1) * 128, :], osb[:])
                    for nb in range(2):
                        rsbh = pq.tile([128, 512], BF, tag="rsbh", bufs=2)
                        nc.sync.dma_start(rsbh[:], rs3_out[nb][:])
                        osbh = pq.tile([128, 512], FP, tag="osbh", bufs=2)
                        nc.vector.tensor_copy(osbh[:], rsbh[:])
                        nc.sync.dma_start(
                            out_d[3 * 128:4 * 128, nb * 512:(nb + 1) * 512], osbh[:])

    nc.compile()
    return nc


def host_prep(inputs):
    """Build the 8 per-core input maps from full inputs."""
    import ml_dtypes
    f32 = np.float32
    bf = ml_dtypes.bfloat16
    x = np.ascontiguousarray(np.asarray(inputs["x"], f32).reshape(B * T, D))
    n1 = np.asarray(inputs["norm1_w"], f32)
    n2 = np.asarray(inputs["norm2_w"], f32)
    ipw = np.ascontiguousarray(np.asarray(inputs["in_proj_w"], f32) * n1[:, None]).astype(bf)
    gw = np.ascontiguousarray(np.asarray(inputs["gate_w"], f32) * n2[:, None])
    ew1f = np.asarray(inputs["e_w1"], f32) * n2[None, :, None]
    ew1q = np.clip(ew1f * 64.0, -240.0, 240.0).astype(ml_dtypes.float8_e4m3)
    # [E, k, hid] -> [E, p, i, h, two, m]: k = i*256 + two*128 + p, hid = h*128 + m
    ew1b = ew1q.reshape(E, 4, 2, 128, HID // 128, 128).transpose(0, 3, 1, 4, 2, 5)
    ew1b = np.ascontiguousarray(ew1b.reshape(E, 128, -1))
    ew2b = np.asarray(inputs["e_w2"], f32).astype(bf)
    ident = np.eye(128, dtype=f32)
    ones1 = np.ones((1, 128), f32)
    shared = {
        "ipw": ipw, "ipb": np.asarray(inputs["in_proj_b"], f32),
        "cw": np.ascontiguousarray(np.asarray(inputs["conv_w"], f32)[:, 0, :]),
        "cb": np.asarray(inputs["conv_b"], f32),
        "dtw": np.asarray(inputs["dt_w"], f32).astype(bf),
        "dtb": np.asarray(inputs["dt_b"], f32),
        "bpw": np.asarray(inputs["bp_w"], f32).astype(bf),
        "bpb": np.asarray(inputs["bp_b"], f32),
        "cpw": np.asarray(inputs["cp_w"], f32).astype(bf),
        "cpb": np.asarray(inputs["cp_b"], f32),
        "s2iw": np.asarray(inputs["s2i_w"], f32).astype(bf),
        "s2ib": np.asarray(inputs["s2i_b"], f32),
        "Dp": np.asarray(inputs["D_param"], f32),
        "ow": np.asarray(inputs["out_w"], f32).astype(bf),
        "ob": np.asarray(inputs["out_b"], f32),
        "gw": gw, "gb": np.asarray(inputs["gate_b"], f32),
        "ident": ident, "identb": ident.astype(bf), "ones1": ones1,
    }
    eb1 = np.asarray(inputs["e_b1"], f32)
    eb2 = np.asarray(inputs["e_b2"], f32)
    in_maps = []
    for c in range(N_CORES):
        e, th = c // 2, c % 2
        g0 = th * (B * T // 2) + e * OWN
        if e == 0:
            x_sh = np.concatenate([np.zeros((HALO, D), f32), x[g0:g0 + OWN]])
        else:
            x_sh = x[g0 - HALO:g0 + OWN]
        m = dict(shared)
        m["x_sh"] = np.ascontiguousarray(x_sh)
        m["ew1"] = np.ascontiguousarray(ew1b[e])
        m["eb1"] = np.ascontiguousarray(eb1[e])
        m["ew2"] = np.ascontiguousarray(ew2b[e])
        m["eb2h"] = np.ascontiguousarray(eb2[e])
        esel = np.zeros((128, E), f32)
        esel[:, e] = 1.0
        m["esel"] = esel
        rmask = np.zeros((128, 4), f32)
        rmask[:, e] = 1.0
        m["rmask"] = rmask
        in_maps.append(m)
    return in_maps


def unshard_out(results):
    """results: list of 8 dicts with 'out' [OWN, D]; rows r*128+i of core c
    hold global token (c%2)*2048 + r*512 + (c//2)*128 + i."""
    full = np.empty((B * T, D), np.float32)
    for c in range(N_CORES):
        e, th = c // 2, c % 2
        oc = results[c]["out"]
        for r in range(4):
            full[th * 2048 + r * OWN + e * 128: th * 2048 + r * OWN + (e + 1) * 128] = \
                oc[r * 128:(r + 1) * 128]
    return full.reshape(B, T, D)


_NC_CACHE = {}


def _get_nc():
    if "nc" not in _NC_CACHE:
        _NC_CACHE["nc"] = build(debug_outputs=False)
    return _NC_CACHE["nc"]


def kernel(**inputs) -> np.ndarray:
    """Full-input entry point: shards across 8 NeuronCores, runs the Bass
    kernel SPMD, reassembles the full [2, 2048, 1024] output."""
    import sys, types
    try:  # NTFF profile hook shim (missing antenv.axon_hooks in this image)
        import antenv.axon_hooks  # noqa: F401
    except ImportError:
        try:
            import antenv
            from trn_agent_boot.trn_boot import _ntff_profile_via_ctypes
            mod = types.ModuleType("antenv.axon_hooks")
            try:
                _hook = _ntff_profile_via_ctypes("/opt/axon/libaxon_pjrt.so")
            except Exception:
                _hook = None
            mod.get_axon_ntff_profile_hook = lambda: _hook
            mod.set_axon_ntff_profile_hook = lambda h: None
            sys.modules["antenv.axon_hooks"] = mod
            antenv.axon_hooks = mod
        except Exception:
            pass
    from concourse.bass_utils import run_bass_kernel_spmd

    nc = _get_nc()
    in_maps = host_prep(inputs)
    res = run_bass_kernel_spmd(nc, in_maps, core_ids=list(range(N_CORES)))
    out = unshard_out(res.results)
    return out.astype(np.float32)
